# revision 1
# baseline (speedup 1.0000x reference)
"""Longformer layer stack (4 layers, sliding-window attention) on 8 TRN2 cores.

Sharding: data-parallel over batch (2) x sequence-parallel (4 blocks of 1024
tokens). Each core computes its sequence block; the banded attention needs a
W=256 token halo, exchanged between neighboring blocks with an AllGather after
each layer (layers 0-2). Residual stream kept transposed ([dmodel, tokens]) in
float32r; attention probs/values in bf16.

v2: the halo exchange is overlapped with compute (boundary-first FFN/LN2 kicks
the collective early; the next layer projects its own-token QKV and runs the
interior attention chunks while the collective is in flight, deferring only
halo K/V projection and the two edge chunks). Softmax normalization is batched
per chunk: denominators staged to SBUF, one reciprocal, a PE select-matmul
broadcast, and one multiply per (chunk, feature-tile) -- replacing per-head
reciprocal/partition-broadcast chains. LayerNorm uses a fused Rsqrt and a
full-width LN1.
"""
import sys

sys.path.insert(0, '/opt/trn_rl_repo')

import numpy as np
import ml_dtypes

import concourse.bass as bass
import concourse.mybir as mybir
import concourse.tile as tile
from concourse import bacc
from concourse import bass_utils

F32 = mybir.dt.float32
F32R = mybir.dt.float32r
BF16 = mybir.dt.bfloat16
I32 = mybir.dt.int32
AF = mybir.ActivationFunctionType
ALU = mybir.AluOpType

NH = 12          # heads
DH = 64          # head dim
HD = 768         # model dim
FF = 3072        # ffn dim
W = 256          # one-sided window
L = 4            # layers
B = 2
S = 4096
EPS = 1e-12
N_CORES = 8
T_OWN = 1024     # tokens per core
T_EXT = 1536     # with halos
FT = 6           # model-dim 128-tiles
FFT = 24         # ffn-dim 128-tiles
NCH = 4          # local chunks of 256 queries
P = 128


def _two_seg(ap2d, o1, o2, n):
    """[128, 2, n] view over cols {o1:o1+n} U {o2:o2+n} of a [128, N] AP."""
    base = ap2d[:, o1:o1 + n]
    return bass.AP(tensor=base.tensor, offset=base.offset,
                   ap=[base.ap[0], [o2 - o1, 2], [1, n]])


LN_BCAST_GPSIMD = False
FFN_BF16 = False


def _ln_T(nc, sb, ps, r_aps, out_aps, ones_r, ones_row, eps_sb, s_ap, b_ap,
          ncols, stage_sbuf=False):
    """LayerNorm over the partition (feature) axis of transposed tiles.

    r_aps: list of FT fp32r SBUF APs [128, ncols] (input; normalized IN PLACE
    up to the final scale/bias which lands in out_aps). ncols in {512, 1024}.
    out_aps[ft] <- LN(r)*s + b. s_ap/b_ap: [128, FT] sbuf. mu/rstd broadcasts
    run as K=1 outer-product matmuls on the PE (gpsimd stays free for the
    collective); stage_sbuf copies them out of PSUM so the banks free early.
    """
    nseg = ncols // 512
    sxs = [ps.tile([1, 512], F32, tag=f"lnsx{j}", name=f"lnsx{j}")
           for j in range(nseg)]
    sqs = [ps.tile([1, 512], F32, tag=f"lnsq{j}", name=f"lnsq{j}")
           for j in range(nseg)]
    for ft in range(FT):
        sqt = sb.tile([P, ncols], F32R, tag="lnsqt", name="lnsqt", bufs=2)
        nc.scalar.activation(sqt[:], r_aps[ft], AF.Square)
        for j in range(nseg):
            cs = slice(j * 512, (j + 1) * 512)
            nc.tensor.matmul(sxs[j][:], ones_r[:, 0:1], r_aps[ft][:, cs],
                             start=(ft == 0), stop=(ft == FT - 1))
            nc.tensor.matmul(sqs[j][:], ones_r[:, 0:1], sqt[:, cs],
                             start=(ft == 0), stop=(ft == FT - 1))
    mu = sb.tile([1, ncols], F32R, tag="lnmu", name="lnmu")
    var = sb.tile([1, ncols], F32, tag="lnvar", name="lnvar")
    musq = sb.tile([1, ncols], F32, tag="lnmusq", name="lnmusq")
    rstd = sb.tile([1, ncols], F32R, tag="lnrstd", name="lnrstd")
    for j in range(nseg):
        cs = slice(j * 512, (j + 1) * 512)
        nc.scalar.activation(mu[:, cs], sxs[j][:], AF.Identity, scale=1.0 / HD)
    nc.vector.tensor_tensor(musq[:], mu[:], mu[:], op=ALU.mult)
    for j in range(nseg):
        cs = slice(j * 512, (j + 1) * 512)
        nc.vector.scalar_tensor_tensor(out=var[:, cs], in0=sqs[j][:],
                                       scalar=1.0 / HD, in1=musq[:, cs],
                                       op0=ALU.mult, op1=ALU.subtract)
    sd = musq  # musq is dead; reuse its slot for sd
    nc.scalar.activation(sd[:], var[:], AF.Sqrt, bias=eps_sb[0:1, :])
    with nc.allow_low_precision(reason="f32r recip holds full fp32 bits"):
        nc.vector.reciprocal(rstd[:], sd[:])
    if LN_BCAST_GPSIMD:
        mu_b = sb.tile([P, ncols], F32R, tag="lnmus", name="lnmubS")
        nc.gpsimd.partition_broadcast(mu_b[:], mu[:], channels=P)
        rstd_b = sb.tile([P, ncols], F32R, tag="lnrstds", name="lnrstdbS")
        nc.gpsimd.partition_broadcast(rstd_b[:], rstd[:], channels=P)
        for ft in range(FT):
            nc.vector.tensor_tensor(r_aps[ft], r_aps[ft], mu_b[:],
                                    op=ALU.subtract)
            nc.vector.tensor_tensor(r_aps[ft], r_aps[ft], rstd_b[:],
                                    op=ALU.mult)
            nc.scalar.activation(out_aps[ft], r_aps[ft], AF.Identity,
                                 scale=s_ap[:, ft:ft + 1],
                                 bias=b_ap[:, ft:ft + 1])
        return
    mu_b = ps.tile([P, ncols], F32, tag="lnmub", name="lnmub")
    rstd_b = ps.tile([P, ncols], F32, tag="lnrstdb", name="lnrstdb")
    for j in range(nseg):
        cs = slice(j * 512, (j + 1) * 512)
        nc.tensor.matmul(mu_b[:, cs], ones_row[:], mu[0:1, cs],
                         start=True, stop=True)
        nc.tensor.matmul(rstd_b[:, cs], ones_row[:], rstd[0:1, cs],
                         start=True, stop=True)
    if stage_sbuf:
        mu_s = sb.tile([P, ncols], F32, tag="lnmus", name="lnmus")
        nc.scalar.activation(mu_s[:], mu_b[:], AF.Identity)
        rstd_s = sb.tile([P, ncols], F32, tag="lnrstds", name="lnrstds")
        nc.vector.tensor_copy(rstd_s[:], rstd_b[:])
        mu_b, rstd_b = mu_s, rstd_s
    for ft in range(FT):
        nc.vector.tensor_tensor(r_aps[ft], r_aps[ft], mu_b[:], op=ALU.subtract)
        nc.vector.tensor_tensor(r_aps[ft], r_aps[ft], rstd_b[:], op=ALU.mult)
        nc.scalar.activation(out_aps[ft], r_aps[ft], AF.Identity,
                             scale=s_ap[:, ft:ft + 1], bias=b_ap[:, ft:ft + 1])


def build_nc(n_layers=L):
    nc = bacc.Bacc("TRN2", target_bir_lowering=False, debug=False,
                   num_devices=N_CORES)
    dt_ = nc.dram_tensor
    t = {}
    t["emb"] = dt_("emb_word", [32000, HD], F32, kind="ExternalInput").ap()
    t["ids"] = dt_("ids", [P, 12], I32, kind="ExternalInput").ap()
    t["pos"] = dt_("pos", [T_EXT, HD], F32, kind="ExternalInput").ap()
    t["eln_s"] = dt_("eln_s", [HD], F32, kind="ExternalInput").ap()
    t["eln_b"] = dt_("eln_b", [HD], F32, kind="ExternalInput").ap()
    t["wq"] = dt_("wq", [L, FT, P, FT, P], F32R, kind="ExternalInput").ap()
    t["wk"] = dt_("wk", [L, FT, P, FT, P], F32R, kind="ExternalInput").ap()
    t["wv"] = dt_("wv", [L, HD, HD], F32R, kind="ExternalInput").ap()
    t["wo"] = dt_("wo", [L, FT, P, FT, P], BF16, kind="ExternalInput").ap()
    t["w1"] = dt_("w1", [L, FFT, P, FT, P], BF16 if FFN_BF16 else F32R, kind="ExternalInput").ap()
    t["w2"] = dt_("w2", [L, FF, HD], BF16 if FFN_BF16 else F32R, kind="ExternalInput").ap()
    for nm in ["bq", "bk", "bo", "b2", "ls1", "lb1", "ls2", "lb2"]:
        t[nm] = dt_(nm, [L, P, FT], F32, kind="ExternalInput").ap()
    t["b1"] = dt_("b1", [L, P, FFT], F32, kind="ExternalInput").ap()
    t["ml"] = dt_("ml", [NCH, P, 512], BF16, kind="ExternalInput").ap()
    t["mr"] = dt_("mr", [NCH, P, 512], BF16, kind="ExternalInput").ap()
    t["halo_ids"] = dt_("halo_ids", [P, 12], I32, kind="ExternalInput").ap()
    t["out"] = dt_("out", [FT, P, T_OWN], F32, kind="ExternalOutput").ap()

    with tile.TileContext(nc) as tc:
        _build_body(nc, tc, n_layers, t)
    nc.compile()
    return nc


def _build_body(nc, tc, n_layers, t):
    from contextlib import ExitStack
    with ExitStack() as ctx:
        persist = ctx.enter_context(tc.tile_pool(name="persist", bufs=1))
        # residual stream, transposed, with halos: x[ft] = [128, T_EXT]
        x = [persist.tile([P, T_EXT], F32R, tag=f"x{ft}", name=f"x{ft}") for ft in range(FT)]
        ml_sb = [persist.tile([P, 512], BF16, tag=f"ml{c}", name=f"ml{c}") for c in range(NCH)]
        mr_sb = [persist.tile([P, 512], BF16, tag=f"mr{c}", name=f"mr{c}") for c in range(NCH)]
        for c in range(NCH):
            nc.sync.dma_start(ml_sb[c][:], t["ml"][c])
            nc.sync.dma_start(mr_sb[c][:], t["mr"][c])
        ones_f = persist.tile([P, 1], F32, tag="ones_f", name="ones_f")
        nc.vector.memset(ones_f[:], 1.0)
        ones_r = persist.tile([P, 1], F32R, tag="ones_r", name="ones_r")
        nc.scalar.activation(ones_r[:], ones_f[:], AF.Identity)
        ones_row_f = persist.tile([1, P], F32, tag="ones_row_f",
                                  name="ones_row_f")
        nc.vector.memset(ones_row_f[:], 1.0)
        ones_row = persist.tile([1, P], F32R, tag="ones_row", name="ones_row")
        nc.scalar.activation(ones_row[:], ones_row_f[:], AF.Identity)
        from concourse.masks import make_identity
        ident = persist.tile([P, P], F32, tag="ident", name="ident")
        make_identity(nc, ident[:])
        hid_sb = persist.tile([P, 12], I32, tag="hid", name="hid")
        nc.sync.dma_start(hid_sb[:], t["halo_ids"][:])
        eps_sb = persist.tile([P, 1], F32, tag="eps", name="eps")
        nc.vector.memset(eps_sb[:], EPS)
        # K=1 broadcast rows for the softmax-denominator outer products
        onesf33 = persist.tile([33, 64], F32, tag="o33f", name="onesf33")
        nc.vector.memset(onesf33[:], 0.0)
        nc.vector.memset(onesf33[0:1, :], 1.0)
        nc.vector.memset(onesf33[32:33, :], 1.0)
        ones33 = persist.tile([33, 64], F32R, tag="o33", name="ones33")
        nc.scalar.activation(ones33[:], onesf33[:], AF.Identity)

        # ---- embedding + LN -> x^T ----
        with tc.tile_pool(name="emb_sb", bufs=1) as esb, \
             tc.tile_pool(name="emb_sb2", bufs=2) as esb2, \
             tc.tile_pool(name="emb_ps", bufs=2, space="PSUM") as eps2:
            ids_sb = esb.tile([P, 12], I32, tag="ids", name="ids")
            nc.sync.dma_start(ids_sb[:], t["ids"][:])
            s_bc = esb.tile([P, HD], F32, tag="sbc", name="sbc")
            nc.sync.dma_start(s_bc[:], bass.AP(
                tensor=t["eln_s"].tensor, offset=0, ap=[[0, P], [1, HD]]))
            b_bc = esb.tile([P, HD], F32, tag="bbc", name="bbc")
            nc.sync.dma_start(b_bc[:], bass.AP(
                tensor=t["eln_b"].tensor, offset=0, ap=[[0, P], [1, HD]]))
            e = [esb.tile([P, HD], F32, tag=f"e{tt}", name=f"e{tt}") for tt in range(12)]
            for tt in range(12):
                nc.gpsimd.indirect_dma_start(
                    out=e[tt][:], out_offset=None, in_=t["emb"][:],
                    in_offset=bass.IndirectOffsetOnAxis(
                        ap=ids_sb[:, tt:tt + 1], axis=0))
                p_sb = esb2.tile([P, HD], F32, tag="pos", name="pos")
                nc.sync.dma_start(p_sb[:], t["pos"][tt * P:(tt + 1) * P, :])
                nc.vector.tensor_tensor(e[tt][:], e[tt][:], p_sb[:], op=ALU.add)
                stats = esb2.tile([P, 3, nc.vector.BN_STATS_DIM], F32,
                                  tag="bst", name="bst")
                er = e[tt][:].rearrange("p (g d) -> p g d", g=3)
                for g in range(3):
                    nc.vector.bn_stats(stats[:, g, :], er[:, g, :])
                mv = esb2.tile([P, nc.vector.BN_AGGR_DIM], F32, tag="bag", name="bag")
                nc.vector.bn_aggr(mv[:], stats[:])
                sd = esb2.tile([P, 1], F32, tag="bsd", name="bsd")
                nc.scalar.activation(sd[:], mv[:, 1:2], AF.Sqrt, bias=eps_sb[:])
                rstd = esb2.tile([P, 1], F32, tag="brstd", name="brstd")
                nc.vector.reciprocal(rstd[:], sd[:])
                nc.vector.tensor_scalar(out=e[tt][:], in0=e[tt][:],
                                        scalar1=mv[:, 0:1], scalar2=rstd[:],
                                        op0=ALU.subtract, op1=ALU.mult)
                nc.vector.tensor_tensor(e[tt][:], e[tt][:], s_bc[:], op=ALU.mult)
                nc.vector.tensor_tensor(e[tt][:], e[tt][:], b_bc[:], op=ALU.add)
            for ft in range(FT):
                tr = eps2.tile([P, T_EXT], F32, tag="tr", name="tr")
                for tt in range(12):
                    nc.tensor.transpose(tr[:, tt * P:(tt + 1) * P],
                                        e[tt][:, ft * P:(ft + 1) * P], ident[:])
                nc.scalar.activation(x[ft][:], tr[:], AF.Identity)

        for l in range(n_layers):
            _layer(nc, tc, t, l, x, ml_sb, mr_sb, ones_r, ones_row, eps_sb,
                   hid_sb, ones33, exchange=(l < n_layers - 1), deferred=(l > 0))

        for ft in range(FT):
            nc.gpsimd.dma_start(t["out"][ft], x[ft][:, W:W + T_OWN])


USE_NEW_NORM = True


def _attn_chunk(nc, c, qT, kT, v, o, ml_sb, mr_sb, ones33, bsb, bps, opool,
                dpool):
    """Attention for one 256-query chunk, all 12 heads + normalization."""
    # DVE writes must start 32-partition-aligned: stage the 12 denominator
    # rows on partitions {0, 32} (6 heads each), then DMA-scatter into 12
    den_f = dpool.tile([33, 6 * W], F32, tag="denf", name="den_f", bufs=1)
    for h in range(NH):
        ft, po = h // 2, (h % 2) * 64
        sps = bps.tile([P, 6 * W], F32, tag="sps", name="sps")
        for w in range(6):
            nc.tensor.matmul(
                sps[:, w * W:(w + 1) * W],
                kT[ft][po:po + 64, c * W + w * P:c * W + (w + 1) * P],
                qT[ft][po:po + 64, c * W:(c + 1) * W],
                start=True, stop=True)
        ex = bsb.tile([P, 6 * W], BF16, tag="ex", name="ex")
        nc.scalar.activation(ex[:], sps[:], AF.Exp)
        nc.vector.tensor_tensor(ex[:, 0:512], ex[:, 0:512],
                                ml_sb[c][:], op=ALU.mult)
        nc.vector.tensor_tensor(ex[:, 1024:1536], ex[:, 1024:1536],
                                mr_sb[c][:], op=ALU.mult)
        ops = opool.tile([P, W], F32, tag="ops", name="ops")
        for w in range(6):
            nc.tensor.matmul(
                ops[0:65, :],
                v[c * 2 + w][:, h, :],
                ex[:, w * W:(w + 1) * W],
                start=(w == 0), stop=(w == 5))
        dfr = (h // 6) * 32
        nc.vector.tensor_copy(den_f[dfr:dfr + 1, (h % 6) * W:(h % 6 + 1) * W],
                              ops[64:65, :])
        # unnormalized attention out -> o slot (normalized in bulk below);
        # alternate engines to balance scalar (exp) vs vector (masks) load
        dst = o[ft][po:po + 64, c * W:(c + 1) * W]
        if h % 2 == 0:
            nc.scalar.activation(dst, ops[0:64, :], AF.Identity)
        else:
            nc.vector.tensor_copy(dst, ops[0:64, :])
    dinv_f = dpool.tile([33, 6 * W], F32R, tag="dinvf", name="dinv_f", bufs=1)
    with nc.allow_low_precision(reason="f32r recip holds full fp32 bits"):
        nc.vector.reciprocal(dinv_f[0:1, :], den_f[0:1, :])
        nc.vector.reciprocal(dinv_f[32:33, :], den_f[32:33, :])
    for ft in range(FT):
        bc = opool.tile([P, 2 * W], F32, tag="ops", name="bc")
        for half in range(2):
            h = 2 * ft + half
            r, j = (h // 6) * 32, h % 6
            nc.tensor.matmul(bc[0:64, half * W:(half + 1) * W],
                             ones33[r:r + 1, :],
                             dinv_f[r:r + 1, j * W:(j + 1) * W],
                             start=True, stop=True)
        for half in range(2):
            po = half * 64
            nc.vector.tensor_tensor(
                o[ft][po:po + 64, c * W:(c + 1) * W],
                o[ft][po:po + 64, c * W:(c + 1) * W],
                bc[0:64, half * W:(half + 1) * W], op=ALU.mult)


def _ffn_pass(nc, tc, t, l, tag, y_mm, y_res, r2, b1_sb, b2_sb):
    """One FFN pass over 512 token-columns.

    y_mm: bf16 [128,512] APs (matmul rhs); y_res: f32r views of the same
    columns (residual add). r2: FT [128,512] f32r SBUF tiles for y + FFN(y).
    """
    from contextlib import ExitStack
    with ExitStack() as dctx:
        dsb = dctx.enter_context(tc.tile_pool(name=f"pd_sb{l}_{tag}", bufs=3))
        zps = dctx.enter_context(
            tc.tile_pool(name=f"pd_psz{l}_{tag}", bufs=1, space="PSUM"))
        fps = dctx.enter_context(
            tc.tile_pool(name=f"pd_psf{l}_{tag}", bufs=2, space="PSUM"))
        zp = [zps.tile([P, 512], F32, tag=f"z{mt}", name=f"z{mt}") for mt in range(FT)]
        for ms in range(FFT):
            w1_sb = dsb.tile([P, HD], BF16 if FFN_BF16 else F32R, tag="w1s", name="w1s")
            nc.sync.dma_start(w1_sb[:], t["w1"][l, ms])
            fp = fps.tile([P, 512], F32, tag="fp", name="fp")
            for kt in range(FT):
                nc.tensor.matmul(fp[:], w1_sb[:, kt * P:(kt + 1) * P],
                                 y_mm[kt], start=(kt == 0),
                                 stop=(kt == FT - 1))
            f_sb = dsb.tile([P, 512], BF16 if FFN_BF16 else F32R, tag="fsb", name="fsb")
            nc.scalar.activation(f_sb[:], fp[:], AF.Gelu,
                                 bias=b1_sb[:, ms:ms + 1])
            w2_sb = dsb.tile([P, HD], BF16 if FFN_BF16 else F32R, tag="w2s", name="w2s")
            nc.sync.dma_start(w2_sb[:],
                              t["w2"][l, ms * P:(ms + 1) * P, :])
            for mt in range(FT):
                nc.tensor.matmul(zp[mt][:],
                                 w2_sb[:, mt * P:(mt + 1) * P],
                                 f_sb[:], start=(ms == 0),
                                 stop=(ms == FFT - 1))
        for mt in range(FT):
            nc.vector.scalar_tensor_tensor(
                out=r2[mt][:], in0=zp[mt][:],
                scalar=b2_sb[:, mt:mt + 1], in1=y_res[mt],
                op0=ALU.add, op1=ALU.add)


def _layer(nc, tc, t, l, x, ml_sb, mr_sb, ones_r, ones_row, eps_sb, hid_sb,
           ones33, exchange, deferred):
    from contextlib import ExitStack
    with ExitStack() as ctx:
        lsb = ctx.enter_context(tc.tile_pool(name=f"lsb{l}", bufs=1))

        def bias_tile(name, n=FT):
            bt = lsb.tile([P, n], F32, tag=f"b_{name}", name=f"b_{name}")
            nc.sync.dma_start(bt[:], t[name][l])
            return bt
        bq_sb = bias_tile("bq"); bk_sb = bias_tile("bk"); bo_sb = bias_tile("bo")
        b1_sb = bias_tile("b1", FFT); b2_sb = bias_tile("b2")
        ls1_sb = bias_tile("ls1"); lb1_sb = bias_tile("lb1")
        ls2_sb = bias_tile("ls2"); lb2_sb = bias_tile("lb2")

        qT = [lsb.tile([P, T_OWN], F32R, tag=f"qT{i}", name=f"qT{i}") for i in range(FT)]
        kT = [lsb.tile([P, T_EXT], F32R, tag=f"kT{i}", name=f"kT{i}") for i in range(FT)]
        v = [lsb.tile([P, NH, 65], BF16, tag=f"v{i}", name=f"v{i}") for i in range(12)]
        o = [lsb.tile([P, T_OWN], BF16, tag=f"o{i}", name=f"o{i}") for i in range(FT)]
        r2 = [lsb.tile([P, 512], F32R, tag=f"r2_{i}", name=f"r2_{i}") for i in range(FT)]
        y_bf = [lsb.tile([P, T_OWN], BF16, tag=f"ybf{i}", name=f"ybf{i}") for i in range(FT)]
        # aliases: qT tiles double as r1 (post-attention residual), kT's first
        # 1024 cols double as y (LN1 output) -- their producers are dead by then
        r1 = qT
        y = [kT[i][:, 0:T_OWN] for i in range(FT)]

        # ---- phase A: QKV projections (own tokens; halo deferred if l>0) ----
        # kT col j = ext token j; own tokens are ext cols [W, W+T_OWN)
        wv_sb = ctx.enter_context(tc.tile_pool(name=f"pa_wv{l}", bufs=1))
        wv_all = [wv_sb.tile([P, HD], F32R, tag=f"wv{kt}", name=f"wv{kt}")
                  for kt in range(FT)]
        for kt in range(FT):
            nc.sync.dma_start(wv_all[kt][:],
                              t["wv"][l, kt * P:(kt + 1) * P, :])
        own_tt = range(2, 10) if deferred else range(12)
        halo_tt = (0, 1, 10, 11)
        own_segs = [(W, W + 512), (W + 512, W + T_OWN)]
        with tc.tile_pool(name=f"pa_sb{l}", bufs=3) as pa_sb, \
             tc.tile_pool(name=f"pa_ps{l}", bufs=4, space="PSUM") as pa_ps:
            for mt in range(FT):  # qT over own tokens
                wq_sb = pa_sb.tile([P, HD], F32R, tag="wqs", name="wqs")
                nc.sync.dma_start(wq_sb[:], t["wq"][l, mt])
                for h2 in range(2):
                    ps = pa_ps.tile([P, 512], F32, tag="pp", name="pp")
                    for kt in range(FT):
                        nc.tensor.matmul(
                            ps[:], wq_sb[:, kt * P:(kt + 1) * P],
                            x[kt][:, W + h2 * 512:W + (h2 + 1) * 512],
                            start=(kt == 0), stop=(kt == FT - 1))
                    nc.scalar.activation(qT[mt][:, h2 * 512:(h2 + 1) * 512],
                                         ps[:], AF.Identity,
                                         bias=bq_sb[:, mt:mt + 1])
            for mt in range(FT):  # kT over own tokens (+ halo when l == 0)
                wk_sb = pa_sb.tile([P, HD], F32R, tag="wks", name="wks")
                nc.sync.dma_start(wk_sb[:], t["wk"][l, mt])
                segs = list(own_segs)
                if not deferred:
                    segs.append(None)  # halo 2-range seg
                for sg in segs:
                    ps = pa_ps.tile([P, 512], F32, tag="pp", name="pp")
                    rhs = ([x[kt][:, sg[0]:sg[1]] for kt in range(FT)]
                           if sg is not None else
                           [_two_seg(x[kt], 0, W + T_OWN, W)
                            for kt in range(FT)])
                    for kt in range(FT):
                        nc.tensor.matmul(
                            ps[:], wk_sb[:, kt * P:(kt + 1) * P], rhs[kt],
                            start=(kt == 0), stop=(kt == FT - 1))
                    dst = (kT[mt][:, sg[0]:sg[1]] if sg is not None
                           else _two_seg(kT[mt], 0, W + T_OWN, W))
                    nc.scalar.activation(dst, ps[:], AF.Identity,
                                         bias=bk_sb[:, mt:mt + 1])
            # v natural [tok, d]: lhsT = x slice, rhs = wv strip
            vtts = list(own_tt) + ([] if deferred else list(halo_tt))
            for tt in vtts:
                for hf in range(2):
                    ps = pa_ps.tile([P, 384], F32, tag="ppv", name="ppv",
                                    bufs=2)
                    for kt in range(FT):
                        nc.tensor.matmul(
                            ps[:], x[kt][:, tt * P:(tt + 1) * P],
                            wv_all[kt][:, hf * 384:(hf + 1) * 384],
                            start=(kt == 0), stop=(kt == FT - 1))
                    nc.scalar.activation(
                        v[tt][:, hf * 6:(hf + 1) * 6, 0:64],
                        ps[:].rearrange("p (h d) -> p h d", h=6), AF.Identity)
            for tt in range(12):
                nc.vector.memset(v[tt][:, :, 64:65], 1.0)

        # ---- phase B: banded attention (interior chunks first) ----
        with tc.tile_pool(name=f"pb_sb{l}", bufs=3) as bsb, \
             tc.tile_pool(name=f"pb_dsb{l}", bufs=2) as dpool, \
             tc.tile_pool(name=f"pb_ps{l}", bufs=2, space="PSUM") as bps, \
             tc.tile_pool(name=f"pb_ps2{l}", bufs=2, space="PSUM") as opool:
            for c in (1, 2):
                _attn_chunk(nc, c, qT, kT, v, o, ml_sb, mr_sb, ones33,
                            bsb, bps, opool, dpool)
            if deferred:
                # halo K/V: x halo cols were written by last layer's exchange;
                # psum borrowed from the sps slots (WAR-interleaves with the
                # score pipeline), weights re-streamed
                xh = [_two_seg(x[kt], 0, W + T_OWN, W) for kt in range(FT)]
                for mt in range(FT):
                    wk_sb = bsb.tile([P, HD], F32R, tag="wkh", name="wkh", bufs=2)
                    nc.sync.dma_start(wk_sb[:], t["wk"][l, mt])
                    ps = bps.tile([P, 512], F32, tag="sps", name="hps")
                    for kt in range(FT):
                        nc.tensor.matmul(
                            ps[:], wk_sb[:, kt * P:(kt + 1) * P], xh[kt],
                            start=(kt == 0), stop=(kt == FT - 1))
                    nc.scalar.activation(_two_seg(kT[mt], 0, W + T_OWN, W),
                                         ps[:], AF.Identity,
                                         bias=bk_sb[:, mt:mt + 1])
                for tt in halo_tt:
                    for hf in range(2):
                        ps = bps.tile([P, 384], F32, tag="sps", name="hpsv")
                        for kt in range(FT):
                            nc.tensor.matmul(
                                ps[:], x[kt][:, tt * P:(tt + 1) * P],
                                wv_all[kt][:, hf * 384:(hf + 1) * 384],
                                start=(kt == 0), stop=(kt == FT - 1))
                        nc.scalar.activation(
                            v[tt][:, hf * 6:(hf + 1) * 6, 0:64],
                            ps[:].rearrange("p (h d) -> p h d", h=6),
                            AF.Identity)
            for c in (0, 3):
                _attn_chunk(nc, c, qT, kT, v, o, ml_sb, mr_sb, ones33,
                            bsb, bps, opool, dpool)

        # ---- phase C: O-proj + residual (r1 <- x + O@Wo + bo) ----
        with tc.tile_pool(name=f"pc_sb{l}", bufs=3) as csb, \
             tc.tile_pool(name=f"pc_ps{l}", bufs=4, space="PSUM") as cps:
            for mt in range(FT):
                wo_sb = csb.tile([P, HD], BF16, tag="wos", name="wos")
                nc.sync.dma_start(wo_sb[:], t["wo"][l, mt])
                for h2 in range(2):
                    ps = cps.tile([P, 512], F32, tag="ppo", name="ppo")
                    for kt in range(FT):
                        nc.tensor.matmul(
                            ps[:], wo_sb[:, kt * P:(kt + 1) * P],
                            o[kt][:, h2 * 512:(h2 + 1) * 512],
                            start=(kt == 0), stop=(kt == FT - 1))
                    nc.vector.scalar_tensor_tensor(
                        out=r1[mt][:, h2 * 512:(h2 + 1) * 512], in0=ps[:],
                        scalar=bo_sb[:, mt:mt + 1],
                        in1=x[mt][:, W + h2 * 512:W + (h2 + 1) * 512],
                        op0=ALU.add, op1=ALU.add)
        # ---- LN1 full-width: y <- LN(r1)*s+b; y_bf: bf16 copy for matmuls ----
        with tc.tile_pool(name=f"ln1_sb{l}", bufs=1) as l1sb, \
             tc.tile_pool(name=f"ln1_ps{l}", bufs=1, space="PSUM") as l1ps:
            _ln_T(nc, l1sb, l1ps, [r1[ft][:] for ft in range(FT)],
                  [y[ft] for ft in range(FT)],
                  ones_r, ones_row, eps_sb, ls1_sb, lb1_sb, T_OWN)
        if FFN_BF16:
            for ft in range(FT):
                nc.vector.tensor_copy(y_bf[ft][:], y[ft])

        # ---- FFN boundary pass (token cols {0:W} U {768:1024} of own) ----
        ysrc = y_bf if FFN_BF16 else kT
        y_bd = [_two_seg(ysrc[ft], 0, 3 * W, W) for ft in range(FT)]
        yr_bd = [_two_seg(kT[ft], 0, 3 * W, W) for ft in range(FT)]
        _ffn_pass(nc, tc, t, l, "bd", y_bd, yr_bd, r2, b1_sb, b2_sb)
        with tc.tile_pool(name=f"ln2b_sb{l}", bufs=1) as l2sb, \
             tc.tile_pool(name=f"ln2b_ps{l}", bufs=1, space="PSUM") as l2ps:
            _ln_T(nc, l2sb, l2ps, [r2[ft][:] for ft in range(FT)],
                  [_two_seg(x[ft], W, W + 3 * W, W) for ft in range(FT)],
                  ones_r, ones_row, eps_sb, ls2_sb, lb2_sb, 512,
                  stage_sbuf=True)

        # ---- halo exchange (overlaps FFN interior + next layer's QKV) ----
        if exchange:
            edram = ctx.enter_context(
                tc.tile_pool(name=f"pe_dram{l}", bufs=1, space="DRAM"))
            b_in = edram.tile([2, FT, P, W], F32R, tag="bin", name="bin")
            b_out = edram.tile([4 * 2 * FT * P, W], F32R, tag="bout",
                               name="bout")
            for ft in range(FT):
                nc.sync.dma_start(b_in[0, ft], x[ft][:, W:2 * W])
                nc.sync.dma_start(b_in[1, ft], x[ft][:, T_OWN:T_OWN + W])
            nc.gpsimd.collective_compute(
                "AllGather", ALU.bypass,
                replica_groups=[[0, 1, 2, 3], [4, 5, 6, 7]],
                ins=[b_in[:].opt()], outs=[b_out[:].opt()])
            for side in range(2):
                for ft in range(FT):
                    dst = (x[ft][:, 0:W] if side == 0
                           else x[ft][:, T_OWN + W:T_EXT])
                    nc.gpsimd.indirect_dma_start(
                        out=dst, out_offset=None, in_=b_out[:],
                        in_offset=bass.IndirectOffsetOnAxis(
                            ap=hid_sb[:, side * FT + ft:side * FT + ft + 1],
                            axis=0))

        # ---- FFN interior pass (token cols [W:768) of own) ----
        y_int = [ysrc[ft][:, W:3 * W] for ft in range(FT)]
        yr_int = [kT[ft][:, W:3 * W] for ft in range(FT)]
        _ffn_pass(nc, tc, t, l, "int", y_int, yr_int, r2, b1_sb, b2_sb)
        with tc.tile_pool(name=f"ln2i_sb{l}", bufs=1) as l2sb, \
             tc.tile_pool(name=f"ln2i_ps{l}", bufs=1, space="PSUM") as l2ps:
            _ln_T(nc, l2sb, l2ps, [r2[ft][:] for ft in range(FT)],
                  [x[ft][:, 2 * W:4 * W] for ft in range(FT)],
                  ones_r, ones_row, eps_sb, ls2_sb, lb2_sb, 512)


# ---------------- host side ----------------

def _blocked(w, n_k, n_m):
    """[n_k*128, n_m*128] -> [n_m, 128, n_k, 128] (lhsT strips by out-tile)."""
    return np.ascontiguousarray(
        w.reshape(n_k, P, n_m, P).transpose(2, 1, 0, 3))


def _bias_lay(b, n):
    return np.ascontiguousarray(b.reshape(n, P).T)


def prepare(inputs):
    """Build per-core in_maps from full inputs."""
    ids_full = np.asarray(inputs["input_ids"]).astype(np.int32)
    am = np.asarray(inputs["attention_mask"]).astype(np.int32)
    emb_word = np.asarray(inputs["emb_word"], dtype=np.float32)
    emb_pos = np.asarray(inputs["emb_pos"], dtype=np.float32)
    Wq = np.asarray(inputs["Wq"], np.float32) / np.sqrt(DH)
    bq = np.asarray(inputs["bq"], np.float32) / np.sqrt(DH)
    Wk = np.asarray(inputs["Wk"], np.float32)
    bk = np.asarray(inputs["bk"], np.float32)
    Wv = np.asarray(inputs["Wv"], np.float32)
    bv = np.asarray(inputs["bv"], np.float32)
    Wo = np.asarray(inputs["Wo"], np.float32)
    bo = np.asarray(inputs["bo"], np.float32)
    W1 = np.asarray(inputs["W1"], np.float32)
    b1 = np.asarray(inputs["b1"], np.float32)
    W2 = np.asarray(inputs["W2"], np.float32)
    b2 = np.asarray(inputs["b2"], np.float32)
    assert np.all(am == 1), "general attention_mask needs mid-tile masks too"

    shared = {
        "emb_word": emb_word,
        "eln_s": np.asarray(inputs["emb_ln_s"], np.float32),
        "eln_b": np.asarray(inputs["emb_ln_b"], np.float32),
        "wq": np.stack([_blocked(Wq[i], FT, FT) for i in range(L)]),
        "wk": np.stack([_blocked(Wk[i], FT, FT) for i in range(L)]),
        "wv": Wv,
        "wo": np.stack([_blocked(Wo[i], FT, FT) for i in range(L)]).astype(
            ml_dtypes.bfloat16),
        "w1": np.stack([_blocked(W1[i], FT, FFT) for i in range(L)]).astype(
            ml_dtypes.bfloat16 if FFN_BF16 else np.float32),
        "w2": W2.astype(ml_dtypes.bfloat16 if FFN_BF16 else np.float32),
        "bq": np.stack([_bias_lay(bq[i], FT) for i in range(L)]),
        "bk": np.stack([_bias_lay(bk[i], FT) for i in range(L)]),
        "bo": np.stack([_bias_lay(bv[i] @ Wo[i] + bo[i], FT)
                        for i in range(L)]),
        "b1": np.stack([_bias_lay(b1[i], FFT) for i in range(L)]),
        "b2": np.stack([_bias_lay(b2[i], FT) for i in range(L)]),
        "ls1": np.stack([_bias_lay(np.asarray(inputs["ln1_s"], np.float32)[i],
                                   FT) for i in range(L)]),
        "lb1": np.stack([_bias_lay(np.asarray(inputs["ln1_b"], np.float32)[i],
                                   FT) for i in range(L)]),
        "ls2": np.stack([_bias_lay(np.asarray(inputs["ln2_s"], np.float32)[i],
                                   FT) for i in range(L)]),
        "lb2": np.stack([_bias_lay(np.asarray(inputs["ln2_b"], np.float32)[i],
                                   FT) for i in range(L)]),
    }

    in_maps = []
    i_idx = np.arange(W)
    for core in range(N_CORES):
        b, sb = core // 4, core % 4
        s0 = sb * T_OWN
        ext_pos = np.clip(np.arange(s0 - W, s0 + T_OWN + W), 0, S - 1)
        m = dict(shared)
        m["ids"] = np.ascontiguousarray(
            ids_full[b, ext_pos].reshape(12, P).T)
        m["pos"] = np.ascontiguousarray(emb_pos[ext_pos])
        # masks: global chunk gc, window key j in [0,768), query i in [0,256):
        #   key_abs = gc*W - W + j ; allowed = |j - W - i| <= W
        #             & 0 <= key_abs < S & attention_mask[b, key_abs]
        mlm = np.zeros((NCH, P, 512), np.float32)
        mrm = np.zeros((NCH, P, 512), np.float32)
        for c in range(NCH):
            gc = sb * NCH + c
            for kt2 in range(2):
                for mm_, j0 in ((mlm, 0), (mrm, 512)):
                    j = j0 + kt2 * P + np.arange(P)[:, None]
                    key_abs = gc * W - W + j
                    ok = (np.abs(j - W - i_idx[None, :]) <= W)
                    ok &= (key_abs >= 0) & (key_abs < S)
                    ok &= am[b, np.clip(key_abs, 0, S - 1)] > 0
                    mm_[c, :, kt2 * W:(kt2 + 1) * W] = ok
        m["ml"] = mlm.astype(ml_dtypes.bfloat16)
        m["mr"] = mrm.astype(ml_dtypes.bfloat16)
        # halo row ids into the gathered [4, 2, FT, 128, W] row table
        hid = np.zeros((2, FT, P), np.int64)
        for side in range(2):
            nb = sb - 1 if side == 0 else sb + 1
            if 0 <= nb <= 3:
                osd = 1 - side  # left halo <- neighbor's right block
                for ft in range(FT):
                    hid[side, ft] = ((nb * 2 + osd) * FT + ft) * P \
                        + np.arange(P)
            else:
                for ft in range(FT):
                    hid[side, ft] = ((sb * 2 + side) * FT + ft) * P \
                        + np.arange(P)
        m["halo_ids"] = np.ascontiguousarray(
            hid.reshape(12, P).T.astype(np.int32))
        in_maps.append(m)
    return in_maps


_NC_CACHE = {}


def get_nc(n_layers=L):
    if n_layers not in _NC_CACHE:
        _NC_CACHE[n_layers] = build_nc(n_layers)
    return _NC_CACHE[n_layers]


def run(inputs, n_layers=L, trace=False):
    nc = get_nc(n_layers)
    in_maps = prepare(inputs)
    res = bass_utils.run_bass_kernel_spmd(
        nc, in_maps, core_ids=list(range(N_CORES)), trace=trace)
    outs = np.empty((B, S, HD), np.float32)
    for core in range(N_CORES):
        b, sb = core // 4, core % 4
        ot = res.results[core]["out"]  # [FT, 128, T_OWN]
        outs[b, sb * T_OWN:(sb + 1) * T_OWN] = ot.reshape(HD, T_OWN).T
    return outs, res


def kernel(**inputs) -> np.ndarray:
    out, _ = run(inputs)
    return out



# revision 16
# speedup vs baseline: 1.0659x; 1.0659x over previous
"""Longformer layer stack (4 layers, sliding-window attention) on 8 TRN2 cores.

Sharding: data-parallel over batch (2) x sequence-parallel (4 blocks of 1024
tokens). Each core computes its sequence block; the banded attention needs a
W=256 token halo, exchanged between neighboring blocks with an AllGather after
each layer (layers 0-2). Residual stream kept transposed ([dmodel, tokens]) in
float32r; attention probs/values in bf16.

v2: the halo exchange is overlapped with compute (boundary-first FFN/LN2 kicks
the collective early; the next layer projects its own-token QKV and runs the
interior attention chunks while the collective is in flight, deferring only
halo K/V projection and the two edge chunks). Softmax normalization is batched
per chunk: denominators staged to SBUF, one reciprocal, a PE select-matmul
broadcast, and one multiply per (chunk, feature-tile) -- replacing per-head
reciprocal/partition-broadcast chains. LayerNorm uses a fused Rsqrt and a
full-width LN1.
"""
import sys

sys.path.insert(0, '/opt/trn_rl_repo')

import numpy as np
import ml_dtypes

import concourse.bass as bass
import concourse.mybir as mybir
import concourse.tile as tile
from concourse import bacc
from concourse import bass_utils

F32 = mybir.dt.float32
F32R = mybir.dt.float32r
BF16 = mybir.dt.bfloat16
I32 = mybir.dt.int32
AF = mybir.ActivationFunctionType
ALU = mybir.AluOpType

NH = 12          # heads
DH = 64          # head dim
HD = 768         # model dim
FF = 3072        # ffn dim
W = 256          # one-sided window
L = 4            # layers
B = 2
S = 4096
EPS = 1e-12
N_CORES = 8
T_OWN = 1024     # tokens per core
T_EXT = 1536     # with halos
FT = 6           # model-dim 128-tiles
FFT = 24         # ffn-dim 128-tiles
NCH = 4          # local chunks of 256 queries
P = 128


def _two_seg(ap2d, o1, o2, n):
    """[128, 2, n] view over cols {o1:o1+n} U {o2:o2+n} of a [128, N] AP."""
    base = ap2d[:, o1:o1 + n]
    return bass.AP(tensor=base.tensor, offset=base.offset,
                   ap=[base.ap[0], [o2 - o1, 2], [1, n]])


LN_BCAST_GPSIMD = False


def _ln_T(nc, sb, ps, r_aps, out_aps, ones_r, ones_row, eps_sb, s_ap, b_ap,
          ncols, stage_sbuf=False, out2_aps=None):
    """LayerNorm over the partition (feature) axis of transposed tiles.

    r_aps: list of FT fp32r SBUF APs [128, ncols] (input; normalized IN PLACE
    up to the final scale/bias which lands in out_aps). ncols in {512, 1024}.
    out_aps[ft] <- LN(r)*s + b. s_ap/b_ap: [128, FT] sbuf. mu/rstd broadcasts
    run as K=1 outer-product matmuls on the PE (gpsimd stays free for the
    collective); stage_sbuf copies them out of PSUM so the banks free early.
    """
    nseg = ncols // 512
    sxs = [ps.tile([1, 512], F32, tag=f"lnsx{j}", name=f"lnsx{j}")
           for j in range(nseg)]
    sqs = [ps.tile([1, 512], F32, tag=f"lnsq{j}", name=f"lnsq{j}")
           for j in range(nseg)]
    for ft in range(FT):
        sqt = sb.tile([P, ncols], F32R, tag="lnsqt", name="lnsqt", bufs=2)
        nc.scalar.activation(sqt[:], r_aps[ft], AF.Square)
        for j in range(nseg):
            cs = slice(j * 512, (j + 1) * 512)
            nc.tensor.matmul(sxs[j][:], ones_r[:, 0:1], r_aps[ft][:, cs],
                             start=(ft == 0), stop=(ft == FT - 1))
            nc.tensor.matmul(sqs[j][:], ones_r[:, 0:1], sqt[:, cs],
                             start=(ft == 0), stop=(ft == FT - 1))
    mu = sb.tile([1, ncols], F32R, tag="lnmu", name="lnmu")
    var = sb.tile([1, ncols], F32, tag="lnvar", name="lnvar")
    musq = sb.tile([1, ncols], F32, tag="lnmusq", name="lnmusq")
    rstd = sb.tile([1, ncols], F32R, tag="lnrstd", name="lnrstd")
    for j in range(nseg):
        cs = slice(j * 512, (j + 1) * 512)
        nc.scalar.activation(mu[:, cs], sxs[j][:], AF.Identity, scale=1.0 / HD)
    nc.vector.tensor_tensor(musq[:], mu[:], mu[:], op=ALU.mult)
    for j in range(nseg):
        cs = slice(j * 512, (j + 1) * 512)
        nc.vector.scalar_tensor_tensor(out=var[:, cs], in0=sqs[j][:],
                                       scalar=1.0 / HD, in1=musq[:, cs],
                                       op0=ALU.mult, op1=ALU.subtract)
    sd = musq  # musq is dead; reuse its slot for sd
    nc.scalar.activation(sd[:], var[:], AF.Sqrt, bias=eps_sb[0:1, :])
    with nc.allow_low_precision(reason="f32r recip holds full fp32 bits"):
        nc.vector.reciprocal(rstd[:], sd[:])
    if LN_BCAST_GPSIMD:
        mu_b = sb.tile([P, ncols], F32R, tag="lnmus", name="lnmubS")
        nc.gpsimd.partition_broadcast(mu_b[:], mu[:], channels=P)
        rstd_b = sb.tile([P, ncols], F32R, tag="lnrstds", name="lnrstdbS")
        nc.gpsimd.partition_broadcast(rstd_b[:], rstd[:], channels=P)
        for ft in range(FT):
            nc.vector.tensor_tensor(r_aps[ft], r_aps[ft], mu_b[:],
                                    op=ALU.subtract)
            nc.vector.tensor_tensor(r_aps[ft], r_aps[ft], rstd_b[:],
                                    op=ALU.mult)
            nc.scalar.activation(out_aps[ft], r_aps[ft], AF.Identity,
                                 scale=s_ap[:, ft:ft + 1],
                                 bias=b_ap[:, ft:ft + 1])
        return
    mu_b = ps.tile([P, ncols], F32, tag="lnmub", name="lnmub")
    rstd_b = ps.tile([P, ncols], F32, tag="lnrstdb", name="lnrstdb")
    for j in range(nseg):
        cs = slice(j * 512, (j + 1) * 512)
        nc.tensor.matmul(mu_b[:, cs], ones_row[:], mu[0:1, cs],
                         start=True, stop=True)
        nc.tensor.matmul(rstd_b[:, cs], ones_row[:], rstd[0:1, cs],
                         start=True, stop=True)
    if stage_sbuf:
        mu_s = sb.tile([P, ncols], F32, tag="lnmus", name="lnmus")
        nc.scalar.activation(mu_s[:], mu_b[:], AF.Identity)
        rstd_s = sb.tile([P, ncols], F32, tag="lnrstds", name="lnrstds")
        nc.vector.tensor_copy(rstd_s[:], rstd_b[:])
        mu_b, rstd_b = mu_s, rstd_s
    for ft in range(FT):
        nc.vector.tensor_tensor(r_aps[ft], r_aps[ft], mu_b[:], op=ALU.subtract)
        nc.vector.tensor_tensor(r_aps[ft], r_aps[ft], rstd_b[:], op=ALU.mult)
        # out2 (bf16 shadow) must read r BEFORE the in-place scale/bias ACT
        if out2_aps is not None:
            nc.vector.tensor_scalar(out=out2_aps[ft], in0=r_aps[ft],
                                    scalar1=s_ap[:, ft:ft + 1],
                                    scalar2=b_ap[:, ft:ft + 1],
                                    op0=ALU.mult, op1=ALU.add)
        nc.scalar.activation(out_aps[ft], r_aps[ft], AF.Identity,
                             scale=s_ap[:, ft:ft + 1], bias=b_ap[:, ft:ft + 1])


def build_nc(n_layers=L):
    nc = bacc.Bacc("TRN2", target_bir_lowering=False, debug=False,
                   num_devices=N_CORES)
    dt_ = nc.dram_tensor
    t = {}
    t["emb"] = dt_("emb_word", [32000, HD], F32, kind="ExternalInput").ap()
    t["ids"] = dt_("ids", [P, 12], I32, kind="ExternalInput").ap()
    t["pos"] = dt_("pos", [T_EXT, HD], F32, kind="ExternalInput").ap()
    t["eln_s"] = dt_("eln_s", [HD], F32, kind="ExternalInput").ap()
    t["eln_b"] = dt_("eln_b", [HD], F32, kind="ExternalInput").ap()
    t["wq"] = dt_("wq", [L, FT, P, FT, P], BF16, kind="ExternalInput").ap()
    t["wk"] = dt_("wk", [L, FT, P, FT, P], BF16, kind="ExternalInput").ap()
    t["wv"] = dt_("wv", [L, HD, HD], BF16, kind="ExternalInput").ap()
    t["wo"] = dt_("wo", [L, FT, P, FT, P], BF16, kind="ExternalInput").ap()
    t["w1"] = dt_("w1", [L, FFT, P, FT, P], BF16, kind="ExternalInput").ap()
    t["w2"] = dt_("w2", [L, FF, HD], BF16, kind="ExternalInput").ap()
    for nm in ["bq", "bk", "bo", "b2", "ls1", "lb1", "ls2", "lb2"]:
        t[nm] = dt_(nm, [L, P, FT], F32, kind="ExternalInput").ap()
    t["b1"] = dt_("b1", [L, P, FFT], F32, kind="ExternalInput").ap()
    t["ml"] = dt_("ml", [NCH, P, 512], BF16, kind="ExternalInput").ap()
    t["mr"] = dt_("mr", [NCH, P, 512], BF16, kind="ExternalInput").ap()
    t["halo_ids"] = dt_("halo_ids", [P, 12], I32, kind="ExternalInput").ap()
    t["out"] = dt_("out", [FT, P, T_OWN], F32, kind="ExternalOutput").ap()

    with tile.TileContext(nc) as tc:
        _build_body(nc, tc, n_layers, t)
    nc.compile()
    return nc


def _build_body(nc, tc, n_layers, t):
    from contextlib import ExitStack
    with ExitStack() as ctx:
        persist = ctx.enter_context(tc.tile_pool(name="persist", bufs=1))
        # residual stream, transposed, with halos: x[ft] = [128, T_EXT].
        # x is the f32r master; x_bf is its bf16 shadow used as matmul input
        # (PE can't mix 32-bit and 16-bit matmul operands).
        x = [persist.tile([P, T_EXT], F32R, tag=f"x{ft}", name=f"x{ft}") for ft in range(FT)]
        x_bf = [persist.tile([P, T_EXT], BF16, tag=f"xb{ft}", name=f"xb{ft}")
                for ft in range(FT)]
        ml_sb = [persist.tile([P, 512], BF16, tag=f"ml{c}", name=f"ml{c}") for c in range(NCH)]
        mr_sb = [persist.tile([P, 512], BF16, tag=f"mr{c}", name=f"mr{c}") for c in range(NCH)]
        for c in range(NCH):
            nc.sync.dma_start(ml_sb[c][:], t["ml"][c])
            nc.sync.dma_start(mr_sb[c][:], t["mr"][c])
        ones_f = persist.tile([P, 1], F32, tag="ones_f", name="ones_f")
        nc.vector.memset(ones_f[:], 1.0)
        ones_r = persist.tile([P, 1], F32R, tag="ones_r", name="ones_r")
        nc.scalar.activation(ones_r[:], ones_f[:], AF.Identity)
        ones_row_f = persist.tile([1, P], F32, tag="ones_row_f",
                                  name="ones_row_f")
        nc.vector.memset(ones_row_f[:], 1.0)
        ones_row = persist.tile([1, P], F32R, tag="ones_row", name="ones_row")
        nc.scalar.activation(ones_row[:], ones_row_f[:], AF.Identity)
        from concourse.masks import make_identity
        ident = persist.tile([P, P], F32, tag="ident", name="ident")
        make_identity(nc, ident[:])
        hid_sb = persist.tile([P, 12], I32, tag="hid", name="hid")
        nc.sync.dma_start(hid_sb[:], t["halo_ids"][:])
        eps_sb = persist.tile([P, 1], F32, tag="eps", name="eps")
        nc.vector.memset(eps_sb[:], EPS)
        # K=1 broadcast rows for the softmax-denominator outer products
        onesf33 = persist.tile([33, 64], F32, tag="o33f", name="onesf33")
        nc.vector.memset(onesf33[:], 0.0)
        nc.vector.memset(onesf33[0:1, :], 1.0)
        nc.vector.memset(onesf33[32:33, :], 1.0)
        ones33 = persist.tile([33, 64], F32R, tag="o33", name="ones33")
        nc.scalar.activation(ones33[:], onesf33[:], AF.Identity)

        # ---- embedding + LN -> x^T ----
        with tc.tile_pool(name="emb_sb", bufs=1) as esb, \
             tc.tile_pool(name="emb_sb2", bufs=2) as esb2, \
             tc.tile_pool(name="emb_ps", bufs=2, space="PSUM") as eps2:
            ids_sb = esb.tile([P, 12], I32, tag="ids", name="ids")
            nc.sync.dma_start(ids_sb[:], t["ids"][:])
            s_bc = esb.tile([P, HD], F32, tag="sbc", name="sbc")
            nc.sync.dma_start(s_bc[:], bass.AP(
                tensor=t["eln_s"].tensor, offset=0, ap=[[0, P], [1, HD]]))
            b_bc = esb.tile([P, HD], F32, tag="bbc", name="bbc")
            nc.sync.dma_start(b_bc[:], bass.AP(
                tensor=t["eln_b"].tensor, offset=0, ap=[[0, P], [1, HD]]))
            e = [esb.tile([P, HD], F32, tag=f"e{tt}", name=f"e{tt}") for tt in range(12)]
            for tt in range(12):
                nc.gpsimd.indirect_dma_start(
                    out=e[tt][:], out_offset=None, in_=t["emb"][:],
                    in_offset=bass.IndirectOffsetOnAxis(
                        ap=ids_sb[:, tt:tt + 1], axis=0))
                p_sb = esb2.tile([P, HD], F32, tag="pos", name="pos")
                nc.sync.dma_start(p_sb[:], t["pos"][tt * P:(tt + 1) * P, :])
                nc.vector.tensor_tensor(e[tt][:], e[tt][:], p_sb[:], op=ALU.add)
                stats = esb2.tile([P, 3, nc.vector.BN_STATS_DIM], F32,
                                  tag="bst", name="bst")
                er = e[tt][:].rearrange("p (g d) -> p g d", g=3)
                for g in range(3):
                    nc.vector.bn_stats(stats[:, g, :], er[:, g, :])
                mv = esb2.tile([P, nc.vector.BN_AGGR_DIM], F32, tag="bag", name="bag")
                nc.vector.bn_aggr(mv[:], stats[:])
                sd = esb2.tile([P, 1], F32, tag="bsd", name="bsd")
                nc.scalar.activation(sd[:], mv[:, 1:2], AF.Sqrt, bias=eps_sb[:])
                rstd = esb2.tile([P, 1], F32, tag="brstd", name="brstd")
                nc.vector.reciprocal(rstd[:], sd[:])
                nc.vector.tensor_scalar(out=e[tt][:], in0=e[tt][:],
                                        scalar1=mv[:, 0:1], scalar2=rstd[:],
                                        op0=ALU.subtract, op1=ALU.mult)
                nc.vector.tensor_tensor(e[tt][:], e[tt][:], s_bc[:], op=ALU.mult)
                nc.vector.tensor_tensor(e[tt][:], e[tt][:], b_bc[:], op=ALU.add)
            for ft in range(FT):
                tr = eps2.tile([P, T_EXT], F32, tag="tr", name="tr")
                for tt in range(12):
                    nc.tensor.transpose(tr[:, tt * P:(tt + 1) * P],
                                        e[tt][:, ft * P:(ft + 1) * P], ident[:])
                nc.scalar.activation(x[ft][:], tr[:], AF.Identity)
                nc.vector.tensor_copy(x_bf[ft][:], tr[:])

        for l in range(n_layers):
            _layer(nc, tc, t, l, x, x_bf, ml_sb, mr_sb, ones_r, ones_row,
                   eps_sb, hid_sb, ones33, exchange=(l < n_layers - 1),
                   deferred=(l > 0))

        for ft in range(FT):
            nc.gpsimd.dma_start(t["out"][ft], x[ft][:, W:W + T_OWN])


USE_NEW_NORM = True


def _attn_chunk(nc, c, qT, kT, v, o, ml_sb, mr_sb, ones33, bsb, bps, opool,
                dpool):
    """Attention for one 256-query chunk, all 12 heads + normalization."""
    # DVE writes must start 32-partition-aligned: stage the 12 denominator
    # rows on partitions {0, 32} (6 heads each), then DMA-scatter into 12
    den_f = dpool.tile([33, 6 * W], F32, tag="denf", name="den_f", bufs=1)
    for h in range(NH):
        ft, po = h // 2, (h % 2) * 64
        sps = bps.tile([P, 6 * W], F32, tag="sps", name="sps")
        for w in range(6):
            nc.tensor.matmul(
                sps[:, w * W:(w + 1) * W],
                kT[ft][po:po + 64, c * W + w * P:c * W + (w + 1) * P],
                qT[ft][po:po + 64, c * W:(c + 1) * W],
                start=True, stop=True)
        ex = bsb.tile([P, 6 * W], BF16, tag="ex", name="ex")
        nc.scalar.activation(ex[:], sps[:], AF.Exp)
        nc.vector.tensor_tensor(ex[:, 0:512], ex[:, 0:512],
                                ml_sb[c][:], op=ALU.mult)
        nc.vector.tensor_tensor(ex[:, 1024:1536], ex[:, 1024:1536],
                                mr_sb[c][:], op=ALU.mult)
        ops = opool.tile([P, W], F32, tag="ops", name="ops")
        for w in range(6):
            nc.tensor.matmul(
                ops[0:65, :],
                v[c * 2 + w][:, h, :],
                ex[:, w * W:(w + 1) * W],
                start=(w == 0), stop=(w == 5))
        dfr = (h // 6) * 32
        nc.vector.tensor_copy(den_f[dfr:dfr + 1, (h % 6) * W:(h % 6 + 1) * W],
                              ops[64:65, :])
        # unnormalized attention out -> o slot (normalized in bulk below);
        # alternate engines to balance scalar (exp) vs vector (masks) load
        dst = o[ft][po:po + 64, c * W:(c + 1) * W]
        if h % 2 == 0:
            nc.scalar.activation(dst, ops[0:64, :], AF.Identity)
        else:
            nc.vector.tensor_copy(dst, ops[0:64, :])
    dinv_f = dpool.tile([33, 6 * W], F32R, tag="dinvf", name="dinv_f", bufs=1)
    with nc.allow_low_precision(reason="f32r recip holds full fp32 bits"):
        nc.vector.reciprocal(dinv_f[0:1, :], den_f[0:1, :])
        nc.vector.reciprocal(dinv_f[32:33, :], den_f[32:33, :])
    for ft in range(FT):
        bc = opool.tile([P, 2 * W], F32, tag="ops", name="bc")
        for half in range(2):
            h = 2 * ft + half
            r, j = (h // 6) * 32, h % 6
            nc.tensor.matmul(bc[0:64, half * W:(half + 1) * W],
                             ones33[r:r + 1, :],
                             dinv_f[r:r + 1, j * W:(j + 1) * W],
                             start=True, stop=True)
        for half in range(2):
            po = half * 64
            nc.vector.tensor_tensor(
                o[ft][po:po + 64, c * W:(c + 1) * W],
                o[ft][po:po + 64, c * W:(c + 1) * W],
                bc[0:64, half * W:(half + 1) * W], op=ALU.mult)


def _ffn_pass(nc, tc, t, l, tag, y_mm, y_res, r2, b1_sb, b2_sb):
    """One FFN pass over 512 token-columns.

    y_mm: bf16 [128,512] APs (matmul rhs); y_res: f32r views of the same
    columns (residual add). r2: FT [128,512] f32r SBUF tiles for y + FFN(y).
    """
    from contextlib import ExitStack
    with ExitStack() as dctx:
        dsb = dctx.enter_context(tc.tile_pool(name=f"pd_sb{l}_{tag}", bufs=3))
        zps = dctx.enter_context(
            tc.tile_pool(name=f"pd_psz{l}_{tag}", bufs=1, space="PSUM"))
        fps = dctx.enter_context(
            tc.tile_pool(name=f"pd_psf{l}_{tag}", bufs=2, space="PSUM"))
        zp = [zps.tile([P, 512], F32, tag=f"z{mt}", name=f"z{mt}") for mt in range(FT)]
        for ms in range(FFT):
            w1_sb = dsb.tile([P, HD], BF16, tag="w1s", name="w1s")
            nc.sync.dma_start(w1_sb[:], t["w1"][l, ms])
            fp = fps.tile([P, 512], F32, tag="fp", name="fp")
            for kt in range(FT):
                nc.tensor.matmul(fp[:], w1_sb[:, kt * P:(kt + 1) * P],
                                 y_mm[kt], start=(kt == 0),
                                 stop=(kt == FT - 1))
            f_sb = dsb.tile([P, 512], BF16, tag="fsb", name="fsb")
            nc.scalar.activation(f_sb[:], fp[:], AF.Gelu,
                                 bias=b1_sb[:, ms:ms + 1])
            w2_sb = dsb.tile([P, HD], BF16, tag="w2s", name="w2s")
            nc.sync.dma_start(w2_sb[:],
                              t["w2"][l, ms * P:(ms + 1) * P, :])
            for mt in range(FT):
                nc.tensor.matmul(zp[mt][:],
                                 w2_sb[:, mt * P:(mt + 1) * P],
                                 f_sb[:], start=(ms == 0),
                                 stop=(ms == FFT - 1))
        for mt in range(FT):
            nc.vector.scalar_tensor_tensor(
                out=r2[mt][:], in0=zp[mt][:],
                scalar=b2_sb[:, mt:mt + 1], in1=y_res[mt],
                op0=ALU.add, op1=ALU.add)


def _layer(nc, tc, t, l, x, x_bf, ml_sb, mr_sb, ones_r, ones_row, eps_sb,
           hid_sb, ones33, exchange, deferred):
    from contextlib import ExitStack
    with ExitStack() as ctx:
        lsb = ctx.enter_context(tc.tile_pool(name=f"lsb{l}", bufs=1))

        def bias_tile(name, n=FT):
            bt = lsb.tile([P, n], F32, tag=f"b_{name}", name=f"b_{name}")
            nc.sync.dma_start(bt[:], t[name][l])
            return bt
        bq_sb = bias_tile("bq"); bk_sb = bias_tile("bk"); bo_sb = bias_tile("bo")
        b1_sb = bias_tile("b1", FFT); b2_sb = bias_tile("b2")
        ls1_sb = bias_tile("ls1"); lb1_sb = bias_tile("lb1")
        ls2_sb = bias_tile("ls2"); lb2_sb = bias_tile("lb2")

        qT = [lsb.tile([P, T_OWN], BF16, tag=f"qT{i}", name=f"qT{i}") for i in range(FT)]
        kT = [lsb.tile([P, T_EXT], BF16, tag=f"kT{i}", name=f"kT{i}") for i in range(FT)]
        v = [lsb.tile([P, NH, 65], BF16, tag=f"v{i}", name=f"v{i}") for i in range(12)]
        o = [lsb.tile([P, T_OWN], BF16, tag=f"o{i}", name=f"o{i}") for i in range(FT)]
        r2 = [lsb.tile([P, 512], F32R, tag=f"r2_{i}", name=f"r2_{i}") for i in range(FT)]
        r1 = [lsb.tile([P, T_OWN], F32R, tag=f"r1_{i}", name=f"r1_{i}")
              for i in range(FT)]
        y_bf = [lsb.tile([P, T_OWN], BF16, tag=f"yb{i}", name=f"yb{i}")
                for i in range(FT)]
        # r1 doubles as y (LN1 output, written in place; f32r residual side)
        y = [r1[i][:] for i in range(FT)]

        # ---- phase A: QKV projections (own tokens; halo deferred if l>0) ----
        # kT col j = ext token j; own tokens are ext cols [W, W+T_OWN)
        wv_sb = ctx.enter_context(tc.tile_pool(name=f"pa_wv{l}", bufs=1))
        wv_all = [wv_sb.tile([P, HD], BF16, tag=f"wv{kt}", name=f"wv{kt}")
                  for kt in range(FT)]
        for kt in range(FT):
            nc.sync.dma_start(wv_all[kt][:],
                              t["wv"][l, kt * P:(kt + 1) * P, :])
        own_tt = range(2, 10) if deferred else range(12)
        halo_tt = (0, 1, 10, 11)
        own_segs = [(W, W + 512), (W + 512, W + T_OWN)]
        with tc.tile_pool(name=f"pa_sb{l}", bufs=3) as pa_sb, \
             tc.tile_pool(name=f"pa_ps{l}", bufs=4, space="PSUM") as pa_ps:
            for mt in range(FT):  # qT over own tokens
                wq_sb = pa_sb.tile([P, HD], BF16, tag="wqs", name="wqs")
                nc.sync.dma_start(wq_sb[:], t["wq"][l, mt])
                for h2 in range(2):
                    ps = pa_ps.tile([P, 512], F32, tag="pp", name="pp")
                    for kt in range(FT):
                        nc.tensor.matmul(
                            ps[:], wq_sb[:, kt * P:(kt + 1) * P],
                            x_bf[kt][:, W + h2 * 512:W + (h2 + 1) * 512],
                            start=(kt == 0), stop=(kt == FT - 1))
                    nc.scalar.activation(qT[mt][:, h2 * 512:(h2 + 1) * 512],
                                         ps[:], AF.Identity,
                                         bias=bq_sb[:, mt:mt + 1])
            for mt in range(FT):  # kT over own tokens (+ halo when l == 0)
                wk_sb = pa_sb.tile([P, HD], BF16, tag="wks", name="wks")
                nc.sync.dma_start(wk_sb[:], t["wk"][l, mt])
                segs = list(own_segs)
                if not deferred:
                    segs.append(None)  # halo 2-range seg
                for sg in segs:
                    ps = pa_ps.tile([P, 512], F32, tag="pp", name="pp")
                    rhs = ([x_bf[kt][:, sg[0]:sg[1]] for kt in range(FT)]
                           if sg is not None else
                           [_two_seg(x_bf[kt], 0, W + T_OWN, W)
                            for kt in range(FT)])
                    for kt in range(FT):
                        nc.tensor.matmul(
                            ps[:], wk_sb[:, kt * P:(kt + 1) * P], rhs[kt],
                            start=(kt == 0), stop=(kt == FT - 1))
                    dst = (kT[mt][:, sg[0]:sg[1]] if sg is not None
                           else _two_seg(kT[mt], 0, W + T_OWN, W))
                    nc.scalar.activation(dst, ps[:], AF.Identity,
                                         bias=bk_sb[:, mt:mt + 1])
            # v natural [tok, d]: lhsT = x slice, rhs = wv strip
            vtts = list(own_tt) + ([] if deferred else list(halo_tt))
            for tt in vtts:
                for hf in range(2):
                    ps = pa_ps.tile([P, 384], F32, tag="ppv", name="ppv",
                                    bufs=2)
                    for kt in range(FT):
                        nc.tensor.matmul(
                            ps[:], x_bf[kt][:, tt * P:(tt + 1) * P],
                            wv_all[kt][:, hf * 384:(hf + 1) * 384],
                            start=(kt == 0), stop=(kt == FT - 1))
                    nc.scalar.activation(
                        v[tt][:, hf * 6:(hf + 1) * 6, 0:64],
                        ps[:].rearrange("p (h d) -> p h d", h=6), AF.Identity)
            for tt in range(12):
                nc.vector.memset(v[tt][:, :, 64:65], 1.0)

        # ---- phase B: banded attention (interior chunks first) ----
        with tc.tile_pool(name=f"pb_sb{l}", bufs=3) as bsb, \
             tc.tile_pool(name=f"pb_dsb{l}", bufs=2) as dpool, \
             tc.tile_pool(name=f"pb_ps{l}", bufs=2, space="PSUM") as bps, \
             tc.tile_pool(name=f"pb_ps2{l}", bufs=2, space="PSUM") as opool:
            for c in (1, 2):
                _attn_chunk(nc, c, qT, kT, v, o, ml_sb, mr_sb, ones33,
                            bsb, bps, opool, dpool)
            if deferred:
                # halo K/V: x halo cols were written by last layer's exchange;
                # psum borrowed from the sps slots (WAR-interleaves with the
                # score pipeline), weights re-streamed
                xh = [_two_seg(x_bf[kt], 0, W + T_OWN, W) for kt in range(FT)]
                for mt in range(FT):
                    wk_sb = bsb.tile([P, HD], BF16, tag="wkh", name="wkh", bufs=2)
                    nc.sync.dma_start(wk_sb[:], t["wk"][l, mt])
                    ps = bps.tile([P, 512], F32, tag="sps", name="hps")
                    for kt in range(FT):
                        nc.tensor.matmul(
                            ps[:], wk_sb[:, kt * P:(kt + 1) * P], xh[kt],
                            start=(kt == 0), stop=(kt == FT - 1))
                    nc.scalar.activation(_two_seg(kT[mt], 0, W + T_OWN, W),
                                         ps[:], AF.Identity,
                                         bias=bk_sb[:, mt:mt + 1])
                for tt in halo_tt:
                    for hf in range(2):
                        ps = bps.tile([P, 384], F32, tag="sps", name="hpsv")
                        for kt in range(FT):
                            nc.tensor.matmul(
                                ps[:], x_bf[kt][:, tt * P:(tt + 1) * P],
                                wv_all[kt][:, hf * 384:(hf + 1) * 384],
                                start=(kt == 0), stop=(kt == FT - 1))
                        nc.scalar.activation(
                            v[tt][:, hf * 6:(hf + 1) * 6, 0:64],
                            ps[:].rearrange("p (h d) -> p h d", h=6),
                            AF.Identity)
            for c in (0, 3):
                _attn_chunk(nc, c, qT, kT, v, o, ml_sb, mr_sb, ones33,
                            bsb, bps, opool, dpool)

        # ---- phase C: O-proj + residual (r1 <- x + O@Wo + bo) ----
        with tc.tile_pool(name=f"pc_sb{l}", bufs=3) as csb, \
             tc.tile_pool(name=f"pc_ps{l}", bufs=4, space="PSUM") as cps:
            for mt in range(FT):
                wo_sb = csb.tile([P, HD], BF16, tag="wos", name="wos")
                nc.sync.dma_start(wo_sb[:], t["wo"][l, mt])
                for h2 in range(2):
                    ps = cps.tile([P, 512], F32, tag="ppo", name="ppo")
                    for kt in range(FT):
                        nc.tensor.matmul(
                            ps[:], wo_sb[:, kt * P:(kt + 1) * P],
                            o[kt][:, h2 * 512:(h2 + 1) * 512],
                            start=(kt == 0), stop=(kt == FT - 1))
                    nc.vector.scalar_tensor_tensor(
                        out=r1[mt][:, h2 * 512:(h2 + 1) * 512], in0=ps[:],
                        scalar=bo_sb[:, mt:mt + 1],
                        in1=x[mt][:, W + h2 * 512:W + (h2 + 1) * 512],
                        op0=ALU.add, op1=ALU.add)
        # ---- LN1 full-width: y <- LN(r1)*s+b (in place over r1/qT) ----
        with tc.tile_pool(name=f"ln1_sb{l}", bufs=1) as l1sb, \
             tc.tile_pool(name=f"ln1_ps{l}", bufs=1, space="PSUM") as l1ps:
            _ln_T(nc, l1sb, l1ps, [r1[ft][:] for ft in range(FT)],
                  [y[ft] for ft in range(FT)],
                  ones_r, ones_row, eps_sb, ls1_sb, lb1_sb, T_OWN,
                  out2_aps=[y_bf[ft][:] for ft in range(FT)])

        # ---- FFN boundary pass (token cols {0:W} U {768:1024} of own) ----
        y_bd = [_two_seg(y_bf[ft], 0, 3 * W, W) for ft in range(FT)]
        yr_bd = [_two_seg(r1[ft], 0, 3 * W, W) for ft in range(FT)]
        _ffn_pass(nc, tc, t, l, "bd", y_bd, yr_bd, r2, b1_sb, b2_sb)
        with tc.tile_pool(name=f"ln2b_sb{l}", bufs=1) as l2sb, \
             tc.tile_pool(name=f"ln2b_ps{l}", bufs=1, space="PSUM") as l2ps:
            _ln_T(nc, l2sb, l2ps, [r2[ft][:] for ft in range(FT)],
                  [_two_seg(x[ft], W, W + 3 * W, W) for ft in range(FT)],
                  ones_r, ones_row, eps_sb, ls2_sb, lb2_sb, 512,
                  stage_sbuf=True,
                  out2_aps=[_two_seg(x_bf[ft], W, W + 3 * W, W)
                            for ft in range(FT)])

        # ---- halo exchange (overlaps FFN interior + next layer's QKV) ----
        if exchange:
            edram = ctx.enter_context(
                tc.tile_pool(name=f"pe_dram{l}", bufs=1, space="DRAM"))
            b_in = edram.tile([2, FT, P, W], BF16, tag="bin", name="bin")
            b_out = edram.tile([4 * 2 * FT * P, W], BF16, tag="bout",
                               name="bout")
            for ft in range(FT):
                nc.sync.dma_start(b_in[0, ft], x_bf[ft][:, W:2 * W])
                nc.sync.dma_start(b_in[1, ft], x_bf[ft][:, T_OWN:T_OWN + W])
            nc.gpsimd.collective_compute(
                "AllGather", ALU.bypass,
                replica_groups=[[0, 1, 2, 3], [4, 5, 6, 7]],
                ins=[b_in[:].opt()], outs=[b_out[:].opt()])
            for side in range(2):
                for ft in range(FT):
                    dst = (x_bf[ft][:, 0:W] if side == 0
                           else x_bf[ft][:, T_OWN + W:T_EXT])
                    nc.gpsimd.indirect_dma_start(
                        out=dst, out_offset=None, in_=b_out[:],
                        in_offset=bass.IndirectOffsetOnAxis(
                            ap=hid_sb[:, side * FT + ft:side * FT + ft + 1],
                            axis=0))

        # ---- FFN interior pass (token cols [W:768) of own) ----
        y_int = [y_bf[ft][:, W:3 * W] for ft in range(FT)]
        yr_int = [r1[ft][:, W:3 * W] for ft in range(FT)]
        _ffn_pass(nc, tc, t, l, "int", y_int, yr_int, r2, b1_sb, b2_sb)
        with tc.tile_pool(name=f"ln2i_sb{l}", bufs=1) as l2sb, \
             tc.tile_pool(name=f"ln2i_ps{l}", bufs=1, space="PSUM") as l2ps:
            _ln_T(nc, l2sb, l2ps, [r2[ft][:] for ft in range(FT)],
                  [x[ft][:, 2 * W:4 * W] for ft in range(FT)],
                  ones_r, ones_row, eps_sb, ls2_sb, lb2_sb, 512,
                  out2_aps=[x_bf[ft][:, 2 * W:4 * W] for ft in range(FT)])


# ---------------- host side ----------------

def _blocked(w, n_k, n_m):
    """[n_k*128, n_m*128] -> [n_m, 128, n_k, 128] (lhsT strips by out-tile)."""
    return np.ascontiguousarray(
        w.reshape(n_k, P, n_m, P).transpose(2, 1, 0, 3))


def _bias_lay(b, n):
    return np.ascontiguousarray(b.reshape(n, P).T)


def prepare(inputs):
    """Build per-core in_maps from full inputs."""
    ids_full = np.asarray(inputs["input_ids"]).astype(np.int32)
    am = np.asarray(inputs["attention_mask"]).astype(np.int32)
    emb_word = np.asarray(inputs["emb_word"], dtype=np.float32)
    emb_pos = np.asarray(inputs["emb_pos"], dtype=np.float32)
    Wq = np.asarray(inputs["Wq"], np.float32) / np.sqrt(DH)
    bq = np.asarray(inputs["bq"], np.float32) / np.sqrt(DH)
    Wk = np.asarray(inputs["Wk"], np.float32)
    bk = np.asarray(inputs["bk"], np.float32)
    Wv = np.asarray(inputs["Wv"], np.float32)
    bv = np.asarray(inputs["bv"], np.float32)
    Wo = np.asarray(inputs["Wo"], np.float32)
    bo = np.asarray(inputs["bo"], np.float32)
    W1 = np.asarray(inputs["W1"], np.float32)
    b1 = np.asarray(inputs["b1"], np.float32)
    W2 = np.asarray(inputs["W2"], np.float32)
    b2 = np.asarray(inputs["b2"], np.float32)
    assert np.all(am == 1), "general attention_mask needs mid-tile masks too"

    shared = {
        "emb_word": emb_word,
        "eln_s": np.asarray(inputs["emb_ln_s"], np.float32),
        "eln_b": np.asarray(inputs["emb_ln_b"], np.float32),
        "wq": np.stack([_blocked(Wq[i], FT, FT) for i in range(L)]).astype(
            ml_dtypes.bfloat16),
        "wk": np.stack([_blocked(Wk[i], FT, FT) for i in range(L)]).astype(
            ml_dtypes.bfloat16),
        "wv": Wv.astype(ml_dtypes.bfloat16),
        "wo": np.stack([_blocked(Wo[i], FT, FT) for i in range(L)]).astype(
            ml_dtypes.bfloat16),
        "w1": np.stack([_blocked(W1[i], FT, FFT) for i in range(L)]).astype(
            ml_dtypes.bfloat16),
        "w2": W2.astype(ml_dtypes.bfloat16),
        "bq": np.stack([_bias_lay(bq[i], FT) for i in range(L)]),
        "bk": np.stack([_bias_lay(bk[i], FT) for i in range(L)]),
        "bo": np.stack([_bias_lay(bv[i] @ Wo[i] + bo[i], FT)
                        for i in range(L)]),
        "b1": np.stack([_bias_lay(b1[i], FFT) for i in range(L)]),
        "b2": np.stack([_bias_lay(b2[i], FT) for i in range(L)]),
        "ls1": np.stack([_bias_lay(np.asarray(inputs["ln1_s"], np.float32)[i],
                                   FT) for i in range(L)]),
        "lb1": np.stack([_bias_lay(np.asarray(inputs["ln1_b"], np.float32)[i],
                                   FT) for i in range(L)]),
        "ls2": np.stack([_bias_lay(np.asarray(inputs["ln2_s"], np.float32)[i],
                                   FT) for i in range(L)]),
        "lb2": np.stack([_bias_lay(np.asarray(inputs["ln2_b"], np.float32)[i],
                                   FT) for i in range(L)]),
    }

    in_maps = []
    i_idx = np.arange(W)
    for core in range(N_CORES):
        b, sb = core // 4, core % 4
        s0 = sb * T_OWN
        ext_pos = np.clip(np.arange(s0 - W, s0 + T_OWN + W), 0, S - 1)
        m = dict(shared)
        m["ids"] = np.ascontiguousarray(
            ids_full[b, ext_pos].reshape(12, P).T)
        m["pos"] = np.ascontiguousarray(emb_pos[ext_pos])
        # masks: global chunk gc, window key j in [0,768), query i in [0,256):
        #   key_abs = gc*W - W + j ; allowed = |j - W - i| <= W
        #             & 0 <= key_abs < S & attention_mask[b, key_abs]
        mlm = np.zeros((NCH, P, 512), np.float32)
        mrm = np.zeros((NCH, P, 512), np.float32)
        for c in range(NCH):
            gc = sb * NCH + c
            for kt2 in range(2):
                for mm_, j0 in ((mlm, 0), (mrm, 512)):
                    j = j0 + kt2 * P + np.arange(P)[:, None]
                    key_abs = gc * W - W + j
                    ok = (np.abs(j - W - i_idx[None, :]) <= W)
                    ok &= (key_abs >= 0) & (key_abs < S)
                    ok &= am[b, np.clip(key_abs, 0, S - 1)] > 0
                    mm_[c, :, kt2 * W:(kt2 + 1) * W] = ok
        m["ml"] = mlm.astype(ml_dtypes.bfloat16)
        m["mr"] = mrm.astype(ml_dtypes.bfloat16)
        # halo row ids into the gathered [4, 2, FT, 128, W] row table
        hid = np.zeros((2, FT, P), np.int64)
        for side in range(2):
            nb = sb - 1 if side == 0 else sb + 1
            if 0 <= nb <= 3:
                osd = 1 - side  # left halo <- neighbor's right block
                for ft in range(FT):
                    hid[side, ft] = ((nb * 2 + osd) * FT + ft) * P \
                        + np.arange(P)
            else:
                for ft in range(FT):
                    hid[side, ft] = ((sb * 2 + side) * FT + ft) * P \
                        + np.arange(P)
        m["halo_ids"] = np.ascontiguousarray(
            hid.reshape(12, P).T.astype(np.int32))
        in_maps.append(m)
    return in_maps


_NC_CACHE = {}


def get_nc(n_layers=L):
    if n_layers not in _NC_CACHE:
        _NC_CACHE[n_layers] = build_nc(n_layers)
    return _NC_CACHE[n_layers]


def run(inputs, n_layers=L, trace=False):
    nc = get_nc(n_layers)
    in_maps = prepare(inputs)
    res = bass_utils.run_bass_kernel_spmd(
        nc, in_maps, core_ids=list(range(N_CORES)), trace=trace)
    outs = np.empty((B, S, HD), np.float32)
    for core in range(N_CORES):
        b, sb = core // 4, core % 4
        ot = res.results[core]["out"]  # [FT, 128, T_OWN]
        outs[b, sb * T_OWN:(sb + 1) * T_OWN] = ot.reshape(HD, T_OWN).T
    return outs, res


def kernel(**inputs) -> np.ndarray:
    out, _ = run(inputs)
    return out



# revision 20
# speedup vs baseline: 1.0816x; 1.0147x over previous
"""Longformer layer stack (4 layers, sliding-window attention) on 8 TRN2 cores.

Sharding: data-parallel over batch (2) x sequence-parallel (4 blocks of 1024
tokens). Each core computes its sequence block; the banded attention needs a
W=256 token halo, exchanged between neighboring blocks with an AllGather after
each layer (layers 0-2). Residual stream kept transposed ([dmodel, tokens]) in
float32r; attention probs/values in bf16.

v2: the halo exchange is overlapped with compute (boundary-first FFN/LN2 kicks
the collective early; the next layer projects its own-token QKV and runs the
interior attention chunks while the collective is in flight, deferring only
halo K/V projection and the two edge chunks). Softmax normalization is batched
per chunk: denominators staged to SBUF, one reciprocal, a PE select-matmul
broadcast, and one multiply per (chunk, feature-tile) -- replacing per-head
reciprocal/partition-broadcast chains. LayerNorm uses a fused Rsqrt and a
full-width LN1.
"""
import sys

sys.path.insert(0, '/opt/trn_rl_repo')

import numpy as np
import ml_dtypes

import concourse.bass as bass
import concourse.mybir as mybir
import concourse.tile as tile
from concourse import bacc
from concourse import bass_utils

F32 = mybir.dt.float32
F32R = mybir.dt.float32r
BF16 = mybir.dt.bfloat16
I32 = mybir.dt.int32
AF = mybir.ActivationFunctionType
ALU = mybir.AluOpType

NH = 12          # heads
DH = 64          # head dim
HD = 768         # model dim
FF = 3072        # ffn dim
W = 256          # one-sided window
L = 4            # layers
B = 2
S = 4096
EPS = 1e-12
N_CORES = 8
T_OWN = 1024     # tokens per core
T_EXT = 1536     # with halos
FT = 6           # model-dim 128-tiles
FFT = 24         # ffn-dim 128-tiles
NCH = 4          # local chunks of 256 queries
P = 128


def _two_seg(ap2d, o1, o2, n):
    """[128, 2, n] view over cols {o1:o1+n} U {o2:o2+n} of a [128, N] AP."""
    base = ap2d[:, o1:o1 + n]
    return bass.AP(tensor=base.tensor, offset=base.offset,
                   ap=[base.ap[0], [o2 - o1, 2], [1, n]])


LN_BCAST_GPSIMD = False


def _ln_T(nc, sb, ps, r_aps, out_aps, ones_r, ones_row, eps_sb, s_ap, b_ap,
          ncols, stage_sbuf=False, out2_aps=None):
    """LayerNorm over the partition (feature) axis of transposed tiles.

    r_aps: list of FT fp32r SBUF APs [128, ncols] (input; normalized IN PLACE
    up to the final scale/bias which lands in out_aps). ncols in {512, 1024}.
    out_aps[ft] <- LN(r)*s + b. s_ap/b_ap: [128, FT] sbuf. mu/rstd broadcasts
    run as K=1 outer-product matmuls on the PE (gpsimd stays free for the
    collective); stage_sbuf copies them out of PSUM so the banks free early.
    """
    nseg = ncols // 512
    sxs = [ps.tile([1, 512], F32, tag=f"lnsx{j}", name=f"lnsx{j}")
           for j in range(nseg)]
    sqs = [ps.tile([1, 512], F32, tag=f"lnsq{j}", name=f"lnsq{j}")
           for j in range(nseg)]
    for ft in range(FT):
        sqt = sb.tile([P, ncols], F32R, tag="lnsqt", name="lnsqt", bufs=2)
        nc.scalar.activation(sqt[:], r_aps[ft], AF.Square)
        for j in range(nseg):
            cs = slice(j * 512, (j + 1) * 512)
            nc.tensor.matmul(sxs[j][:], ones_r[:, 0:1], r_aps[ft][:, cs],
                             start=(ft == 0), stop=(ft == FT - 1))
            nc.tensor.matmul(sqs[j][:], ones_r[:, 0:1], sqt[:, cs],
                             start=(ft == 0), stop=(ft == FT - 1))
    mu = sb.tile([1, ncols], F32R, tag="lnmu", name="lnmu")
    var = sb.tile([1, ncols], F32, tag="lnvar", name="lnvar")
    musq = sb.tile([1, ncols], F32, tag="lnmusq", name="lnmusq")
    rstd = sb.tile([1, ncols], F32R, tag="lnrstd", name="lnrstd")
    for j in range(nseg):
        cs = slice(j * 512, (j + 1) * 512)
        nc.scalar.activation(mu[:, cs], sxs[j][:], AF.Identity, scale=1.0 / HD)
    nc.vector.tensor_tensor(musq[:], mu[:], mu[:], op=ALU.mult)
    for j in range(nseg):
        cs = slice(j * 512, (j + 1) * 512)
        nc.vector.scalar_tensor_tensor(out=var[:, cs], in0=sqs[j][:],
                                       scalar=1.0 / HD, in1=musq[:, cs],
                                       op0=ALU.mult, op1=ALU.subtract)
    sd = musq  # musq is dead; reuse its slot for sd
    nc.scalar.activation(sd[:], var[:], AF.Sqrt, bias=eps_sb[0:1, :])
    with nc.allow_low_precision(reason="f32r recip holds full fp32 bits"):
        nc.vector.reciprocal(rstd[:], sd[:])
    if LN_BCAST_GPSIMD:
        mu_b = sb.tile([P, ncols], F32R, tag="lnmus", name="lnmubS")
        nc.gpsimd.partition_broadcast(mu_b[:], mu[:], channels=P)
        rstd_b = sb.tile([P, ncols], F32R, tag="lnrstds", name="lnrstdbS")
        nc.gpsimd.partition_broadcast(rstd_b[:], rstd[:], channels=P)
        for ft in range(FT):
            nc.vector.tensor_tensor(r_aps[ft], r_aps[ft], mu_b[:],
                                    op=ALU.subtract)
            nc.vector.tensor_tensor(r_aps[ft], r_aps[ft], rstd_b[:],
                                    op=ALU.mult)
            nc.scalar.activation(out_aps[ft], r_aps[ft], AF.Identity,
                                 scale=s_ap[:, ft:ft + 1],
                                 bias=b_ap[:, ft:ft + 1])
        return
    mu_b = ps.tile([P, ncols], F32, tag="lnmub", name="lnmub")
    rstd_b = ps.tile([P, ncols], F32, tag="lnrstdb", name="lnrstdb")
    for j in range(nseg):
        cs = slice(j * 512, (j + 1) * 512)
        nc.tensor.matmul(mu_b[:, cs], ones_row[:], mu[0:1, cs],
                         start=True, stop=True)
        nc.tensor.matmul(rstd_b[:, cs], ones_row[:], rstd[0:1, cs],
                         start=True, stop=True)
    if stage_sbuf:
        mu_s = sb.tile([P, ncols], F32, tag="lnmus", name="lnmus")
        nc.scalar.activation(mu_s[:], mu_b[:], AF.Identity)
        rstd_s = sb.tile([P, ncols], F32, tag="lnrstds", name="lnrstds")
        nc.vector.tensor_copy(rstd_s[:], rstd_b[:])
        mu_b, rstd_b = mu_s, rstd_s
    for ft in range(FT):
        nc.vector.tensor_tensor(r_aps[ft], r_aps[ft], mu_b[:], op=ALU.subtract)
        nc.vector.tensor_tensor(r_aps[ft], r_aps[ft], rstd_b[:], op=ALU.mult)
        # out2 (bf16 shadow) must read r BEFORE the in-place scale/bias ACT
        if out2_aps is not None:
            nc.vector.tensor_scalar(out=out2_aps[ft], in0=r_aps[ft],
                                    scalar1=s_ap[:, ft:ft + 1],
                                    scalar2=b_ap[:, ft:ft + 1],
                                    op0=ALU.mult, op1=ALU.add)
        nc.scalar.activation(out_aps[ft], r_aps[ft], AF.Identity,
                             scale=s_ap[:, ft:ft + 1], bias=b_ap[:, ft:ft + 1])


def build_nc(n_layers=L):
    nc = bacc.Bacc("TRN2", target_bir_lowering=False, debug=False,
                   num_devices=N_CORES)
    dt_ = nc.dram_tensor
    t = {}
    t["emb"] = dt_("emb_word", [32000, HD], F32, kind="ExternalInput").ap()
    t["ids"] = dt_("ids", [P, 12], I32, kind="ExternalInput").ap()
    t["pos"] = dt_("pos", [T_EXT, HD], F32, kind="ExternalInput").ap()
    t["eln_s"] = dt_("eln_s", [HD], F32, kind="ExternalInput").ap()
    t["eln_b"] = dt_("eln_b", [HD], F32, kind="ExternalInput").ap()
    t["wq"] = dt_("wq", [L, FT, P, FT, P], BF16, kind="ExternalInput").ap()
    t["wk"] = dt_("wk", [L, FT, P, FT, P], BF16, kind="ExternalInput").ap()
    t["wv"] = dt_("wv", [L, HD, HD], BF16, kind="ExternalInput").ap()
    t["wo"] = dt_("wo", [L, FT, P, FT, P], BF16, kind="ExternalInput").ap()
    t["w1"] = dt_("w1", [L, FFT, P, FT, P], BF16, kind="ExternalInput").ap()
    t["w2"] = dt_("w2", [L, FF, HD], BF16, kind="ExternalInput").ap()
    for nm in ["bq", "bk", "bo", "b2", "ls1", "lb1", "ls2", "lb2"]:
        t[nm] = dt_(nm, [L, P, FT], F32, kind="ExternalInput").ap()
    t["b1"] = dt_("b1", [L, P, FFT], F32, kind="ExternalInput").ap()
    t["ml"] = dt_("ml", [NCH, P, 512], BF16, kind="ExternalInput").ap()
    t["mr"] = dt_("mr", [NCH, P, 512], BF16, kind="ExternalInput").ap()
    t["halo_ids"] = dt_("halo_ids", [P, 12], I32, kind="ExternalInput").ap()
    t["out"] = dt_("out", [FT, P, T_OWN], F32, kind="ExternalOutput").ap()

    with tile.TileContext(nc) as tc:
        _build_body(nc, tc, n_layers, t)
    nc.compile()
    return nc


def _build_body(nc, tc, n_layers, t):
    from contextlib import ExitStack
    with ExitStack() as ctx:
        persist = ctx.enter_context(tc.tile_pool(name="persist", bufs=1))
        # residual stream, transposed, with halos: x[ft] = [128, T_EXT].
        # x is the f32r master; x_bf is its bf16 shadow used as matmul input
        # (PE can't mix 32-bit and 16-bit matmul operands).
        x = [persist.tile([P, T_OWN], F32R, tag=f"x{ft}", name=f"x{ft}") for ft in range(FT)]
        x_bf = [persist.tile([P, T_EXT], BF16, tag=f"xb{ft}", name=f"xb{ft}")
                for ft in range(FT)]
        ml_sb = [persist.tile([P, 512], BF16, tag=f"ml{c}", name=f"ml{c}") for c in range(NCH)]
        mr_sb = [persist.tile([P, 512], BF16, tag=f"mr{c}", name=f"mr{c}") for c in range(NCH)]
        for c in range(NCH):
            nc.sync.dma_start(ml_sb[c][:], t["ml"][c])
            nc.sync.dma_start(mr_sb[c][:], t["mr"][c])
        ones_f = persist.tile([P, 1], F32, tag="ones_f", name="ones_f")
        nc.vector.memset(ones_f[:], 1.0)
        ones_r = persist.tile([P, 1], F32R, tag="ones_r", name="ones_r")
        nc.scalar.activation(ones_r[:], ones_f[:], AF.Identity)
        ones_row_f = persist.tile([1, P], F32, tag="ones_row_f",
                                  name="ones_row_f")
        nc.vector.memset(ones_row_f[:], 1.0)
        ones_row = persist.tile([1, P], F32R, tag="ones_row", name="ones_row")
        nc.scalar.activation(ones_row[:], ones_row_f[:], AF.Identity)
        from concourse.masks import make_identity
        ident = persist.tile([P, P], F32, tag="ident", name="ident")
        make_identity(nc, ident[:])
        hid_sb = persist.tile([P, 12], I32, tag="hid", name="hid")
        nc.sync.dma_start(hid_sb[:], t["halo_ids"][:])
        eps_sb = persist.tile([P, 1], F32, tag="eps", name="eps")
        nc.vector.memset(eps_sb[:], EPS)
        # K=1 broadcast rows for the softmax-denominator outer products
        onesf33 = persist.tile([33, 64], F32, tag="o33f", name="onesf33")
        nc.vector.memset(onesf33[:], 0.0)
        nc.vector.memset(onesf33[0:1, :], 1.0)
        nc.vector.memset(onesf33[32:33, :], 1.0)
        ones33 = persist.tile([33, 64], F32R, tag="o33", name="ones33")
        nc.scalar.activation(ones33[:], onesf33[:], AF.Identity)

        # ---- embedding + LN -> x^T ----
        with tc.tile_pool(name="emb_sb", bufs=1) as esb, \
             tc.tile_pool(name="emb_sb2", bufs=2) as esb2, \
             tc.tile_pool(name="emb_ps", bufs=2, space="PSUM") as eps2:
            ids_sb = esb.tile([P, 12], I32, tag="ids", name="ids")
            nc.sync.dma_start(ids_sb[:], t["ids"][:])
            s_bc = esb.tile([P, HD], F32, tag="sbc", name="sbc")
            nc.sync.dma_start(s_bc[:], bass.AP(
                tensor=t["eln_s"].tensor, offset=0, ap=[[0, P], [1, HD]]))
            b_bc = esb.tile([P, HD], F32, tag="bbc", name="bbc")
            nc.sync.dma_start(b_bc[:], bass.AP(
                tensor=t["eln_b"].tensor, offset=0, ap=[[0, P], [1, HD]]))
            e = [esb.tile([P, HD], F32, tag=f"e{tt}", name=f"e{tt}") for tt in range(12)]
            for tt in range(12):
                nc.gpsimd.indirect_dma_start(
                    out=e[tt][:], out_offset=None, in_=t["emb"][:],
                    in_offset=bass.IndirectOffsetOnAxis(
                        ap=ids_sb[:, tt:tt + 1], axis=0))
                p_sb = esb2.tile([P, HD], F32, tag="pos", name="pos")
                nc.sync.dma_start(p_sb[:], t["pos"][tt * P:(tt + 1) * P, :])
                nc.vector.tensor_tensor(e[tt][:], e[tt][:], p_sb[:], op=ALU.add)
                stats = esb2.tile([P, 3, nc.vector.BN_STATS_DIM], F32,
                                  tag="bst", name="bst")
                er = e[tt][:].rearrange("p (g d) -> p g d", g=3)
                for g in range(3):
                    nc.vector.bn_stats(stats[:, g, :], er[:, g, :])
                mv = esb2.tile([P, nc.vector.BN_AGGR_DIM], F32, tag="bag", name="bag")
                nc.vector.bn_aggr(mv[:], stats[:])
                sd = esb2.tile([P, 1], F32, tag="bsd", name="bsd")
                nc.scalar.activation(sd[:], mv[:, 1:2], AF.Sqrt, bias=eps_sb[:])
                rstd = esb2.tile([P, 1], F32, tag="brstd", name="brstd")
                nc.vector.reciprocal(rstd[:], sd[:])
                nc.vector.tensor_scalar(out=e[tt][:], in0=e[tt][:],
                                        scalar1=mv[:, 0:1], scalar2=rstd[:],
                                        op0=ALU.subtract, op1=ALU.mult)
                nc.vector.tensor_tensor(e[tt][:], e[tt][:], s_bc[:], op=ALU.mult)
                nc.vector.tensor_tensor(e[tt][:], e[tt][:], b_bc[:], op=ALU.add)
            for ft in range(FT):
                tr = eps2.tile([P, T_EXT], F32, tag="tr", name="tr")
                for tt in range(12):
                    nc.tensor.transpose(tr[:, tt * P:(tt + 1) * P],
                                        e[tt][:, ft * P:(ft + 1) * P], ident[:])
                nc.scalar.activation(x[ft][:], tr[:, W:W + T_OWN],
                                     AF.Identity)
                nc.vector.tensor_copy(x_bf[ft][:], tr[:])

        for l in range(n_layers):
            _layer(nc, tc, t, l, x, x_bf, ml_sb, mr_sb, ones_r, ones_row,
                   eps_sb, hid_sb, ones33, exchange=(l < n_layers - 1),
                   deferred=(l > 0))

        for ft in range(FT):
            nc.gpsimd.dma_start(t["out"][ft], x[ft][:, 0:T_OWN])


USE_NEW_NORM = True


def _attn_chunk(nc, c, qT, kT, v, o, ml_sb, mr_sb, ones33, bsb, bps, opool,
                dpool):
    """Attention for one 256-query chunk, all 12 heads + normalization."""
    # DVE writes must start 32-partition-aligned: stage the 12 denominator
    # rows on partitions {0, 32} (6 heads each); a DMA then re-stages them
    # onto 12 partitions so the reciprocal runs wide (not 1536-serial)
    den_f = dpool.tile([33, 6 * W], F32, tag="denf", name="den_f", bufs=1)
    for h in range(NH):
        ft, po = h // 2, (h % 2) * 64
        sps = bps.tile([P, 6 * W], F32, tag="sps", name="sps")
        for w in range(6):
            nc.tensor.matmul(
                sps[:, w * W:(w + 1) * W],
                kT[ft][po:po + 64, c * W + w * P:c * W + (w + 1) * P],
                qT[ft][po:po + 64, c * W:(c + 1) * W],
                start=True, stop=True)
        ex = bsb.tile([P, 6 * W], BF16, tag="ex", name="ex")
        nc.scalar.activation(ex[:], sps[:], AF.Exp)
        nc.vector.tensor_tensor(ex[:, 0:512], ex[:, 0:512],
                                ml_sb[c][:], op=ALU.mult)
        nc.vector.tensor_tensor(ex[:, 1024:1536], ex[:, 1024:1536],
                                mr_sb[c][:], op=ALU.mult)
        ops = opool.tile([P, W], F32, tag="ops", name="ops")
        for w in range(6):
            nc.tensor.matmul(
                ops[:, :],
                v[c * 2 + w][:, 65 * h:65 * h + 128],
                ex[:, w * W:(w + 1) * W],
                start=(w == 0), stop=(w == 5))
        dfr = (h // 6) * 32
        nc.vector.tensor_copy(den_f[dfr:dfr + 1, (h % 6) * W:(h % 6 + 1) * W],
                              ops[64:65, :])
        # unnormalized attention out -> o slot (normalized in bulk below);
        # alternate engines to balance scalar (exp) vs vector (masks) load
        dst = o[ft][po:po + 64, c * W:(c + 1) * W]
        if h % 2 == 0:
            nc.scalar.activation(dst, ops[0:64, :], AF.Identity)
        else:
            nc.vector.tensor_copy(dst, ops[0:64, :])
    dinv_f = dpool.tile([33, 6 * W], F32R, tag="dinvf", name="dinv_f", bufs=1)
    den12 = dpool.tile([NH, W], F32, tag="den12", name="den12", bufs=1)
    dinv12 = dpool.tile([NH, W], F32R, tag="dinv12", name="dinv12", bufs=1)
    dfb = den_f[:]
    nc.sync.dma_start(
        den12[:], bass.AP(tensor=dfb.tensor, offset=dfb.offset,
                          ap=[[32 * 6 * W, 2], [W, 6], [1, W]]))
    with nc.allow_low_precision(reason="f32r recip holds full fp32 bits"):
        nc.vector.reciprocal(dinv12[:], den12[:])
    dib = dinv_f[:]
    nc.sync.dma_start(
        bass.AP(tensor=dib.tensor, offset=dib.offset,
                ap=[[32 * 6 * W, 2], [W, 6], [1, W]]),
        dinv12[:])
    for ft in range(FT):
        bc = opool.tile([P, 2 * W], F32, tag="ops", name="bc")
        for half in range(2):
            h = 2 * ft + half
            r, j = (h // 6) * 32, h % 6
            nc.tensor.matmul(bc[0:64, half * W:(half + 1) * W],
                             ones33[r:r + 1, :],
                             dinv_f[r:r + 1, j * W:(j + 1) * W],
                             start=True, stop=True)
        for half in range(2):
            po = half * 64
            nc.vector.tensor_tensor(
                o[ft][po:po + 64, c * W:(c + 1) * W],
                o[ft][po:po + 64, c * W:(c + 1) * W],
                bc[0:64, half * W:(half + 1) * W], op=ALU.mult)


def _ffn_pass(nc, tc, t, l, tag, y_mm, y_res, r2, b1_sb, b2_sb):
    """One FFN pass over 512 token-columns.

    y_mm: bf16 [128,512] APs (matmul rhs); y_res: f32r views of the same
    columns (residual add). r2: FT [128,512] f32r SBUF tiles for y + FFN(y).
    """
    from contextlib import ExitStack
    with ExitStack() as dctx:
        dsb = dctx.enter_context(tc.tile_pool(name=f"pd_sb{l}_{tag}", bufs=3))
        zps = dctx.enter_context(
            tc.tile_pool(name=f"pd_psz{l}_{tag}", bufs=1, space="PSUM"))
        fps = dctx.enter_context(
            tc.tile_pool(name=f"pd_psf{l}_{tag}", bufs=2, space="PSUM"))
        zp = [zps.tile([P, 512], F32, tag=f"z{mt}", name=f"z{mt}") for mt in range(FT)]
        for ms in range(FFT):
            w1_sb = dsb.tile([P, HD], BF16, tag="w1s", name="w1s")
            nc.sync.dma_start(w1_sb[:], t["w1"][l, ms])
            fp = fps.tile([P, 512], F32, tag="fp", name="fp")
            for kt in range(FT):
                nc.tensor.matmul(fp[:], w1_sb[:, kt * P:(kt + 1) * P],
                                 y_mm[kt], start=(kt == 0),
                                 stop=(kt == FT - 1))
            f_sb = dsb.tile([P, 512], BF16, tag="fsb", name="fsb")
            nc.scalar.activation(f_sb[:], fp[:], AF.Gelu,
                                 bias=b1_sb[:, ms:ms + 1])
            w2_sb = dsb.tile([P, HD], BF16, tag="w2s", name="w2s")
            nc.sync.dma_start(w2_sb[:],
                              t["w2"][l, ms * P:(ms + 1) * P, :])
            for mt in range(FT):
                nc.tensor.matmul(zp[mt][:],
                                 w2_sb[:, mt * P:(mt + 1) * P],
                                 f_sb[:], start=(ms == 0),
                                 stop=(ms == FFT - 1))
        for mt in range(FT):
            nc.vector.scalar_tensor_tensor(
                out=r2[mt][:], in0=zp[mt][:],
                scalar=b2_sb[:, mt:mt + 1], in1=y_res[mt],
                op0=ALU.add, op1=ALU.add)


def _layer(nc, tc, t, l, x, x_bf, ml_sb, mr_sb, ones_r, ones_row, eps_sb,
           hid_sb, ones33, exchange, deferred):
    from contextlib import ExitStack
    with ExitStack() as ctx:
        lsb = ctx.enter_context(tc.tile_pool(name=f"lsb{l}", bufs=1))

        def bias_tile(name, n=FT):
            bt = lsb.tile([P, n], F32, tag=f"b_{name}", name=f"b_{name}")
            nc.sync.dma_start(bt[:], t[name][l])
            return bt
        bq_sb = bias_tile("bq"); bk_sb = bias_tile("bk"); bo_sb = bias_tile("bo")
        b1_sb = bias_tile("b1", FFT); b2_sb = bias_tile("b2")
        ls1_sb = bias_tile("ls1"); lb1_sb = bias_tile("lb1")
        ls2_sb = bias_tile("ls2"); lb2_sb = bias_tile("lb2")

        qT = [lsb.tile([P, T_OWN], BF16, tag=f"qT{i}", name=f"qT{i}") for i in range(FT)]
        kT = [lsb.tile([P, T_EXT], BF16, tag=f"kT{i}", name=f"kT{i}") for i in range(FT)]
        # v[tt]: per head h, cols [65h:65h+64] = V_h, col 65h+64 = 1.0; the
        # attnout stationary is the contiguous 128-col block [65h:65h+128]
        # (fast LDWEIGHTS path); psum rows 65-127 are garbage and unread.
        v = [lsb.tile([P, NH * 65 + 128], BF16, tag=f"v{i}", name=f"v{i}")
             for i in range(12)]
        o = [lsb.tile([P, T_OWN], BF16, tag=f"o{i}", name=f"o{i}") for i in range(FT)]
        r2 = [lsb.tile([P, 512], F32R, tag=f"r2_{i}", name=f"r2_{i}") for i in range(FT)]
        r1 = [lsb.tile([P, T_OWN], F32R, tag=f"r1_{i}", name=f"r1_{i}")
              for i in range(FT)]
        # r1 doubles as y (LN1 output, written in place; f32r residual side);
        # qT doubles as y_bf (the bf16 LN1 shadow) -- dead after the scores
        y = [r1[i][:] for i in range(FT)]
        y_bf = qT

        # ---- phase A: QKV projections (own tokens; halo deferred if l>0) ----
        # kT col j = ext token j; own tokens are ext cols [W, W+T_OWN)
        wv_sb = ctx.enter_context(tc.tile_pool(name=f"pa_wv{l}", bufs=1))
        wv_all = [wv_sb.tile([P, HD], BF16, tag=f"wv{kt}", name=f"wv{kt}")
                  for kt in range(FT)]
        for kt in range(FT):
            nc.sync.dma_start(wv_all[kt][:],
                              t["wv"][l, kt * P:(kt + 1) * P, :])
        own_tt = range(2, 10) if deferred else range(12)
        halo_tt = (0, 1, 10, 11)
        own_segs = [(W, W + 512), (W + 512, W + T_OWN)]
        with tc.tile_pool(name=f"pa_sb{l}", bufs=3) as pa_sb, \
             tc.tile_pool(name=f"pa_ps{l}", bufs=4, space="PSUM") as pa_ps:
            for mt in range(FT):  # qT over own tokens
                wq_sb = pa_sb.tile([P, HD], BF16, tag="wqs", name="wqs")
                nc.sync.dma_start(wq_sb[:], t["wq"][l, mt])
                for h2 in range(2):
                    ps = pa_ps.tile([P, 512], F32, tag="pp", name="pp")
                    for kt in range(FT):
                        nc.tensor.matmul(
                            ps[:], wq_sb[:, kt * P:(kt + 1) * P],
                            x_bf[kt][:, W + h2 * 512:W + (h2 + 1) * 512],
                            start=(kt == 0), stop=(kt == FT - 1))
                    nc.scalar.activation(qT[mt][:, h2 * 512:(h2 + 1) * 512],
                                         ps[:], AF.Identity,
                                         bias=bq_sb[:, mt:mt + 1])
            for mt in range(FT):  # kT over own tokens (+ halo when l == 0)
                wk_sb = pa_sb.tile([P, HD], BF16, tag="wks", name="wks")
                nc.sync.dma_start(wk_sb[:], t["wk"][l, mt])
                segs = list(own_segs)
                if not deferred:
                    segs.append(None)  # halo 2-range seg
                for sg in segs:
                    ps = pa_ps.tile([P, 512], F32, tag="pp", name="pp")
                    rhs = ([x_bf[kt][:, sg[0]:sg[1]] for kt in range(FT)]
                           if sg is not None else
                           [_two_seg(x_bf[kt], 0, W + T_OWN, W)
                            for kt in range(FT)])
                    for kt in range(FT):
                        nc.tensor.matmul(
                            ps[:], wk_sb[:, kt * P:(kt + 1) * P], rhs[kt],
                            start=(kt == 0), stop=(kt == FT - 1))
                    dst = (kT[mt][:, sg[0]:sg[1]] if sg is not None
                           else _two_seg(kT[mt], 0, W + T_OWN, W))
                    nc.scalar.activation(dst, ps[:], AF.Identity,
                                         bias=bk_sb[:, mt:mt + 1])
            # v natural [tok, d]: lhsT = x slice, rhs = wv strip
            vtts = list(own_tt) + ([] if deferred else list(halo_tt))
            for tt in vtts:
                for hf in range(2):
                    ps = pa_ps.tile([P, 384], F32, tag="ppv", name="ppv",
                                    bufs=2)
                    for kt in range(FT):
                        nc.tensor.matmul(
                            ps[:], x_bf[kt][:, tt * P:(tt + 1) * P],
                            wv_all[kt][:, hf * 384:(hf + 1) * 384],
                            start=(kt == 0), stop=(kt == FT - 1))
                    vw = v[tt][:, hf * 390:(hf + 1) * 390].rearrange(
                        "p (h c) -> p h c", c=65)
                    nc.scalar.activation(
                        vw[:, :, 0:64],
                        ps[:].rearrange("p (h d) -> p h d", h=6), AF.Identity)
            for tt in range(12):
                vb = v[tt][:]
                ones_view = bass.AP(tensor=vb.tensor, offset=vb.offset + 64,
                                    ap=[vb.ap[0], [65, NH], [1, 1]])
                nc.vector.memset(ones_view, 1.0)

        # ---- phase B: banded attention (interior chunks first) ----
        with tc.tile_pool(name=f"pb_sb{l}", bufs=3) as bsb, \
             tc.tile_pool(name=f"pb_dsb{l}", bufs=2) as dpool, \
             tc.tile_pool(name=f"pb_ps{l}", bufs=2, space="PSUM") as bps, \
             tc.tile_pool(name=f"pb_ps2{l}", bufs=2, space="PSUM") as opool:
            for c in (1, 2):
                _attn_chunk(nc, c, qT, kT, v, o, ml_sb, mr_sb, ones33,
                            bsb, bps, opool, dpool)
            if deferred:
                # halo K/V: x halo cols were written by last layer's exchange;
                # psum borrowed from the sps slots (WAR-interleaves with the
                # score pipeline), weights re-streamed
                xh = [_two_seg(x_bf[kt], 0, W + T_OWN, W) for kt in range(FT)]
                for mt in range(FT):
                    wk_sb = bsb.tile([P, HD], BF16, tag="wkh", name="wkh", bufs=2)
                    nc.sync.dma_start(wk_sb[:], t["wk"][l, mt])
                    ps = bps.tile([P, 512], F32, tag="sps", name="hps")
                    for kt in range(FT):
                        nc.tensor.matmul(
                            ps[:], wk_sb[:, kt * P:(kt + 1) * P], xh[kt],
                            start=(kt == 0), stop=(kt == FT - 1))
                    nc.scalar.activation(_two_seg(kT[mt], 0, W + T_OWN, W),
                                         ps[:], AF.Identity,
                                         bias=bk_sb[:, mt:mt + 1])
                for tt in halo_tt:
                    for hf in range(2):
                        ps = bps.tile([P, 384], F32, tag="sps", name="hpsv")
                        for kt in range(FT):
                            nc.tensor.matmul(
                                ps[:], x_bf[kt][:, tt * P:(tt + 1) * P],
                                wv_all[kt][:, hf * 384:(hf + 1) * 384],
                                start=(kt == 0), stop=(kt == FT - 1))
                        vw = v[tt][:, hf * 390:(hf + 1) * 390].rearrange(
                            "p (h c) -> p h c", c=65)
                        nc.scalar.activation(
                            vw[:, :, 0:64],
                            ps[:].rearrange("p (h d) -> p h d", h=6),
                            AF.Identity)
            for c in (0, 3):
                _attn_chunk(nc, c, qT, kT, v, o, ml_sb, mr_sb, ones33,
                            bsb, bps, opool, dpool)

        # ---- phase C: O-proj + residual (r1 <- x + O@Wo + bo) ----
        with tc.tile_pool(name=f"pc_sb{l}", bufs=3) as csb, \
             tc.tile_pool(name=f"pc_ps{l}", bufs=4, space="PSUM") as cps:
            for mt in range(FT):
                wo_sb = csb.tile([P, HD], BF16, tag="wos", name="wos")
                nc.sync.dma_start(wo_sb[:], t["wo"][l, mt])
                for h2 in range(2):
                    ps = cps.tile([P, 512], F32, tag="ppo", name="ppo")
                    for kt in range(FT):
                        nc.tensor.matmul(
                            ps[:], wo_sb[:, kt * P:(kt + 1) * P],
                            o[kt][:, h2 * 512:(h2 + 1) * 512],
                            start=(kt == 0), stop=(kt == FT - 1))
                    nc.vector.scalar_tensor_tensor(
                        out=r1[mt][:, h2 * 512:(h2 + 1) * 512], in0=ps[:],
                        scalar=bo_sb[:, mt:mt + 1],
                        in1=x[mt][:, h2 * 512:(h2 + 1) * 512],
                        op0=ALU.add, op1=ALU.add)
        # ---- LN1 full-width: y <- LN(r1)*s+b (in place over r1/qT) ----
        with tc.tile_pool(name=f"ln1_sb{l}", bufs=1) as l1sb, \
             tc.tile_pool(name=f"ln1_ps{l}", bufs=1, space="PSUM") as l1ps:
            _ln_T(nc, l1sb, l1ps, [r1[ft][:] for ft in range(FT)],
                  [y[ft] for ft in range(FT)],
                  ones_r, ones_row, eps_sb, ls1_sb, lb1_sb, T_OWN,
                  out2_aps=[y_bf[ft][:] for ft in range(FT)])

        # ---- FFN boundary pass (token cols {0:W} U {768:1024} of own) ----
        y_bd = [_two_seg(y_bf[ft], 0, 3 * W, W) for ft in range(FT)]
        yr_bd = [_two_seg(r1[ft], 0, 3 * W, W) for ft in range(FT)]
        _ffn_pass(nc, tc, t, l, "bd", y_bd, yr_bd, r2, b1_sb, b2_sb)
        with tc.tile_pool(name=f"ln2b_sb{l}", bufs=1) as l2sb, \
             tc.tile_pool(name=f"ln2b_ps{l}", bufs=1, space="PSUM") as l2ps:
            _ln_T(nc, l2sb, l2ps, [r2[ft][:] for ft in range(FT)],
                  [_two_seg(x[ft], 0, 3 * W, W) for ft in range(FT)],
                  ones_r, ones_row, eps_sb, ls2_sb, lb2_sb, 512,
                  stage_sbuf=True,
                  out2_aps=[_two_seg(x_bf[ft], W, W + 3 * W, W)
                            for ft in range(FT)])

        # ---- halo exchange (overlaps FFN interior + next layer's QKV) ----
        if exchange:
            edram = ctx.enter_context(
                tc.tile_pool(name=f"pe_dram{l}", bufs=1, space="DRAM"))
            b_in = edram.tile([2, FT, P, W], BF16, tag="bin", name="bin")
            b_out = edram.tile([4 * 2 * FT * P, W], BF16, tag="bout",
                               name="bout")
            for ft in range(FT):
                nc.sync.dma_start(b_in[0, ft], x_bf[ft][:, W:2 * W])
                nc.sync.dma_start(b_in[1, ft], x_bf[ft][:, T_OWN:T_OWN + W])
            nc.gpsimd.collective_compute(
                "AllGather", ALU.bypass,
                replica_groups=[[0, 1, 2, 3], [4, 5, 6, 7]],
                ins=[b_in[:].opt()], outs=[b_out[:].opt()])
            for side in range(2):
                for ft in range(FT):
                    dst = (x_bf[ft][:, 0:W] if side == 0
                           else x_bf[ft][:, T_OWN + W:T_EXT])
                    nc.gpsimd.indirect_dma_start(
                        out=dst, out_offset=None, in_=b_out[:],
                        in_offset=bass.IndirectOffsetOnAxis(
                            ap=hid_sb[:, side * FT + ft:side * FT + ft + 1],
                            axis=0))

        # ---- FFN interior pass (token cols [W:768) of own) ----
        y_int = [y_bf[ft][:, W:3 * W] for ft in range(FT)]
        yr_int = [r1[ft][:, W:3 * W] for ft in range(FT)]
        _ffn_pass(nc, tc, t, l, "int", y_int, yr_int, r2, b1_sb, b2_sb)
        with tc.tile_pool(name=f"ln2i_sb{l}", bufs=1) as l2sb, \
             tc.tile_pool(name=f"ln2i_ps{l}", bufs=1, space="PSUM") as l2ps:
            _ln_T(nc, l2sb, l2ps, [r2[ft][:] for ft in range(FT)],
                  [x[ft][:, W:3 * W] for ft in range(FT)],
                  ones_r, ones_row, eps_sb, ls2_sb, lb2_sb, 512,
                  out2_aps=[x_bf[ft][:, 2 * W:4 * W] for ft in range(FT)])


# ---------------- host side ----------------

def _blocked(w, n_k, n_m):
    """[n_k*128, n_m*128] -> [n_m, 128, n_k, 128] (lhsT strips by out-tile)."""
    return np.ascontiguousarray(
        w.reshape(n_k, P, n_m, P).transpose(2, 1, 0, 3))


def _bias_lay(b, n):
    return np.ascontiguousarray(b.reshape(n, P).T)


def prepare(inputs):
    """Build per-core in_maps from full inputs."""
    ids_full = np.asarray(inputs["input_ids"]).astype(np.int32)
    am = np.asarray(inputs["attention_mask"]).astype(np.int32)
    emb_word = np.asarray(inputs["emb_word"], dtype=np.float32)
    emb_pos = np.asarray(inputs["emb_pos"], dtype=np.float32)
    Wq = np.asarray(inputs["Wq"], np.float32) / np.sqrt(DH)
    bq = np.asarray(inputs["bq"], np.float32) / np.sqrt(DH)
    Wk = np.asarray(inputs["Wk"], np.float32)
    bk = np.asarray(inputs["bk"], np.float32)
    Wv = np.asarray(inputs["Wv"], np.float32)
    bv = np.asarray(inputs["bv"], np.float32)
    Wo = np.asarray(inputs["Wo"], np.float32)
    bo = np.asarray(inputs["bo"], np.float32)
    W1 = np.asarray(inputs["W1"], np.float32)
    b1 = np.asarray(inputs["b1"], np.float32)
    W2 = np.asarray(inputs["W2"], np.float32)
    b2 = np.asarray(inputs["b2"], np.float32)
    assert np.all(am == 1), "general attention_mask needs mid-tile masks too"

    shared = {
        "emb_word": emb_word,
        "eln_s": np.asarray(inputs["emb_ln_s"], np.float32),
        "eln_b": np.asarray(inputs["emb_ln_b"], np.float32),
        "wq": np.stack([_blocked(Wq[i], FT, FT) for i in range(L)]).astype(
            ml_dtypes.bfloat16),
        "wk": np.stack([_blocked(Wk[i], FT, FT) for i in range(L)]).astype(
            ml_dtypes.bfloat16),
        "wv": Wv.astype(ml_dtypes.bfloat16),
        "wo": np.stack([_blocked(Wo[i], FT, FT) for i in range(L)]).astype(
            ml_dtypes.bfloat16),
        "w1": np.stack([_blocked(W1[i], FT, FFT) for i in range(L)]).astype(
            ml_dtypes.bfloat16),
        "w2": W2.astype(ml_dtypes.bfloat16),
        "bq": np.stack([_bias_lay(bq[i], FT) for i in range(L)]),
        "bk": np.stack([_bias_lay(bk[i], FT) for i in range(L)]),
        "bo": np.stack([_bias_lay(bv[i] @ Wo[i] + bo[i], FT)
                        for i in range(L)]),
        "b1": np.stack([_bias_lay(b1[i], FFT) for i in range(L)]),
        "b2": np.stack([_bias_lay(b2[i], FT) for i in range(L)]),
        "ls1": np.stack([_bias_lay(np.asarray(inputs["ln1_s"], np.float32)[i],
                                   FT) for i in range(L)]),
        "lb1": np.stack([_bias_lay(np.asarray(inputs["ln1_b"], np.float32)[i],
                                   FT) for i in range(L)]),
        "ls2": np.stack([_bias_lay(np.asarray(inputs["ln2_s"], np.float32)[i],
                                   FT) for i in range(L)]),
        "lb2": np.stack([_bias_lay(np.asarray(inputs["ln2_b"], np.float32)[i],
                                   FT) for i in range(L)]),
    }

    in_maps = []
    i_idx = np.arange(W)
    for core in range(N_CORES):
        b, sb = core // 4, core % 4
        s0 = sb * T_OWN
        ext_pos = np.clip(np.arange(s0 - W, s0 + T_OWN + W), 0, S - 1)
        m = dict(shared)
        m["ids"] = np.ascontiguousarray(
            ids_full[b, ext_pos].reshape(12, P).T)
        m["pos"] = np.ascontiguousarray(emb_pos[ext_pos])
        # masks: global chunk gc, window key j in [0,768), query i in [0,256):
        #   key_abs = gc*W - W + j ; allowed = |j - W - i| <= W
        #             & 0 <= key_abs < S & attention_mask[b, key_abs]
        mlm = np.zeros((NCH, P, 512), np.float32)
        mrm = np.zeros((NCH, P, 512), np.float32)
        for c in range(NCH):
            gc = sb * NCH + c
            for kt2 in range(2):
                for mm_, j0 in ((mlm, 0), (mrm, 512)):
                    j = j0 + kt2 * P + np.arange(P)[:, None]
                    key_abs = gc * W - W + j
                    ok = (np.abs(j - W - i_idx[None, :]) <= W)
                    ok &= (key_abs >= 0) & (key_abs < S)
                    ok &= am[b, np.clip(key_abs, 0, S - 1)] > 0
                    mm_[c, :, kt2 * W:(kt2 + 1) * W] = ok
        m["ml"] = mlm.astype(ml_dtypes.bfloat16)
        m["mr"] = mrm.astype(ml_dtypes.bfloat16)
        # halo row ids into the gathered [4, 2, FT, 128, W] row table
        hid = np.zeros((2, FT, P), np.int64)
        for side in range(2):
            nb = sb - 1 if side == 0 else sb + 1
            if 0 <= nb <= 3:
                osd = 1 - side  # left halo <- neighbor's right block
                for ft in range(FT):
                    hid[side, ft] = ((nb * 2 + osd) * FT + ft) * P \
                        + np.arange(P)
            else:
                for ft in range(FT):
                    hid[side, ft] = ((sb * 2 + side) * FT + ft) * P \
                        + np.arange(P)
        m["halo_ids"] = np.ascontiguousarray(
            hid.reshape(12, P).T.astype(np.int32))
        in_maps.append(m)
    return in_maps


_NC_CACHE = {}


def get_nc(n_layers=L):
    if n_layers not in _NC_CACHE:
        _NC_CACHE[n_layers] = build_nc(n_layers)
    return _NC_CACHE[n_layers]


def run(inputs, n_layers=L, trace=False):
    nc = get_nc(n_layers)
    in_maps = prepare(inputs)
    res = bass_utils.run_bass_kernel_spmd(
        nc, in_maps, core_ids=list(range(N_CORES)), trace=trace)
    outs = np.empty((B, S, HD), np.float32)
    for core in range(N_CORES):
        b, sb = core // 4, core % 4
        ot = res.results[core]["out"]  # [FT, 128, T_OWN]
        outs[b, sb * T_OWN:(sb + 1) * T_OWN] = ot.reshape(HD, T_OWN).T
    return outs, res


def kernel(**inputs) -> np.ndarray:
    out, _ = run(inputs)
    return out



# revision 21
# speedup vs baseline: 1.1871x; 1.0975x over previous
"""Longformer layer stack (4 layers, sliding-window attention) on 8 TRN2 cores.

Sharding: data-parallel over batch (2) x sequence-parallel (4 blocks of 1024
tokens). Each core computes its sequence block; the banded attention needs a
W=256 token halo, exchanged between neighboring blocks with an AllGather after
each layer (layers 0-2). Residual stream kept transposed ([dmodel, tokens]) in
float32r; attention probs/values in bf16.

v2: the halo exchange is overlapped with compute (boundary-first FFN/LN2 kicks
the collective early; the next layer projects its own-token QKV and runs the
interior attention chunks while the collective is in flight, deferring only
halo K/V projection and the two edge chunks). Softmax normalization is batched
per chunk: denominators staged to SBUF, one reciprocal, a PE select-matmul
broadcast, and one multiply per (chunk, feature-tile) -- replacing per-head
reciprocal/partition-broadcast chains. LayerNorm uses a fused Rsqrt and a
full-width LN1.
"""
import sys

sys.path.insert(0, '/opt/trn_rl_repo')

import numpy as np
import ml_dtypes

import concourse.bass as bass
import concourse.mybir as mybir
import concourse.tile as tile
from concourse import bacc
from concourse import bass_utils

F32 = mybir.dt.float32
F32R = mybir.dt.float32r
BF16 = mybir.dt.bfloat16
I32 = mybir.dt.int32
AF = mybir.ActivationFunctionType
ALU = mybir.AluOpType

NH = 12          # heads
DH = 64          # head dim
HD = 768         # model dim
FF = 3072        # ffn dim
W = 256          # one-sided window
L = 4            # layers
B = 2
S = 4096
EPS = 1e-12
N_CORES = 8
T_OWN = 1024     # tokens per core
T_EXT = 1536     # with halos
FT = 6           # model-dim 128-tiles
FFT = 24         # ffn-dim 128-tiles
NCH = 4          # local chunks of 256 queries
P = 128


def _two_seg(ap2d, o1, o2, n):
    """[128, 2, n] view over cols {o1:o1+n} U {o2:o2+n} of a [128, N] AP."""
    base = ap2d[:, o1:o1 + n]
    return bass.AP(tensor=base.tensor, offset=base.offset,
                   ap=[base.ap[0], [o2 - o1, 2], [1, n]])


LN_BCAST_GPSIMD = False


def _ln_T(nc, sb, ps, r_aps, out_aps, ones_r, ones_row, eps_sb, s_ap, b_ap,
          ncols, stage_sbuf=False, out2_aps=None):
    """LayerNorm over the partition (feature) axis of transposed tiles.

    r_aps: list of FT fp32r SBUF APs [128, ncols] (input; normalized IN PLACE
    up to the final scale/bias which lands in out_aps). ncols in {512, 1024}.
    out_aps[ft] <- LN(r)*s + b. s_ap/b_ap: [128, FT] sbuf. mu/rstd broadcasts
    run as K=1 outer-product matmuls on the PE (gpsimd stays free for the
    collective); stage_sbuf copies them out of PSUM so the banks free early.
    """
    nseg = ncols // 512
    sxs = [ps.tile([1, 512], F32, tag=f"lnsx{j}", name=f"lnsx{j}")
           for j in range(nseg)]
    sqs = [ps.tile([1, 512], F32, tag=f"lnsq{j}", name=f"lnsq{j}")
           for j in range(nseg)]
    for ft in range(FT):
        sqt = sb.tile([P, ncols], F32R, tag="lnsqt", name="lnsqt", bufs=2)
        nc.scalar.activation(sqt[:], r_aps[ft], AF.Square)
        for j in range(nseg):
            cs = slice(j * 512, (j + 1) * 512)
            rseg = r_aps[ft] if nseg == 1 else r_aps[ft][:, cs]
            nc.tensor.matmul(sxs[j][:], ones_r[:, 0:1], rseg,
                             start=(ft == 0), stop=(ft == FT - 1))
            nc.tensor.matmul(sqs[j][:], ones_r[:, 0:1], sqt[:, cs],
                             start=(ft == 0), stop=(ft == FT - 1))
    mu = sb.tile([1, ncols], F32R, tag="lnmu", name="lnmu")
    var = sb.tile([1, ncols], F32, tag="lnvar", name="lnvar")
    musq = sb.tile([1, ncols], F32, tag="lnmusq", name="lnmusq")
    rstd = sb.tile([1, ncols], F32R, tag="lnrstd", name="lnrstd")
    for j in range(nseg):
        cs = slice(j * 512, (j + 1) * 512)
        nc.scalar.activation(mu[:, cs], sxs[j][:], AF.Identity, scale=1.0 / HD)
    nc.vector.tensor_tensor(musq[:], mu[:], mu[:], op=ALU.mult)
    for j in range(nseg):
        cs = slice(j * 512, (j + 1) * 512)
        nc.vector.scalar_tensor_tensor(out=var[:, cs], in0=sqs[j][:],
                                       scalar=1.0 / HD, in1=musq[:, cs],
                                       op0=ALU.mult, op1=ALU.subtract)
    sd = musq  # musq is dead; reuse its slot for sd
    nc.scalar.activation(sd[:], var[:], AF.Sqrt, bias=eps_sb[0:1, :])
    with nc.allow_low_precision(reason="f32r recip holds full fp32 bits"):
        nc.vector.reciprocal(rstd[:], sd[:])
    if LN_BCAST_GPSIMD:
        mu_b = sb.tile([P, ncols], F32R, tag="lnmus", name="lnmubS")
        nc.gpsimd.partition_broadcast(mu_b[:], mu[:], channels=P)
        rstd_b = sb.tile([P, ncols], F32R, tag="lnrstds", name="lnrstdbS")
        nc.gpsimd.partition_broadcast(rstd_b[:], rstd[:], channels=P)
        for ft in range(FT):
            nc.vector.tensor_tensor(r_aps[ft], r_aps[ft], mu_b[:],
                                    op=ALU.subtract)
            nc.vector.tensor_tensor(r_aps[ft], r_aps[ft], rstd_b[:],
                                    op=ALU.mult)
            nc.scalar.activation(out_aps[ft], r_aps[ft], AF.Identity,
                                 scale=s_ap[:, ft:ft + 1],
                                 bias=b_ap[:, ft:ft + 1])
        return
    mu_b = ps.tile([P, ncols], F32, tag="lnmub", name="lnmub")
    rstd_b = ps.tile([P, ncols], F32, tag="lnrstdb", name="lnrstdb")
    for j in range(nseg):
        cs = slice(j * 512, (j + 1) * 512)
        nc.tensor.matmul(mu_b[:, cs], ones_row[:], mu[0:1, cs],
                         start=True, stop=True)
        nc.tensor.matmul(rstd_b[:, cs], ones_row[:], rstd[0:1, cs],
                         start=True, stop=True)
    if stage_sbuf:
        mu_s = sb.tile([P, ncols], F32, tag="lnmus", name="lnmus")
        nc.scalar.activation(mu_s[:], mu_b[:], AF.Identity)
        rstd_s = sb.tile([P, ncols], F32, tag="lnrstds", name="lnrstds")
        nc.vector.tensor_copy(rstd_s[:], rstd_b[:])
        mu_b, rstd_b = mu_s, rstd_s
    for ft in range(FT):
        nc.vector.tensor_tensor(r_aps[ft], r_aps[ft], mu_b[:], op=ALU.subtract)
        nc.vector.tensor_tensor(r_aps[ft], r_aps[ft], rstd_b[:], op=ALU.mult)
        # out2 (bf16 shadow) must read r BEFORE the in-place scale/bias ACT
        if out2_aps is not None:
            nc.vector.tensor_scalar(out=out2_aps[ft], in0=r_aps[ft],
                                    scalar1=s_ap[:, ft:ft + 1],
                                    scalar2=b_ap[:, ft:ft + 1],
                                    op0=ALU.mult, op1=ALU.add)
        nc.scalar.activation(out_aps[ft], r_aps[ft], AF.Identity,
                             scale=s_ap[:, ft:ft + 1], bias=b_ap[:, ft:ft + 1])


def build_nc(n_layers=L):
    nc = bacc.Bacc("TRN2", target_bir_lowering=False, debug=False,
                   num_devices=N_CORES)
    dt_ = nc.dram_tensor
    t = {}
    t["emb"] = dt_("emb_word", [32000, HD], F32, kind="ExternalInput").ap()
    t["ids"] = dt_("ids", [P, 12], I32, kind="ExternalInput").ap()
    t["pos"] = dt_("pos", [T_EXT, HD], F32, kind="ExternalInput").ap()
    t["eln_s"] = dt_("eln_s", [HD], F32, kind="ExternalInput").ap()
    t["eln_b"] = dt_("eln_b", [HD], F32, kind="ExternalInput").ap()
    t["wq"] = dt_("wq", [L, FT, P, FT, P], BF16, kind="ExternalInput").ap()
    t["wk"] = dt_("wk", [L, FT, P, FT, P], BF16, kind="ExternalInput").ap()
    t["wv"] = dt_("wv", [L, HD, HD], BF16, kind="ExternalInput").ap()
    t["wo"] = dt_("wo", [L, FT, P, FT, P], BF16, kind="ExternalInput").ap()
    t["w1"] = dt_("w1", [L, FFT, P, FT, P], BF16, kind="ExternalInput").ap()
    t["w2"] = dt_("w2", [L, FF, HD], BF16, kind="ExternalInput").ap()
    for nm in ["bq", "bk", "bo", "b2", "ls1", "lb1", "ls2", "lb2"]:
        t[nm] = dt_(nm, [L, P, FT], F32, kind="ExternalInput").ap()
    t["b1"] = dt_("b1", [L, P, FFT], F32, kind="ExternalInput").ap()
    t["ml"] = dt_("ml", [NCH, P, 512], BF16, kind="ExternalInput").ap()
    t["mr"] = dt_("mr", [NCH, P, 512], BF16, kind="ExternalInput").ap()
    t["halo_ids"] = dt_("halo_ids", [P, 12], I32, kind="ExternalInput").ap()
    t["out"] = dt_("out", [FT, P, T_OWN], F32, kind="ExternalOutput").ap()

    with tile.TileContext(nc) as tc:
        _build_body(nc, tc, n_layers, t)
    nc.compile()
    return nc


def _build_body(nc, tc, n_layers, t):
    from contextlib import ExitStack
    with ExitStack() as ctx:
        persist = ctx.enter_context(tc.tile_pool(name="persist", bufs=1))
        # residual stream, transposed, with halos: x[ft] = [128, T_EXT].
        # x is the f32r master; x_bf is its bf16 shadow used as matmul input
        # (PE can't mix 32-bit and 16-bit matmul operands).
        x = [persist.tile([P, T_OWN], F32R, tag=f"x{ft}", name=f"x{ft}") for ft in range(FT)]
        x_bf = [persist.tile([P, T_EXT], BF16, tag=f"xb{ft}", name=f"xb{ft}")
                for ft in range(FT)]
        ml_sb = [persist.tile([P, 512], BF16, tag=f"ml{c}", name=f"ml{c}") for c in range(NCH)]
        mr_sb = [persist.tile([P, 512], BF16, tag=f"mr{c}", name=f"mr{c}") for c in range(NCH)]
        for c in range(NCH):
            nc.sync.dma_start(ml_sb[c][:], t["ml"][c])
            nc.sync.dma_start(mr_sb[c][:], t["mr"][c])
        ones_f = persist.tile([P, 1], F32, tag="ones_f", name="ones_f")
        nc.vector.memset(ones_f[:], 1.0)
        ones_r = persist.tile([P, 1], F32R, tag="ones_r", name="ones_r")
        nc.scalar.activation(ones_r[:], ones_f[:], AF.Identity)
        ones_row_f = persist.tile([1, P], F32, tag="ones_row_f",
                                  name="ones_row_f")
        nc.vector.memset(ones_row_f[:], 1.0)
        ones_row = persist.tile([1, P], F32R, tag="ones_row", name="ones_row")
        nc.scalar.activation(ones_row[:], ones_row_f[:], AF.Identity)
        from concourse.masks import make_identity
        ident = persist.tile([P, P], F32, tag="ident", name="ident")
        make_identity(nc, ident[:])
        hid_sb = persist.tile([P, 12], I32, tag="hid", name="hid")
        nc.sync.dma_start(hid_sb[:], t["halo_ids"][:])
        eps_sb = persist.tile([P, 1], F32, tag="eps", name="eps")
        nc.vector.memset(eps_sb[:], EPS)
        # K=1 broadcast rows for the softmax-denominator outer products
        onesf33 = persist.tile([33, 64], F32, tag="o33f", name="onesf33")
        nc.vector.memset(onesf33[:], 0.0)
        nc.vector.memset(onesf33[0:1, :], 1.0)
        nc.vector.memset(onesf33[32:33, :], 1.0)
        ones33 = persist.tile([33, 64], F32R, tag="o33", name="ones33")
        nc.scalar.activation(ones33[:], onesf33[:], AF.Identity)

        # ---- embedding + LN -> x^T ----
        with tc.tile_pool(name="emb_sb", bufs=1) as esb, \
             tc.tile_pool(name="emb_sb2", bufs=2) as esb2, \
             tc.tile_pool(name="emb_ps", bufs=2, space="PSUM") as eps2:
            ids_sb = esb.tile([P, 12], I32, tag="ids", name="ids")
            nc.sync.dma_start(ids_sb[:], t["ids"][:])
            s_bc = esb.tile([P, HD], F32, tag="sbc", name="sbc")
            nc.sync.dma_start(s_bc[:], bass.AP(
                tensor=t["eln_s"].tensor, offset=0, ap=[[0, P], [1, HD]]))
            b_bc = esb.tile([P, HD], F32, tag="bbc", name="bbc")
            nc.sync.dma_start(b_bc[:], bass.AP(
                tensor=t["eln_b"].tensor, offset=0, ap=[[0, P], [1, HD]]))
            e = [esb.tile([P, HD], F32, tag=f"e{tt}", name=f"e{tt}") for tt in range(12)]
            for tt in range(12):
                nc.gpsimd.indirect_dma_start(
                    out=e[tt][:], out_offset=None, in_=t["emb"][:],
                    in_offset=bass.IndirectOffsetOnAxis(
                        ap=ids_sb[:, tt:tt + 1], axis=0))
                p_sb = esb2.tile([P, HD], F32, tag="pos", name="pos")
                nc.sync.dma_start(p_sb[:], t["pos"][tt * P:(tt + 1) * P, :])
                nc.vector.tensor_tensor(e[tt][:], e[tt][:], p_sb[:], op=ALU.add)
                stats = esb2.tile([P, 3, nc.vector.BN_STATS_DIM], F32,
                                  tag="bst", name="bst")
                er = e[tt][:].rearrange("p (g d) -> p g d", g=3)
                for g in range(3):
                    nc.vector.bn_stats(stats[:, g, :], er[:, g, :])
                mv = esb2.tile([P, nc.vector.BN_AGGR_DIM], F32, tag="bag", name="bag")
                nc.vector.bn_aggr(mv[:], stats[:])
                sd = esb2.tile([P, 1], F32, tag="bsd", name="bsd")
                nc.scalar.activation(sd[:], mv[:, 1:2], AF.Sqrt, bias=eps_sb[:])
                rstd = esb2.tile([P, 1], F32, tag="brstd", name="brstd")
                nc.vector.reciprocal(rstd[:], sd[:])
                nc.vector.tensor_scalar(out=e[tt][:], in0=e[tt][:],
                                        scalar1=mv[:, 0:1], scalar2=rstd[:],
                                        op0=ALU.subtract, op1=ALU.mult)
                nc.vector.tensor_tensor(e[tt][:], e[tt][:], s_bc[:], op=ALU.mult)
                nc.vector.tensor_tensor(e[tt][:], e[tt][:], b_bc[:], op=ALU.add)
            for ft in range(FT):
                tr = eps2.tile([P, T_EXT], F32, tag="tr", name="tr")
                for tt in range(12):
                    nc.tensor.transpose(tr[:, tt * P:(tt + 1) * P],
                                        e[tt][:, ft * P:(ft + 1) * P], ident[:])
                nc.scalar.activation(x[ft][:], tr[:, W:W + T_OWN],
                                     AF.Identity)
                nc.vector.tensor_copy(x_bf[ft][:], tr[:])

        for l in range(n_layers):
            _layer(nc, tc, t, l, x, x_bf, ml_sb, mr_sb, ones_r, ones_row,
                   eps_sb, hid_sb, ones33, exchange=(l < n_layers - 1),
                   deferred=(l > 0))

        for ft in range(FT):
            nc.gpsimd.dma_start(t["out"][ft], x[ft][:, 0:T_OWN])


USE_NEW_NORM = True


def _attn_chunk(nc, c, qT, kT, v, o, ml_sb, mr_sb, ones33, bsb, bps, opool,
                dpool):
    """Attention for one 256-query chunk, all 12 heads + normalization."""
    # DVE writes must start 32-partition-aligned: stage the 12 denominator
    # rows on partitions {0, 32} (6 heads each); a DMA then re-stages them
    # onto 12 partitions so the reciprocal runs wide (not 1536-serial)
    den_f = dpool.tile([33, 6 * W], F32, tag="denf", name="den_f", bufs=1)
    for h in range(NH):
        ft, po = h // 2, (h % 2) * 64
        sps = bps.tile([P, 6 * W], F32, tag="sps", name="sps")
        for w in range(6):
            nc.tensor.matmul(
                sps[:, w * W:(w + 1) * W],
                kT[ft][po:po + 64, c * W + w * P:c * W + (w + 1) * P],
                qT[ft][po:po + 64, c * W:(c + 1) * W],
                start=True, stop=True)
        ex = bsb.tile([P, 6 * W], BF16, tag="ex", name="ex")
        nc.scalar.activation(ex[:], sps[:], AF.Exp)
        nc.vector.tensor_tensor(ex[:, 0:512], ex[:, 0:512],
                                ml_sb[c][:], op=ALU.mult)
        nc.vector.tensor_tensor(ex[:, 1024:1536], ex[:, 1024:1536],
                                mr_sb[c][:], op=ALU.mult)
        ops = opool.tile([P, W], F32, tag="ops", name="ops")
        for w in range(6):
            nc.tensor.matmul(
                ops[:, :],
                v[c * 2 + w][:, 65 * h:65 * h + 128],
                ex[:, w * W:(w + 1) * W],
                start=(w == 0), stop=(w == 5))
        dfr = (h // 6) * 32
        nc.vector.tensor_copy(den_f[dfr:dfr + 1, (h % 6) * W:(h % 6 + 1) * W],
                              ops[64:65, :])
        # unnormalized attention out -> o slot (normalized in bulk below);
        # alternate engines to balance scalar (exp) vs vector (masks) load
        dst = o[ft][po:po + 64, c * W:(c + 1) * W]
        if h % 2 == 0:
            nc.scalar.activation(dst, ops[0:64, :], AF.Identity)
        else:
            nc.vector.tensor_copy(dst, ops[0:64, :])
    dinv_f = dpool.tile([33, 6 * W], F32R, tag="dinvf", name="dinv_f", bufs=1)
    den12 = dpool.tile([NH, W], F32, tag="den12", name="den12", bufs=1)
    dinv12 = dpool.tile([NH, W], F32R, tag="dinv12", name="dinv12", bufs=1)
    dfb = den_f[:]
    nc.sync.dma_start(
        den12[:], bass.AP(tensor=dfb.tensor, offset=dfb.offset,
                          ap=[[32 * 6 * W, 2], [W, 6], [1, W]]))
    with nc.allow_low_precision(reason="f32r recip holds full fp32 bits"):
        nc.vector.reciprocal(dinv12[:], den12[:])
    dib = dinv_f[:]
    nc.sync.dma_start(
        bass.AP(tensor=dib.tensor, offset=dib.offset,
                ap=[[32 * 6 * W, 2], [W, 6], [1, W]]),
        dinv12[:])
    for ft in range(FT):
        bc = opool.tile([P, 2 * W], F32, tag="ops", name="bc")
        for half in range(2):
            h = 2 * ft + half
            r, j = (h // 6) * 32, h % 6
            nc.tensor.matmul(bc[0:64, half * W:(half + 1) * W],
                             ones33[r:r + 1, :],
                             dinv_f[r:r + 1, j * W:(j + 1) * W],
                             start=True, stop=True)
        for half in range(2):
            po = half * 64
            nc.vector.tensor_tensor(
                o[ft][po:po + 64, c * W:(c + 1) * W],
                o[ft][po:po + 64, c * W:(c + 1) * W],
                bc[0:64, half * W:(half + 1) * W], op=ALU.mult)


def _ffn_pass(nc, tc, t, l, tag, y_mm, y_res, r2, b1_sb, b2_sb):
    """One FFN pass over 512 token-columns.

    y_mm: bf16 [128,512] APs (matmul rhs); y_res: f32r views of the same
    columns (residual add). r2: FT [128,512] f32r SBUF tiles for y + FFN(y).
    """
    from contextlib import ExitStack
    with ExitStack() as dctx:
        dsb = dctx.enter_context(tc.tile_pool(name=f"pd_sb{l}_{tag}", bufs=3))
        zps = dctx.enter_context(
            tc.tile_pool(name=f"pd_psz{l}_{tag}", bufs=1, space="PSUM"))
        fps = dctx.enter_context(
            tc.tile_pool(name=f"pd_psf{l}_{tag}", bufs=2, space="PSUM"))
        zp = [zps.tile([P, 512], F32, tag=f"z{mt}", name=f"z{mt}") for mt in range(FT)]
        for ms in range(FFT):
            w1_sb = dsb.tile([P, HD], BF16, tag="w1s", name="w1s")
            nc.sync.dma_start(w1_sb[:], t["w1"][l, ms])
            fp = fps.tile([P, 512], F32, tag="fp", name="fp")
            for kt in range(FT):
                nc.tensor.matmul(fp[:], w1_sb[:, kt * P:(kt + 1) * P],
                                 y_mm[kt], start=(kt == 0),
                                 stop=(kt == FT - 1))
            f_sb = dsb.tile([P, 512], BF16, tag="fsb", name="fsb")
            nc.scalar.activation(f_sb[:], fp[:], AF.Gelu,
                                 bias=b1_sb[:, ms:ms + 1])
            w2_sb = dsb.tile([P, HD], BF16, tag="w2s", name="w2s")
            nc.sync.dma_start(w2_sb[:],
                              t["w2"][l, ms * P:(ms + 1) * P, :])
            for mt in range(FT):
                nc.tensor.matmul(zp[mt][:],
                                 w2_sb[:, mt * P:(mt + 1) * P],
                                 f_sb[:], start=(ms == 0),
                                 stop=(ms == FFT - 1))
        for mt in range(FT):
            nc.vector.scalar_tensor_tensor(
                out=r2[mt][:], in0=zp[mt][:],
                scalar=b2_sb[:, mt:mt + 1], in1=y_res[mt],
                op0=ALU.add, op1=ALU.add)


def _layer(nc, tc, t, l, x, x_bf, ml_sb, mr_sb, ones_r, ones_row, eps_sb,
           hid_sb, ones33, exchange, deferred):
    from contextlib import ExitStack
    with ExitStack() as ctx:
        lsb = ctx.enter_context(tc.tile_pool(name=f"lsb{l}", bufs=1))

        def bias_tile(name, n=FT):
            bt = lsb.tile([P, n], F32, tag=f"b_{name}", name=f"b_{name}")
            nc.sync.dma_start(bt[:], t[name][l])
            return bt
        bq_sb = bias_tile("bq"); bk_sb = bias_tile("bk"); bo_sb = bias_tile("bo")
        b1_sb = bias_tile("b1", FFT); b2_sb = bias_tile("b2")
        ls1_sb = bias_tile("ls1"); lb1_sb = bias_tile("lb1")
        ls2_sb = bias_tile("ls2"); lb2_sb = bias_tile("lb2")

        qT = [lsb.tile([P, T_OWN], BF16, tag=f"qT{i}", name=f"qT{i}") for i in range(FT)]
        kT = [lsb.tile([P, T_EXT], BF16, tag=f"kT{i}", name=f"kT{i}") for i in range(FT)]
        # v[tt]: per head h, cols [65h:65h+64] = V_h, col 65h+64 = 1.0; the
        # attnout stationary is the contiguous 128-col block [65h:65h+128]
        # (fast LDWEIGHTS path); psum rows 65-127 are garbage and unread.
        v = [lsb.tile([P, NH * 65 + 128], BF16, tag=f"v{i}", name=f"v{i}")
             for i in range(12)]
        o = [lsb.tile([P, T_OWN], BF16, tag=f"o{i}", name=f"o{i}") for i in range(FT)]
        r2b = [lsb.tile([P, 512], F32R, tag=f"r2b{i}", name=f"r2b{i}") for i in range(FT)]
        r2i = [lsb.tile([P, 512], F32R, tag=f"r2i{i}", name=f"r2i{i}") for i in range(FT)]
        r1 = [lsb.tile([P, T_OWN], F32R, tag=f"r1_{i}", name=f"r1_{i}")
              for i in range(FT)]
        # r1 doubles as y (LN1 output, written in place; f32r residual side);
        # qT doubles as y_bf (the bf16 LN1 shadow) -- dead after the scores
        y = [r1[i][:] for i in range(FT)]
        y_bf = qT

        # ---- phase A: QKV projections (own tokens; halo deferred if l>0) ----
        # kT col j = ext token j; own tokens are ext cols [W, W+T_OWN)
        wv_sb = ctx.enter_context(tc.tile_pool(name=f"pa_wv{l}", bufs=1))
        wv_all = [wv_sb.tile([P, HD], BF16, tag=f"wv{kt}", name=f"wv{kt}")
                  for kt in range(FT)]
        for kt in range(FT):
            nc.sync.dma_start(wv_all[kt][:],
                              t["wv"][l, kt * P:(kt + 1) * P, :])
        own_tt = range(2, 10) if deferred else range(12)
        halo_tt = (0, 1, 10, 11)
        own_segs = [(W, W + 512), (W + 512, W + T_OWN)]
        with tc.tile_pool(name=f"pa_sb{l}", bufs=3) as pa_sb, \
             tc.tile_pool(name=f"pa_ps{l}", bufs=4, space="PSUM") as pa_ps:
            for mt in range(FT):  # qT over own tokens
                wq_sb = pa_sb.tile([P, HD], BF16, tag="wqs", name="wqs")
                nc.sync.dma_start(wq_sb[:], t["wq"][l, mt])
                for h2 in range(2):
                    ps = pa_ps.tile([P, 512], F32, tag="pp", name="pp")
                    for kt in range(FT):
                        nc.tensor.matmul(
                            ps[:], wq_sb[:, kt * P:(kt + 1) * P],
                            x_bf[kt][:, W + h2 * 512:W + (h2 + 1) * 512],
                            start=(kt == 0), stop=(kt == FT - 1))
                    nc.scalar.activation(qT[mt][:, h2 * 512:(h2 + 1) * 512],
                                         ps[:], AF.Identity,
                                         bias=bq_sb[:, mt:mt + 1])
            for mt in range(FT):  # kT over own tokens (+ halo when l == 0)
                wk_sb = pa_sb.tile([P, HD], BF16, tag="wks", name="wks")
                nc.sync.dma_start(wk_sb[:], t["wk"][l, mt])
                segs = list(own_segs)
                if not deferred:
                    segs.append(None)  # halo 2-range seg
                for sg in segs:
                    ps = pa_ps.tile([P, 512], F32, tag="pp", name="pp")
                    rhs = ([x_bf[kt][:, sg[0]:sg[1]] for kt in range(FT)]
                           if sg is not None else
                           [_two_seg(x_bf[kt], 0, W + T_OWN, W)
                            for kt in range(FT)])
                    for kt in range(FT):
                        nc.tensor.matmul(
                            ps[:], wk_sb[:, kt * P:(kt + 1) * P], rhs[kt],
                            start=(kt == 0), stop=(kt == FT - 1))
                    dst = (kT[mt][:, sg[0]:sg[1]] if sg is not None
                           else _two_seg(kT[mt], 0, W + T_OWN, W))
                    nc.scalar.activation(dst, ps[:], AF.Identity,
                                         bias=bk_sb[:, mt:mt + 1])
            # v natural [tok, d]: lhsT = x slice, rhs = wv strip
            vtts = list(own_tt) + ([] if deferred else list(halo_tt))
            for tt in vtts:
                for hf in range(2):
                    ps = pa_ps.tile([P, 384], F32, tag="ppv", name="ppv",
                                    bufs=2)
                    for kt in range(FT):
                        nc.tensor.matmul(
                            ps[:], x_bf[kt][:, tt * P:(tt + 1) * P],
                            wv_all[kt][:, hf * 384:(hf + 1) * 384],
                            start=(kt == 0), stop=(kt == FT - 1))
                    vw = v[tt][:, hf * 390:(hf + 1) * 390].rearrange(
                        "p (h c) -> p h c", c=65)
                    nc.scalar.activation(
                        vw[:, :, 0:64],
                        ps[:].rearrange("p (h d) -> p h d", h=6), AF.Identity)
            for tt in range(12):
                vb = v[tt][:]
                ones_view = bass.AP(tensor=vb.tensor, offset=vb.offset + 64,
                                    ap=[vb.ap[0], [65, NH], [1, 1]])
                nc.vector.memset(ones_view, 1.0)

        # ---- phase B: banded attention (interior chunks first) ----
        with tc.tile_pool(name=f"pb_sb{l}", bufs=3) as bsb, \
             tc.tile_pool(name=f"pb_dsb{l}", bufs=2) as dpool, \
             tc.tile_pool(name=f"pb_ps{l}", bufs=2, space="PSUM") as bps, \
             tc.tile_pool(name=f"pb_ps2{l}", bufs=2, space="PSUM") as opool:
            for c in (1, 2):
                _attn_chunk(nc, c, qT, kT, v, o, ml_sb, mr_sb, ones33,
                            bsb, bps, opool, dpool)
            if deferred:
                # halo K/V: x halo cols were written by last layer's exchange;
                # psum borrowed from the sps slots (WAR-interleaves with the
                # score pipeline), weights re-streamed
                xh = [_two_seg(x_bf[kt], 0, W + T_OWN, W) for kt in range(FT)]
                for mt in range(FT):
                    wk_sb = bsb.tile([P, HD], BF16, tag="wkh", name="wkh", bufs=2)
                    nc.sync.dma_start(wk_sb[:], t["wk"][l, mt])
                    ps = bps.tile([P, 512], F32, tag="sps", name="hps")
                    for kt in range(FT):
                        nc.tensor.matmul(
                            ps[:], wk_sb[:, kt * P:(kt + 1) * P], xh[kt],
                            start=(kt == 0), stop=(kt == FT - 1))
                    nc.scalar.activation(_two_seg(kT[mt], 0, W + T_OWN, W),
                                         ps[:], AF.Identity,
                                         bias=bk_sb[:, mt:mt + 1])
                for tt in halo_tt:
                    for hf in range(2):
                        ps = bps.tile([P, 384], F32, tag="sps", name="hpsv")
                        for kt in range(FT):
                            nc.tensor.matmul(
                                ps[:], x_bf[kt][:, tt * P:(tt + 1) * P],
                                wv_all[kt][:, hf * 384:(hf + 1) * 384],
                                start=(kt == 0), stop=(kt == FT - 1))
                        vw = v[tt][:, hf * 390:(hf + 1) * 390].rearrange(
                            "p (h c) -> p h c", c=65)
                        nc.scalar.activation(
                            vw[:, :, 0:64],
                            ps[:].rearrange("p (h d) -> p h d", h=6),
                            AF.Identity)
            for c in (0, 3):
                _attn_chunk(nc, c, qT, kT, v, o, ml_sb, mr_sb, ones33,
                            bsb, bps, opool, dpool)

        # ---- phase C: O-proj + residual (r1 <- x + O@Wo + bo) ----
        with tc.tile_pool(name=f"pc_sb{l}", bufs=3) as csb, \
             tc.tile_pool(name=f"pc_ps{l}", bufs=4, space="PSUM") as cps:
            for mt in range(FT):
                wo_sb = csb.tile([P, HD], BF16, tag="wos", name="wos")
                nc.sync.dma_start(wo_sb[:], t["wo"][l, mt])
                for h2 in range(2):
                    ps = cps.tile([P, 512], F32, tag="ppo", name="ppo")
                    for kt in range(FT):
                        nc.tensor.matmul(
                            ps[:], wo_sb[:, kt * P:(kt + 1) * P],
                            o[kt][:, h2 * 512:(h2 + 1) * 512],
                            start=(kt == 0), stop=(kt == FT - 1))
                    nc.vector.scalar_tensor_tensor(
                        out=r1[mt][:, h2 * 512:(h2 + 1) * 512], in0=ps[:],
                        scalar=bo_sb[:, mt:mt + 1],
                        in1=x[mt][:, h2 * 512:(h2 + 1) * 512],
                        op0=ALU.add, op1=ALU.add)
        # ---- LN1 boundary piece (token cols {0:W} U {768:1024}) ----
        with tc.tile_pool(name=f"ln1b_sb{l}", bufs=1) as l1sb, \
             tc.tile_pool(name=f"ln1b_ps{l}", bufs=1, space="PSUM") as l1ps:
            _ln_T(nc, l1sb, l1ps,
                  [_two_seg(r1[ft], 0, 3 * W, W) for ft in range(FT)],
                  [_two_seg(r1[ft], 0, 3 * W, W) for ft in range(FT)],
                  ones_r, ones_row, eps_sb, ls1_sb, lb1_sb, 512,
                  stage_sbuf=True,
                  out2_aps=[_two_seg(y_bf[ft], 0, 3 * W, W)
                            for ft in range(FT)])

        # ---- FFN boundary pass (token cols {0:W} U {768:1024} of own) ----
        y_bd = [_two_seg(y_bf[ft], 0, 3 * W, W) for ft in range(FT)]
        yr_bd = [_two_seg(r1[ft], 0, 3 * W, W) for ft in range(FT)]
        _ffn_pass(nc, tc, t, l, "bd", y_bd, yr_bd, r2b, b1_sb, b2_sb)

        # ---- LN1 interior piece (DVE work overlaps FFN-bd matmuls) ----
        with tc.tile_pool(name=f"ln1i_sb{l}", bufs=1) as l1sb, \
             tc.tile_pool(name=f"ln1i_ps{l}", bufs=1, space="PSUM") as l1ps:
            _ln_T(nc, l1sb, l1ps, [r1[ft][:, W:3 * W] for ft in range(FT)],
                  [r1[ft][:, W:3 * W] for ft in range(FT)],
                  ones_r, ones_row, eps_sb, ls1_sb, lb1_sb, 512,
                  stage_sbuf=True,
                  out2_aps=[y_bf[ft][:, W:3 * W] for ft in range(FT)])

        # ---- FFN interior pass (emitted before LN2-bd: its matmuls fill
        # the PE while LN2-bd's scalar/vector chain runs) ----
        y_int = [y_bf[ft][:, W:3 * W] for ft in range(FT)]
        yr_int = [r1[ft][:, W:3 * W] for ft in range(FT)]
        _ffn_pass(nc, tc, t, l, "int", y_int, yr_int, r2i, b1_sb, b2_sb)

        with tc.tile_pool(name=f"ln2b_sb{l}", bufs=1) as l2sb, \
             tc.tile_pool(name=f"ln2b_ps{l}", bufs=1, space="PSUM") as l2ps:
            _ln_T(nc, l2sb, l2ps, [r2b[ft][:] for ft in range(FT)],
                  [_two_seg(x[ft], 0, 3 * W, W) for ft in range(FT)],
                  ones_r, ones_row, eps_sb, ls2_sb, lb2_sb, 512,
                  stage_sbuf=True,
                  out2_aps=[_two_seg(x_bf[ft], W, W + 3 * W, W)
                            for ft in range(FT)])

        # ---- halo exchange (overlaps FFN interior + next layer's QKV) ----
        if exchange:
            edram = ctx.enter_context(
                tc.tile_pool(name=f"pe_dram{l}", bufs=1, space="DRAM"))
            b_in = edram.tile([2, FT, P, W], BF16, tag="bin", name="bin")
            b_out = edram.tile([4 * 2 * FT * P, W], BF16, tag="bout",
                               name="bout")
            for ft in range(FT):
                nc.sync.dma_start(b_in[0, ft], x_bf[ft][:, W:2 * W])
                nc.sync.dma_start(b_in[1, ft], x_bf[ft][:, T_OWN:T_OWN + W])
            nc.gpsimd.collective_compute(
                "AllGather", ALU.bypass,
                replica_groups=[[0, 1, 2, 3], [4, 5, 6, 7]],
                ins=[b_in[:].opt()], outs=[b_out[:].opt()])
            for side in range(2):
                for ft in range(FT):
                    dst = (x_bf[ft][:, 0:W] if side == 0
                           else x_bf[ft][:, T_OWN + W:T_EXT])
                    nc.gpsimd.indirect_dma_start(
                        out=dst, out_offset=None, in_=b_out[:],
                        in_offset=bass.IndirectOffsetOnAxis(
                            ap=hid_sb[:, side * FT + ft:side * FT + ft + 1],
                            axis=0))

        # ---- LN2 interior (DVE overlaps next layer's QKV matmuls) ----
        with tc.tile_pool(name=f"ln2i_sb{l}", bufs=1) as l2sb, \
             tc.tile_pool(name=f"ln2i_ps{l}", bufs=1, space="PSUM") as l2ps:
            _ln_T(nc, l2sb, l2ps, [r2i[ft][:] for ft in range(FT)],
                  [x[ft][:, W:3 * W] for ft in range(FT)],
                  ones_r, ones_row, eps_sb, ls2_sb, lb2_sb, 512,
                  stage_sbuf=True,
                  out2_aps=[x_bf[ft][:, 2 * W:4 * W] for ft in range(FT)])


# ---------------- host side ----------------

def _blocked(w, n_k, n_m):
    """[n_k*128, n_m*128] -> [n_m, 128, n_k, 128] (lhsT strips by out-tile)."""
    return np.ascontiguousarray(
        w.reshape(n_k, P, n_m, P).transpose(2, 1, 0, 3))


def _bias_lay(b, n):
    return np.ascontiguousarray(b.reshape(n, P).T)


def prepare(inputs):
    """Build per-core in_maps from full inputs."""
    ids_full = np.asarray(inputs["input_ids"]).astype(np.int32)
    am = np.asarray(inputs["attention_mask"]).astype(np.int32)
    emb_word = np.asarray(inputs["emb_word"], dtype=np.float32)
    emb_pos = np.asarray(inputs["emb_pos"], dtype=np.float32)
    Wq = np.asarray(inputs["Wq"], np.float32) / np.sqrt(DH)
    bq = np.asarray(inputs["bq"], np.float32) / np.sqrt(DH)
    Wk = np.asarray(inputs["Wk"], np.float32)
    bk = np.asarray(inputs["bk"], np.float32)
    Wv = np.asarray(inputs["Wv"], np.float32)
    bv = np.asarray(inputs["bv"], np.float32)
    Wo = np.asarray(inputs["Wo"], np.float32)
    bo = np.asarray(inputs["bo"], np.float32)
    W1 = np.asarray(inputs["W1"], np.float32)
    b1 = np.asarray(inputs["b1"], np.float32)
    W2 = np.asarray(inputs["W2"], np.float32)
    b2 = np.asarray(inputs["b2"], np.float32)
    assert np.all(am == 1), "general attention_mask needs mid-tile masks too"

    shared = {
        "emb_word": emb_word,
        "eln_s": np.asarray(inputs["emb_ln_s"], np.float32),
        "eln_b": np.asarray(inputs["emb_ln_b"], np.float32),
        "wq": np.stack([_blocked(Wq[i], FT, FT) for i in range(L)]).astype(
            ml_dtypes.bfloat16),
        "wk": np.stack([_blocked(Wk[i], FT, FT) for i in range(L)]).astype(
            ml_dtypes.bfloat16),
        "wv": Wv.astype(ml_dtypes.bfloat16),
        "wo": np.stack([_blocked(Wo[i], FT, FT) for i in range(L)]).astype(
            ml_dtypes.bfloat16),
        "w1": np.stack([_blocked(W1[i], FT, FFT) for i in range(L)]).astype(
            ml_dtypes.bfloat16),
        "w2": W2.astype(ml_dtypes.bfloat16),
        "bq": np.stack([_bias_lay(bq[i], FT) for i in range(L)]),
        "bk": np.stack([_bias_lay(bk[i], FT) for i in range(L)]),
        "bo": np.stack([_bias_lay(bv[i] @ Wo[i] + bo[i], FT)
                        for i in range(L)]),
        "b1": np.stack([_bias_lay(b1[i], FFT) for i in range(L)]),
        "b2": np.stack([_bias_lay(b2[i], FT) for i in range(L)]),
        "ls1": np.stack([_bias_lay(np.asarray(inputs["ln1_s"], np.float32)[i],
                                   FT) for i in range(L)]),
        "lb1": np.stack([_bias_lay(np.asarray(inputs["ln1_b"], np.float32)[i],
                                   FT) for i in range(L)]),
        "ls2": np.stack([_bias_lay(np.asarray(inputs["ln2_s"], np.float32)[i],
                                   FT) for i in range(L)]),
        "lb2": np.stack([_bias_lay(np.asarray(inputs["ln2_b"], np.float32)[i],
                                   FT) for i in range(L)]),
    }

    in_maps = []
    i_idx = np.arange(W)
    for core in range(N_CORES):
        b, sb = core // 4, core % 4
        s0 = sb * T_OWN
        ext_pos = np.clip(np.arange(s0 - W, s0 + T_OWN + W), 0, S - 1)
        m = dict(shared)
        m["ids"] = np.ascontiguousarray(
            ids_full[b, ext_pos].reshape(12, P).T)
        m["pos"] = np.ascontiguousarray(emb_pos[ext_pos])
        # masks: global chunk gc, window key j in [0,768), query i in [0,256):
        #   key_abs = gc*W - W + j ; allowed = |j - W - i| <= W
        #             & 0 <= key_abs < S & attention_mask[b, key_abs]
        mlm = np.zeros((NCH, P, 512), np.float32)
        mrm = np.zeros((NCH, P, 512), np.float32)
        for c in range(NCH):
            gc = sb * NCH + c
            for kt2 in range(2):
                for mm_, j0 in ((mlm, 0), (mrm, 512)):
                    j = j0 + kt2 * P + np.arange(P)[:, None]
                    key_abs = gc * W - W + j
                    ok = (np.abs(j - W - i_idx[None, :]) <= W)
                    ok &= (key_abs >= 0) & (key_abs < S)
                    ok &= am[b, np.clip(key_abs, 0, S - 1)] > 0
                    mm_[c, :, kt2 * W:(kt2 + 1) * W] = ok
        m["ml"] = mlm.astype(ml_dtypes.bfloat16)
        m["mr"] = mrm.astype(ml_dtypes.bfloat16)
        # halo row ids into the gathered [4, 2, FT, 128, W] row table
        hid = np.zeros((2, FT, P), np.int64)
        for side in range(2):
            nb = sb - 1 if side == 0 else sb + 1
            if 0 <= nb <= 3:
                osd = 1 - side  # left halo <- neighbor's right block
                for ft in range(FT):
                    hid[side, ft] = ((nb * 2 + osd) * FT + ft) * P \
                        + np.arange(P)
            else:
                for ft in range(FT):
                    hid[side, ft] = ((sb * 2 + side) * FT + ft) * P \
                        + np.arange(P)
        m["halo_ids"] = np.ascontiguousarray(
            hid.reshape(12, P).T.astype(np.int32))
        in_maps.append(m)
    return in_maps


_NC_CACHE = {}


def get_nc(n_layers=L):
    if n_layers not in _NC_CACHE:
        _NC_CACHE[n_layers] = build_nc(n_layers)
    return _NC_CACHE[n_layers]


def run(inputs, n_layers=L, trace=False):
    nc = get_nc(n_layers)
    in_maps = prepare(inputs)
    res = bass_utils.run_bass_kernel_spmd(
        nc, in_maps, core_ids=list(range(N_CORES)), trace=trace)
    outs = np.empty((B, S, HD), np.float32)
    for core in range(N_CORES):
        b, sb = core // 4, core % 4
        ot = res.results[core]["out"]  # [FT, 128, T_OWN]
        outs[b, sb * T_OWN:(sb + 1) * T_OWN] = ot.reshape(HD, T_OWN).T
    return outs, res


def kernel(**inputs) -> np.ndarray:
    out, _ = run(inputs)
    return out



# revision 28
# speedup vs baseline: 1.1914x; 1.0036x over previous
"""Longformer layer stack (4 layers, sliding-window attention) on 8 TRN2 cores.

Sharding: data-parallel over batch (2) x sequence-parallel (4 blocks of 1024
tokens). Each core computes its sequence block; the banded attention needs a
W=256 token halo, exchanged between neighboring blocks with an AllGather after
each layer (layers 0-2). Residual stream kept transposed ([dmodel, tokens]) in
float32r; attention probs/values in bf16.

v2: the halo exchange is overlapped with compute (boundary-first FFN/LN2 kicks
the collective early; the next layer projects its own-token QKV and runs the
interior attention chunks while the collective is in flight, deferring only
halo K/V projection and the two edge chunks). Softmax normalization is batched
per chunk: denominators staged to SBUF, one reciprocal, a PE select-matmul
broadcast, and one multiply per (chunk, feature-tile) -- replacing per-head
reciprocal/partition-broadcast chains. LayerNorm uses a fused Rsqrt and a
full-width LN1.
"""
import sys

sys.path.insert(0, '/opt/trn_rl_repo')

import numpy as np
import ml_dtypes

import concourse.bass as bass
import concourse.mybir as mybir
import concourse.tile as tile
from concourse import bacc
from concourse import bass_utils

F32 = mybir.dt.float32
F32R = mybir.dt.float32r
BF16 = mybir.dt.bfloat16
I32 = mybir.dt.int32
AF = mybir.ActivationFunctionType
ALU = mybir.AluOpType

NH = 12          # heads
DH = 64          # head dim
HD = 768         # model dim
FF = 3072        # ffn dim
W = 256          # one-sided window
L = 4            # layers
B = 2
S = 4096
EPS = 1e-12
N_CORES = 8
T_OWN = 1024     # tokens per core
T_EXT = 1536     # with halos
FT = 6           # model-dim 128-tiles
FFT = 24         # ffn-dim 128-tiles
NCH = 4          # local chunks of 256 queries
P = 128


def _two_seg(ap2d, o1, o2, n):
    """[128, 2, n] view over cols {o1:o1+n} U {o2:o2+n} of a [128, N] AP."""
    base = ap2d[:, o1:o1 + n]
    return bass.AP(tensor=base.tensor, offset=base.offset,
                   ap=[base.ap[0], [o2 - o1, 2], [1, n]])


LN_BCAST_GPSIMD = False


def _ln_T(nc, sb, ps, r_aps, out_aps, ones_r, ones_row, eps_sb, s_ap, b_ap,
          ncols, stage_sbuf=False, out2_aps=None):
    """LayerNorm over the partition (feature) axis of transposed tiles.

    r_aps: list of FT fp32r SBUF APs [128, ncols] (input; normalized IN PLACE
    up to the final scale/bias which lands in out_aps). ncols in {512, 1024}.
    out_aps[ft] <- LN(r)*s + b. s_ap/b_ap: [128, FT] sbuf. mu/rstd broadcasts
    run as K=1 outer-product matmuls on the PE (gpsimd stays free for the
    collective); stage_sbuf copies them out of PSUM so the banks free early.
    """
    nseg = ncols // 512
    sxs = [ps.tile([1, 512], F32, tag=f"lnsx{j}", name=f"lnsx{j}")
           for j in range(nseg)]
    sqs = [ps.tile([1, 512], F32, tag=f"lnsq{j}", name=f"lnsq{j}")
           for j in range(nseg)]
    for ft in range(FT):
        sqt = sb.tile([P, ncols], F32R, tag="lnsqt", name="lnsqt", bufs=2)
        nc.scalar.activation(sqt[:], r_aps[ft], AF.Square)
        for j in range(nseg):
            cs = slice(j * 512, (j + 1) * 512)
            rseg = r_aps[ft] if nseg == 1 else r_aps[ft][:, cs]
            nc.tensor.matmul(sxs[j][:], ones_r[:, 0:1], rseg,
                             start=(ft == 0), stop=(ft == FT - 1))
            nc.tensor.matmul(sqs[j][:], ones_r[:, 0:1], sqt[:, cs],
                             start=(ft == 0), stop=(ft == FT - 1))
    mu = sb.tile([1, ncols], F32R, tag="lnmu", name="lnmu")
    var = sb.tile([1, ncols], F32, tag="lnvar", name="lnvar")
    musq = sb.tile([1, ncols], F32, tag="lnmusq", name="lnmusq")
    rstd = sb.tile([1, ncols], F32R, tag="lnrstd", name="lnrstd")
    for j in range(nseg):
        cs = slice(j * 512, (j + 1) * 512)
        nc.scalar.activation(mu[:, cs], sxs[j][:], AF.Identity, scale=1.0 / HD)
    nc.vector.tensor_tensor(musq[:], mu[:], mu[:], op=ALU.mult)
    for j in range(nseg):
        cs = slice(j * 512, (j + 1) * 512)
        nc.vector.scalar_tensor_tensor(out=var[:, cs], in0=sqs[j][:],
                                       scalar=1.0 / HD, in1=musq[:, cs],
                                       op0=ALU.mult, op1=ALU.subtract)
    sd = musq  # musq is dead; reuse its slot for sd
    nc.scalar.activation(sd[:], var[:], AF.Sqrt, bias=eps_sb[0:1, :])
    with nc.allow_low_precision(reason="f32r recip holds full fp32 bits"):
        nc.vector.reciprocal(rstd[:], sd[:])
    if LN_BCAST_GPSIMD:
        mu_b = sb.tile([P, ncols], F32R, tag="lnmus", name="lnmubS")
        nc.gpsimd.partition_broadcast(mu_b[:], mu[:], channels=P)
        rstd_b = sb.tile([P, ncols], F32R, tag="lnrstds", name="lnrstdbS")
        nc.gpsimd.partition_broadcast(rstd_b[:], rstd[:], channels=P)
        for ft in range(FT):
            nc.vector.tensor_tensor(r_aps[ft], r_aps[ft], mu_b[:],
                                    op=ALU.subtract)
            nc.vector.tensor_tensor(r_aps[ft], r_aps[ft], rstd_b[:],
                                    op=ALU.mult)
            nc.scalar.activation(out_aps[ft], r_aps[ft], AF.Identity,
                                 scale=s_ap[:, ft:ft + 1],
                                 bias=b_ap[:, ft:ft + 1])
        return
    mu_b = ps.tile([P, ncols], F32, tag="lnmub", name="lnmub")
    rstd_b = ps.tile([P, ncols], F32, tag="lnrstdb", name="lnrstdb")
    for j in range(nseg):
        cs = slice(j * 512, (j + 1) * 512)
        nc.tensor.matmul(mu_b[:, cs], ones_row[:], mu[0:1, cs],
                         start=True, stop=True)
        nc.tensor.matmul(rstd_b[:, cs], ones_row[:], rstd[0:1, cs],
                         start=True, stop=True)
    if stage_sbuf:
        mu_s = sb.tile([P, ncols], F32, tag="lnmus", name="lnmus")
        nc.scalar.activation(mu_s[:], mu_b[:], AF.Identity)
        rstd_s = sb.tile([P, ncols], F32, tag="lnrstds", name="lnrstds")
        nc.vector.tensor_copy(rstd_s[:], rstd_b[:])
        mu_b, rstd_b = mu_s, rstd_s
    for ft in range(FT):
        nc.vector.tensor_tensor(r_aps[ft], r_aps[ft], mu_b[:], op=ALU.subtract)
        nc.vector.tensor_tensor(r_aps[ft], r_aps[ft], rstd_b[:], op=ALU.mult)
        # out2 (bf16 shadow) must read r BEFORE the in-place scale/bias ACT
        if out2_aps is not None:
            nc.vector.tensor_scalar(out=out2_aps[ft], in0=r_aps[ft],
                                    scalar1=s_ap[:, ft:ft + 1],
                                    scalar2=b_ap[:, ft:ft + 1],
                                    op0=ALU.mult, op1=ALU.add)
        nc.scalar.activation(out_aps[ft], r_aps[ft], AF.Identity,
                             scale=s_ap[:, ft:ft + 1], bias=b_ap[:, ft:ft + 1])


def build_nc(n_layers=L):
    nc = bacc.Bacc("TRN2", target_bir_lowering=False, debug=False,
                   num_devices=N_CORES)
    dt_ = nc.dram_tensor
    t = {}
    t["emb"] = dt_("emb_word", [32000, HD], F32, kind="ExternalInput").ap()
    t["ids"] = dt_("ids", [P, 12], I32, kind="ExternalInput").ap()
    t["pos"] = dt_("pos", [T_EXT, HD], F32, kind="ExternalInput").ap()
    t["eln_s"] = dt_("eln_s", [P, FT], F32, kind="ExternalInput").ap()
    t["eln_b"] = dt_("eln_b", [P, FT], F32, kind="ExternalInput").ap()
    t["wq"] = dt_("wq", [L, FT, P, FT, P], BF16, kind="ExternalInput").ap()
    t["wk"] = dt_("wk", [L, FT, P, FT, P], BF16, kind="ExternalInput").ap()
    t["wv"] = dt_("wv", [L, HD, HD], BF16, kind="ExternalInput").ap()
    t["wo"] = dt_("wo", [L, FT, P, FT, P], BF16, kind="ExternalInput").ap()
    t["w1"] = dt_("w1", [L, FFT, P, FT, P], BF16, kind="ExternalInput").ap()
    t["w2"] = dt_("w2", [L, FF, HD], BF16, kind="ExternalInput").ap()
    for nm in ["bq", "bk", "bo", "b2", "ls1", "lb1", "ls2", "lb2"]:
        t[nm] = dt_(nm, [L, P, FT], F32, kind="ExternalInput").ap()
    t["b1"] = dt_("b1", [L, P, FFT], F32, kind="ExternalInput").ap()
    t["ml"] = dt_("ml", [NCH, P, 512], BF16, kind="ExternalInput").ap()
    t["mr"] = dt_("mr", [NCH, P, 512], BF16, kind="ExternalInput").ap()
    t["halo_ids"] = dt_("halo_ids", [P, 12], I32, kind="ExternalInput").ap()
    t["out"] = dt_("out", [FT, P, T_OWN], F32, kind="ExternalOutput").ap()

    with tile.TileContext(nc) as tc:
        _build_body(nc, tc, n_layers, t)
    nc.compile()
    return nc


def _build_body(nc, tc, n_layers, t):
    from contextlib import ExitStack
    with ExitStack() as ctx:
        persist = ctx.enter_context(tc.tile_pool(name="persist", bufs=1))
        # residual stream, transposed, with halos: x[ft] = [128, T_EXT].
        # x is the f32r master; x_bf is its bf16 shadow used as matmul input
        # (PE can't mix 32-bit and 16-bit matmul operands).
        x = [persist.tile([P, T_OWN], F32R, tag=f"x{ft}", name=f"x{ft}") for ft in range(FT)]
        x_bf = [persist.tile([P, T_EXT], BF16, tag=f"xb{ft}", name=f"xb{ft}")
                for ft in range(FT)]
        ml_sb = [persist.tile([P, 512], BF16, tag=f"ml{c}", name=f"ml{c}") for c in range(NCH)]
        mr_sb = [persist.tile([P, 512], BF16, tag=f"mr{c}", name=f"mr{c}") for c in range(NCH)]
        for c in range(NCH):
            nc.sync.dma_start(ml_sb[c][:], t["ml"][c])
            nc.sync.dma_start(mr_sb[c][:], t["mr"][c])
        ones_f = persist.tile([P, 1], F32, tag="ones_f", name="ones_f")
        nc.vector.memset(ones_f[:], 1.0)
        ones_r = persist.tile([P, 1], F32R, tag="ones_r", name="ones_r")
        nc.scalar.activation(ones_r[:], ones_f[:], AF.Identity)
        ones_row_f = persist.tile([1, P], F32, tag="ones_row_f",
                                  name="ones_row_f")
        nc.vector.memset(ones_row_f[:], 1.0)
        ones_row = persist.tile([1, P], F32R, tag="ones_row", name="ones_row")
        nc.scalar.activation(ones_row[:], ones_row_f[:], AF.Identity)
        from concourse.masks import make_identity
        ident = persist.tile([P, P], F32, tag="ident", name="ident")
        make_identity(nc, ident[:])
        hid_sb = persist.tile([P, 12], I32, tag="hid", name="hid")
        nc.sync.dma_start(hid_sb[:], t["halo_ids"][:])
        eps_sb = persist.tile([P, 1], F32, tag="eps", name="eps")
        nc.vector.memset(eps_sb[:], EPS)
        # K=1 broadcast rows for the softmax-denominator outer products
        onesf33 = persist.tile([33, 64], F32, tag="o33f", name="onesf33")
        nc.vector.memset(onesf33[:], 0.0)
        nc.vector.memset(onesf33[0:1, :], 1.0)
        nc.vector.memset(onesf33[32:33, :], 1.0)
        ones33 = persist.tile([33, 64], F32R, tag="o33", name="ones33")
        nc.scalar.activation(ones33[:], onesf33[:], AF.Identity)

        # ---- embedding: gather+pos, transpose, then LN on the PE ----
        with tc.tile_pool(name="emb_sb", bufs=1) as esb, \
             tc.tile_pool(name="emb_sb2", bufs=2) as esb2:
            ids_sb = esb.tile([P, 12], I32, tag="ids", name="ids")
            nc.sync.dma_start(ids_sb[:], t["ids"][:])
            es_sb = esb.tile([P, FT], F32, tag="es", name="es")
            nc.sync.dma_start(es_sb[:], t["eln_s"][:])
            eb_sb = esb.tile([P, FT], F32, tag="eb", name="eb")
            nc.sync.dma_start(eb_sb[:], t["eln_b"][:])
            e = [esb.tile([P, HD], F32, tag=f"e{tt}", name=f"e{tt}") for tt in range(12)]
            for tt in range(12):
                nc.gpsimd.indirect_dma_start(
                    out=e[tt][:], out_offset=None, in_=t["emb"][:],
                    in_offset=bass.IndirectOffsetOnAxis(
                        ap=ids_sb[:, tt:tt + 1], axis=0))
                p_sb = esb2.tile([P, HD], F32, tag="pos", name="pos")
                nc.sync.dma_start(p_sb[:], t["pos"][tt * P:(tt + 1) * P, :])
                nc.vector.tensor_tensor(e[tt][:], e[tt][:], p_sb[:], op=ALU.add)
            xw = [esb.tile([P, T_EXT], F32R, tag=f"xw{ft}", name=f"xw{ft}")
                  for ft in range(FT)]
            with tc.tile_pool(name="emb_ps", bufs=2, space="PSUM") as eps2:
                for ft in range(FT):
                    tr = eps2.tile([P, T_EXT], F32, tag="tr", name="tr")
                    for tt in range(12):
                        nc.tensor.transpose(
                            tr[:, tt * P:(tt + 1) * P],
                            e[tt][:, ft * P:(ft + 1) * P], ident[:])
                    nc.scalar.activation(xw[ft][:], tr[:], AF.Identity)
            # LN over the feature (partition) axis, 512 token-cols at a time
            for pc in range(3):
                c0 = pc * 512
                with tc.tile_pool(name=f"eln_sb{pc}", bufs=1) as lsb_, \
                     tc.tile_pool(name=f"eln_ps{pc}", bufs=1,
                                  space="PSUM") as lps_:
                    _ln_T(nc, lsb_, lps_,
                          [xw[ft][:, c0:c0 + 512] for ft in range(FT)],
                          [x_bf[ft][:, c0:c0 + 512] for ft in range(FT)],
                          ones_r, ones_row, eps_sb, es_sb, eb_sb, 512,
                          stage_sbuf=True)
                # own-token f32r master from the normalized xw slice
                o0, o1 = max(c0, W), min(c0 + 512, W + T_OWN)
                for ft in range(FT):
                    nc.vector.tensor_scalar(
                        out=x[ft][:, o0 - W:o1 - W],
                        in0=xw[ft][:, o0:o1],
                        scalar1=es_sb[:, ft:ft + 1],
                        scalar2=eb_sb[:, ft:ft + 1],
                        op0=ALU.mult, op1=ALU.add)

        for l in range(n_layers):
            _layer(nc, tc, t, l, x, x_bf, ml_sb, mr_sb, ones_r, ones_row,
                   eps_sb, hid_sb, ones33, exchange=(l < n_layers - 1),
                   deferred=(l > 0))

        for ft in range(FT):
            nc.gpsimd.dma_start(t["out"][ft], x[ft][:, 0:T_OWN])


USE_NEW_NORM = True


def _attn_chunk(nc, c, qT, kT, v, o, ml_sb, mr_sb, ones33, bsb, bps, opool,
                dpool):
    """Attention for one 256-query chunk, all 12 heads + normalization."""
    # DVE writes must start 32-partition-aligned: stage the 12 denominator
    # rows on partitions {0, 32} (6 heads each); a DMA then re-stages them
    # onto 12 partitions so the reciprocal runs wide (not 1536-serial)
    den_f = dpool.tile([33, 6 * W], F32, tag="denf", name="den_f", bufs=1)
    for h in range(NH):
        ft, po = h // 2, (h % 2) * 64
        sps = bps.tile([P, 6 * W], F32, tag="sps", name="sps")
        for w in range(6):
            nc.tensor.matmul(
                sps[:, w * W:(w + 1) * W],
                kT[ft][po:po + 64, c * W + w * P:c * W + (w + 1) * P],
                qT[ft][po:po + 64, c * W:(c + 1) * W],
                start=True, stop=True)
        ex = bsb.tile([P, 6 * W], BF16, tag="ex", name="ex")
        nc.scalar.activation(ex[:], sps[:], AF.Exp)
        nc.vector.tensor_tensor(ex[:, 0:512], ex[:, 0:512],
                                ml_sb[c][:], op=ALU.mult)
        nc.vector.tensor_tensor(ex[:, 1024:1536], ex[:, 1024:1536],
                                mr_sb[c][:], op=ALU.mult)
        ops = opool.tile([P, W], F32, tag="ops", name="ops")
        for w in range(6):
            nc.tensor.matmul(
                ops[:, :],
                v[c * 2 + w][:, 65 * h:65 * h + 128],
                ex[:, w * W:(w + 1) * W],
                start=(w == 0), stop=(w == 5))
        dfr = (h // 6) * 32
        nc.vector.tensor_copy(den_f[dfr:dfr + 1, (h % 6) * W:(h % 6 + 1) * W],
                              ops[64:65, :])
        # unnormalized attention out -> o slot (normalized in bulk below);
        # alternate engines to balance scalar (exp) vs vector (masks) load
        dst = o[ft][po:po + 64, c * W:(c + 1) * W]
        if h % 2 == 0:
            nc.scalar.activation(dst, ops[0:64, :], AF.Identity)
        else:
            nc.vector.tensor_copy(dst, ops[0:64, :])
    dinv_f = dpool.tile([33, 6 * W], F32R, tag="dinvf", name="dinv_f", bufs=1)
    den12 = dpool.tile([NH, W], F32, tag="den12", name="den12", bufs=1)
    dinv12 = dpool.tile([NH, W], F32R, tag="dinv12", name="dinv12", bufs=1)
    dfb = den_f[:]
    nc.sync.dma_start(
        den12[:], bass.AP(tensor=dfb.tensor, offset=dfb.offset,
                          ap=[[32 * 6 * W, 2], [W, 6], [1, W]]))
    with nc.allow_low_precision(reason="f32r recip holds full fp32 bits"):
        nc.vector.reciprocal(dinv12[:], den12[:])
    dib = dinv_f[:]
    nc.sync.dma_start(
        bass.AP(tensor=dib.tensor, offset=dib.offset,
                ap=[[32 * 6 * W, 2], [W, 6], [1, W]]),
        dinv12[:])
    for ft in range(FT):
        bc = opool.tile([P, 2 * W], F32, tag="ops", name="bc")
        for half in range(2):
            h = 2 * ft + half
            r, j = (h // 6) * 32, h % 6
            nc.tensor.matmul(bc[0:64, half * W:(half + 1) * W],
                             ones33[r:r + 1, :],
                             dinv_f[r:r + 1, j * W:(j + 1) * W],
                             start=True, stop=True)
        for half in range(2):
            po = half * 64
            nc.vector.tensor_tensor(
                o[ft][po:po + 64, c * W:(c + 1) * W],
                o[ft][po:po + 64, c * W:(c + 1) * W],
                bc[0:64, half * W:(half + 1) * W], op=ALU.mult)


def _ffn_pass(nc, tc, t, l, tag, y_mm, y_res, r2, b1_sb, b2_sb):
    """One FFN pass over 512 token-columns.

    y_mm: bf16 [128,512] APs (matmul rhs); y_res: f32r views of the same
    columns (residual add). r2: FT [128,512] f32r SBUF tiles for y + FFN(y).
    """
    from contextlib import ExitStack
    with ExitStack() as dctx:
        dsb = dctx.enter_context(tc.tile_pool(name=f"pd_sb{l}_{tag}", bufs=3))
        zps = dctx.enter_context(
            tc.tile_pool(name=f"pd_psz{l}_{tag}", bufs=1, space="PSUM"))
        fps = dctx.enter_context(
            tc.tile_pool(name=f"pd_psf{l}_{tag}", bufs=2, space="PSUM"))
        zp = [zps.tile([P, 512], F32, tag=f"z{mt}", name=f"z{mt}") for mt in range(FT)]
        for ms in range(FFT):
            w1_sb = dsb.tile([P, HD], BF16, tag="w1s", name="w1s")
            nc.sync.dma_start(w1_sb[:], t["w1"][l, ms])
            fp = fps.tile([P, 512], F32, tag="fp", name="fp")
            for kt in range(FT):
                nc.tensor.matmul(fp[:], w1_sb[:, kt * P:(kt + 1) * P],
                                 y_mm[kt], start=(kt == 0),
                                 stop=(kt == FT - 1))
            f_sb = dsb.tile([P, 512], BF16, tag="fsb", name="fsb")
            nc.scalar.activation(f_sb[:], fp[:], AF.Gelu,
                                 bias=b1_sb[:, ms:ms + 1])
            w2_sb = dsb.tile([P, HD], BF16, tag="w2s", name="w2s")
            nc.sync.dma_start(w2_sb[:],
                              t["w2"][l, ms * P:(ms + 1) * P, :])
            for mt in range(FT):
                nc.tensor.matmul(zp[mt][:],
                                 w2_sb[:, mt * P:(mt + 1) * P],
                                 f_sb[:], start=(ms == 0),
                                 stop=(ms == FFT - 1))
        for mt in range(FT):
            nc.vector.scalar_tensor_tensor(
                out=r2[mt][:], in0=zp[mt][:],
                scalar=b2_sb[:, mt:mt + 1], in1=y_res[mt],
                op0=ALU.add, op1=ALU.add)


def _layer(nc, tc, t, l, x, x_bf, ml_sb, mr_sb, ones_r, ones_row, eps_sb,
           hid_sb, ones33, exchange, deferred):
    from contextlib import ExitStack
    with ExitStack() as ctx:
        lsb = ctx.enter_context(tc.tile_pool(name=f"lsb{l}", bufs=1))

        def bias_tile(name, n=FT):
            bt = lsb.tile([P, n], F32, tag=f"b_{name}", name=f"b_{name}")
            nc.sync.dma_start(bt[:], t[name][l])
            return bt
        bq_sb = bias_tile("bq"); bk_sb = bias_tile("bk"); bo_sb = bias_tile("bo")
        b1_sb = bias_tile("b1", FFT); b2_sb = bias_tile("b2")
        ls1_sb = bias_tile("ls1"); lb1_sb = bias_tile("lb1")
        ls2_sb = bias_tile("ls2"); lb2_sb = bias_tile("lb2")

        qT = [lsb.tile([P, T_OWN], BF16, tag=f"qT{i}", name=f"qT{i}") for i in range(FT)]
        kT = [lsb.tile([P, T_EXT], BF16, tag=f"kT{i}", name=f"kT{i}") for i in range(FT)]
        # v[tt]: per head h, cols [65h:65h+64] = V_h, col 65h+64 = 1.0; the
        # attnout stationary is the contiguous 128-col block [65h:65h+128]
        # (fast LDWEIGHTS path); psum rows 65-127 are garbage and unread.
        v = [lsb.tile([P, NH * 65 + 128], BF16, tag=f"v{i}", name=f"v{i}")
             for i in range(12)]
        o = [lsb.tile([P, T_OWN], BF16, tag=f"o{i}", name=f"o{i}") for i in range(FT)]
        r2b = [lsb.tile([P, 512], F32R, tag=f"r2b{i}", name=f"r2b{i}") for i in range(FT)]
        r2i = [lsb.tile([P, 512], F32R, tag=f"r2i{i}", name=f"r2i{i}") for i in range(FT)]
        r1 = [lsb.tile([P, T_OWN], F32R, tag=f"r1_{i}", name=f"r1_{i}")
              for i in range(FT)]
        # r1 doubles as y (LN1 output, written in place; f32r residual side);
        # qT doubles as y_bf (the bf16 LN1 shadow) -- dead after the scores
        y = [r1[i][:] for i in range(FT)]
        y_bf = qT

        # ---- phase A: QKV projections (own tokens; halo deferred if l>0) ----
        # kT col j = ext token j; own tokens are ext cols [W, W+T_OWN)
        wv_sb = ctx.enter_context(tc.tile_pool(name=f"pa_wv{l}", bufs=1))
        wv_all = [wv_sb.tile([P, HD], BF16, tag=f"wv{kt}", name=f"wv{kt}")
                  for kt in range(FT)]
        for kt in range(FT):
            nc.sync.dma_start(wv_all[kt][:],
                              t["wv"][l, kt * P:(kt + 1) * P, :])
        own_tt = range(2, 10) if deferred else range(12)
        halo_tt = (0, 1, 10, 11)
        own_segs = [(W, W + 512), (W + 512, W + T_OWN)]
        with tc.tile_pool(name=f"pa_sb{l}", bufs=3) as pa_sb, \
             tc.tile_pool(name=f"pa_ps{l}", bufs=4, space="PSUM") as pa_ps:
            for mt in range(FT):  # qT over own tokens
                wq_sb = pa_sb.tile([P, HD], BF16, tag="wqs", name="wqs")
                nc.sync.dma_start(wq_sb[:], t["wq"][l, mt])
                for h2 in range(2):
                    ps = pa_ps.tile([P, 512], F32, tag="pp", name="pp")
                    for kt in range(FT):
                        nc.tensor.matmul(
                            ps[:], wq_sb[:, kt * P:(kt + 1) * P],
                            x_bf[kt][:, W + h2 * 512:W + (h2 + 1) * 512],
                            start=(kt == 0), stop=(kt == FT - 1))
                    nc.scalar.activation(qT[mt][:, h2 * 512:(h2 + 1) * 512],
                                         ps[:], AF.Identity,
                                         bias=bq_sb[:, mt:mt + 1])
            for mt in range(FT):  # kT over own tokens (+ halo when l == 0)
                wk_sb = pa_sb.tile([P, HD], BF16, tag="wks", name="wks")
                nc.sync.dma_start(wk_sb[:], t["wk"][l, mt])
                segs = list(own_segs)
                if not deferred:
                    segs.append(None)  # halo 2-range seg
                for sg in segs:
                    ps = pa_ps.tile([P, 512], F32, tag="pp", name="pp")
                    rhs = ([x_bf[kt][:, sg[0]:sg[1]] for kt in range(FT)]
                           if sg is not None else
                           [_two_seg(x_bf[kt], 0, W + T_OWN, W)
                            for kt in range(FT)])
                    for kt in range(FT):
                        nc.tensor.matmul(
                            ps[:], wk_sb[:, kt * P:(kt + 1) * P], rhs[kt],
                            start=(kt == 0), stop=(kt == FT - 1))
                    dst = (kT[mt][:, sg[0]:sg[1]] if sg is not None
                           else _two_seg(kT[mt], 0, W + T_OWN, W))
                    nc.scalar.activation(dst, ps[:], AF.Identity,
                                         bias=bk_sb[:, mt:mt + 1])
            # v natural [tok, d]: lhsT = x slice, rhs = wv strip
            vtts = list(own_tt) + ([] if deferred else list(halo_tt))
            for tt in vtts:
                for hf in range(2):
                    ps = pa_ps.tile([P, 384], F32, tag="ppv", name="ppv",
                                    bufs=2)
                    for kt in range(FT):
                        nc.tensor.matmul(
                            ps[:], x_bf[kt][:, tt * P:(tt + 1) * P],
                            wv_all[kt][:, hf * 384:(hf + 1) * 384],
                            start=(kt == 0), stop=(kt == FT - 1))
                    vw = v[tt][:, hf * 390:(hf + 1) * 390].rearrange(
                        "p (h c) -> p h c", c=65)
                    nc.scalar.activation(
                        vw[:, :, 0:64],
                        ps[:].rearrange("p (h d) -> p h d", h=6), AF.Identity)
            for tt in range(12):
                vb = v[tt][:]
                ones_view = bass.AP(tensor=vb.tensor, offset=vb.offset + 64,
                                    ap=[vb.ap[0], [65, NH], [1, 1]])
                nc.vector.memset(ones_view, 1.0)

        # ---- phase B: banded attention (interior chunks first) ----
        with tc.tile_pool(name=f"pb_sb{l}", bufs=3) as bsb, \
             tc.tile_pool(name=f"pb_dsb{l}", bufs=2) as dpool, \
             tc.tile_pool(name=f"pb_ps{l}", bufs=2, space="PSUM") as bps, \
             tc.tile_pool(name=f"pb_ps2{l}", bufs=2, space="PSUM") as opool:
            for c in (1, 2):
                _attn_chunk(nc, c, qT, kT, v, o, ml_sb, mr_sb, ones33,
                            bsb, bps, opool, dpool)
            if deferred:
                # halo K/V: x halo cols were written by last layer's exchange;
                # psum borrowed from the sps slots (WAR-interleaves with the
                # score pipeline), weights re-streamed
                xh = [_two_seg(x_bf[kt], 0, W + T_OWN, W) for kt in range(FT)]
                for mt in range(FT):
                    wk_sb = bsb.tile([P, HD], BF16, tag="wkh", name="wkh", bufs=2)
                    nc.sync.dma_start(wk_sb[:], t["wk"][l, mt])
                    ps = bps.tile([P, 512], F32, tag="sps", name="hps")
                    for kt in range(FT):
                        nc.tensor.matmul(
                            ps[:], wk_sb[:, kt * P:(kt + 1) * P], xh[kt],
                            start=(kt == 0), stop=(kt == FT - 1))
                    nc.scalar.activation(_two_seg(kT[mt], 0, W + T_OWN, W),
                                         ps[:], AF.Identity,
                                         bias=bk_sb[:, mt:mt + 1])
                for tt in halo_tt:
                    for hf in range(2):
                        ps = bps.tile([P, 384], F32, tag="sps", name="hpsv")
                        for kt in range(FT):
                            nc.tensor.matmul(
                                ps[:], x_bf[kt][:, tt * P:(tt + 1) * P],
                                wv_all[kt][:, hf * 384:(hf + 1) * 384],
                                start=(kt == 0), stop=(kt == FT - 1))
                        vw = v[tt][:, hf * 390:(hf + 1) * 390].rearrange(
                            "p (h c) -> p h c", c=65)
                        nc.scalar.activation(
                            vw[:, :, 0:64],
                            ps[:].rearrange("p (h d) -> p h d", h=6),
                            AF.Identity)
            for c in (0, 3):
                _attn_chunk(nc, c, qT, kT, v, o, ml_sb, mr_sb, ones33,
                            bsb, bps, opool, dpool)

        # ---- phase C: O-proj + residual (r1 <- x + O@Wo + bo) ----
        with tc.tile_pool(name=f"pc_sb{l}", bufs=3) as csb, \
             tc.tile_pool(name=f"pc_ps{l}", bufs=4, space="PSUM") as cps:
            for mt in range(FT):
                wo_sb = csb.tile([P, HD], BF16, tag="wos", name="wos")
                nc.sync.dma_start(wo_sb[:], t["wo"][l, mt])
                for h2 in range(2):
                    ps = cps.tile([P, 512], F32, tag="ppo", name="ppo")
                    for kt in range(FT):
                        nc.tensor.matmul(
                            ps[:], wo_sb[:, kt * P:(kt + 1) * P],
                            o[kt][:, h2 * 512:(h2 + 1) * 512],
                            start=(kt == 0), stop=(kt == FT - 1))
                    nc.vector.scalar_tensor_tensor(
                        out=r1[mt][:, h2 * 512:(h2 + 1) * 512], in0=ps[:],
                        scalar=bo_sb[:, mt:mt + 1],
                        in1=x[mt][:, h2 * 512:(h2 + 1) * 512],
                        op0=ALU.add, op1=ALU.add)
        # ---- LN1 boundary piece (token cols {0:W} U {768:1024}) ----
        with tc.tile_pool(name=f"ln1b_sb{l}", bufs=1) as l1sb, \
             tc.tile_pool(name=f"ln1b_ps{l}", bufs=1, space="PSUM") as l1ps:
            _ln_T(nc, l1sb, l1ps,
                  [_two_seg(r1[ft], 0, 3 * W, W) for ft in range(FT)],
                  [_two_seg(r1[ft], 0, 3 * W, W) for ft in range(FT)],
                  ones_r, ones_row, eps_sb, ls1_sb, lb1_sb, 512,
                  stage_sbuf=True,
                  out2_aps=[_two_seg(y_bf[ft], 0, 3 * W, W)
                            for ft in range(FT)])

        # ---- FFN boundary pass (token cols {0:W} U {768:1024} of own) ----
        y_bd = [_two_seg(y_bf[ft], 0, 3 * W, W) for ft in range(FT)]
        yr_bd = [_two_seg(r1[ft], 0, 3 * W, W) for ft in range(FT)]
        _ffn_pass(nc, tc, t, l, "bd", y_bd, yr_bd, r2b, b1_sb, b2_sb)

        # ---- LN1 interior piece (DVE work overlaps FFN-bd matmuls) ----
        with tc.tile_pool(name=f"ln1i_sb{l}", bufs=1) as l1sb, \
             tc.tile_pool(name=f"ln1i_ps{l}", bufs=1, space="PSUM") as l1ps:
            _ln_T(nc, l1sb, l1ps, [r1[ft][:, W:3 * W] for ft in range(FT)],
                  [r1[ft][:, W:3 * W] for ft in range(FT)],
                  ones_r, ones_row, eps_sb, ls1_sb, lb1_sb, 512,
                  stage_sbuf=True,
                  out2_aps=[y_bf[ft][:, W:3 * W] for ft in range(FT)])

        # ---- FFN interior pass (emitted before LN2-bd: its matmuls fill
        # the PE while LN2-bd's scalar/vector chain runs) ----
        y_int = [y_bf[ft][:, W:3 * W] for ft in range(FT)]
        yr_int = [r1[ft][:, W:3 * W] for ft in range(FT)]
        _ffn_pass(nc, tc, t, l, "int", y_int, yr_int, r2i, b1_sb, b2_sb)

        with tc.tile_pool(name=f"ln2b_sb{l}", bufs=1) as l2sb, \
             tc.tile_pool(name=f"ln2b_ps{l}", bufs=1, space="PSUM") as l2ps:
            _ln_T(nc, l2sb, l2ps, [r2b[ft][:] for ft in range(FT)],
                  [_two_seg(x[ft], 0, 3 * W, W) for ft in range(FT)],
                  ones_r, ones_row, eps_sb, ls2_sb, lb2_sb, 512,
                  stage_sbuf=True,
                  out2_aps=[_two_seg(x_bf[ft], W, W + 3 * W, W)
                            for ft in range(FT)])

        # ---- halo exchange (overlaps FFN interior + next layer's QKV) ----
        if exchange:
            edram = ctx.enter_context(
                tc.tile_pool(name=f"pe_dram{l}", bufs=1, space="DRAM"))
            b_in = edram.tile([2, FT, P, W], BF16, tag="bin", name="bin")
            b_out = edram.tile([4 * 2 * FT * P, W], BF16, tag="bout",
                               name="bout")
            for ft in range(FT):
                nc.sync.dma_start(b_in[0, ft], x_bf[ft][:, W:2 * W])
                nc.sync.dma_start(b_in[1, ft], x_bf[ft][:, T_OWN:T_OWN + W])
            nc.gpsimd.collective_compute(
                "AllGather", ALU.bypass,
                replica_groups=[[0, 1, 2, 3], [4, 5, 6, 7]],
                ins=[b_in[:].opt()], outs=[b_out[:].opt()])
            for side in range(2):
                for ft in range(FT):
                    dst = (x_bf[ft][:, 0:W] if side == 0
                           else x_bf[ft][:, T_OWN + W:T_EXT])
                    nc.gpsimd.indirect_dma_start(
                        out=dst, out_offset=None, in_=b_out[:],
                        in_offset=bass.IndirectOffsetOnAxis(
                            ap=hid_sb[:, side * FT + ft:side * FT + ft + 1],
                            axis=0))

        # ---- LN2 interior (DVE overlaps next layer's QKV matmuls) ----
        with tc.tile_pool(name=f"ln2i_sb{l}", bufs=1) as l2sb, \
             tc.tile_pool(name=f"ln2i_ps{l}", bufs=1, space="PSUM") as l2ps:
            _ln_T(nc, l2sb, l2ps, [r2i[ft][:] for ft in range(FT)],
                  [x[ft][:, W:3 * W] for ft in range(FT)],
                  ones_r, ones_row, eps_sb, ls2_sb, lb2_sb, 512,
                  stage_sbuf=True,
                  out2_aps=[x_bf[ft][:, 2 * W:4 * W] for ft in range(FT)])


# ---------------- host side ----------------

def _blocked(w, n_k, n_m):
    """[n_k*128, n_m*128] -> [n_m, 128, n_k, 128] (lhsT strips by out-tile)."""
    return np.ascontiguousarray(
        w.reshape(n_k, P, n_m, P).transpose(2, 1, 0, 3))


def _bias_lay(b, n):
    return np.ascontiguousarray(b.reshape(n, P).T)


def prepare(inputs):
    """Build per-core in_maps from full inputs."""
    ids_full = np.asarray(inputs["input_ids"]).astype(np.int32)
    am = np.asarray(inputs["attention_mask"]).astype(np.int32)
    emb_word = np.asarray(inputs["emb_word"], dtype=np.float32)
    emb_pos = np.asarray(inputs["emb_pos"], dtype=np.float32)
    Wq = np.asarray(inputs["Wq"], np.float32) / np.sqrt(DH)
    bq = np.asarray(inputs["bq"], np.float32) / np.sqrt(DH)
    Wk = np.asarray(inputs["Wk"], np.float32)
    bk = np.asarray(inputs["bk"], np.float32)
    Wv = np.asarray(inputs["Wv"], np.float32)
    bv = np.asarray(inputs["bv"], np.float32)
    Wo = np.asarray(inputs["Wo"], np.float32)
    bo = np.asarray(inputs["bo"], np.float32)
    W1 = np.asarray(inputs["W1"], np.float32)
    b1 = np.asarray(inputs["b1"], np.float32)
    W2 = np.asarray(inputs["W2"], np.float32)
    b2 = np.asarray(inputs["b2"], np.float32)
    assert np.all(am == 1), "general attention_mask needs mid-tile masks too"

    shared = {
        "emb_word": emb_word,
        "eln_s": _bias_lay(np.asarray(inputs["emb_ln_s"], np.float32), FT),
        "eln_b": _bias_lay(np.asarray(inputs["emb_ln_b"], np.float32), FT),
        "wq": np.stack([_blocked(Wq[i], FT, FT) for i in range(L)]).astype(
            ml_dtypes.bfloat16),
        "wk": np.stack([_blocked(Wk[i], FT, FT) for i in range(L)]).astype(
            ml_dtypes.bfloat16),
        "wv": Wv.astype(ml_dtypes.bfloat16),
        "wo": np.stack([_blocked(Wo[i], FT, FT) for i in range(L)]).astype(
            ml_dtypes.bfloat16),
        "w1": np.stack([_blocked(W1[i], FT, FFT) for i in range(L)]).astype(
            ml_dtypes.bfloat16),
        "w2": W2.astype(ml_dtypes.bfloat16),
        "bq": np.stack([_bias_lay(bq[i], FT) for i in range(L)]),
        "bk": np.stack([_bias_lay(bk[i], FT) for i in range(L)]),
        "bo": np.stack([_bias_lay(bv[i] @ Wo[i] + bo[i], FT)
                        for i in range(L)]),
        "b1": np.stack([_bias_lay(b1[i], FFT) for i in range(L)]),
        "b2": np.stack([_bias_lay(b2[i], FT) for i in range(L)]),
        "ls1": np.stack([_bias_lay(np.asarray(inputs["ln1_s"], np.float32)[i],
                                   FT) for i in range(L)]),
        "lb1": np.stack([_bias_lay(np.asarray(inputs["ln1_b"], np.float32)[i],
                                   FT) for i in range(L)]),
        "ls2": np.stack([_bias_lay(np.asarray(inputs["ln2_s"], np.float32)[i],
                                   FT) for i in range(L)]),
        "lb2": np.stack([_bias_lay(np.asarray(inputs["ln2_b"], np.float32)[i],
                                   FT) for i in range(L)]),
    }

    in_maps = []
    i_idx = np.arange(W)
    for core in range(N_CORES):
        b, sb = core // 4, core % 4
        s0 = sb * T_OWN
        ext_pos = np.clip(np.arange(s0 - W, s0 + T_OWN + W), 0, S - 1)
        m = dict(shared)
        m["ids"] = np.ascontiguousarray(
            ids_full[b, ext_pos].reshape(12, P).T)
        m["pos"] = np.ascontiguousarray(emb_pos[ext_pos])
        # masks: global chunk gc, window key j in [0,768), query i in [0,256):
        #   key_abs = gc*W - W + j ; allowed = |j - W - i| <= W
        #             & 0 <= key_abs < S & attention_mask[b, key_abs]
        mlm = np.zeros((NCH, P, 512), np.float32)
        mrm = np.zeros((NCH, P, 512), np.float32)
        for c in range(NCH):
            gc = sb * NCH + c
            for kt2 in range(2):
                for mm_, j0 in ((mlm, 0), (mrm, 512)):
                    j = j0 + kt2 * P + np.arange(P)[:, None]
                    key_abs = gc * W - W + j
                    ok = (np.abs(j - W - i_idx[None, :]) <= W)
                    ok &= (key_abs >= 0) & (key_abs < S)
                    ok &= am[b, np.clip(key_abs, 0, S - 1)] > 0
                    mm_[c, :, kt2 * W:(kt2 + 1) * W] = ok
        m["ml"] = mlm.astype(ml_dtypes.bfloat16)
        m["mr"] = mrm.astype(ml_dtypes.bfloat16)
        # halo row ids into the gathered [4, 2, FT, 128, W] row table
        hid = np.zeros((2, FT, P), np.int64)
        for side in range(2):
            nb = sb - 1 if side == 0 else sb + 1
            if 0 <= nb <= 3:
                osd = 1 - side  # left halo <- neighbor's right block
                for ft in range(FT):
                    hid[side, ft] = ((nb * 2 + osd) * FT + ft) * P \
                        + np.arange(P)
            else:
                for ft in range(FT):
                    hid[side, ft] = ((sb * 2 + side) * FT + ft) * P \
                        + np.arange(P)
        m["halo_ids"] = np.ascontiguousarray(
            hid.reshape(12, P).T.astype(np.int32))
        in_maps.append(m)
    return in_maps


_NC_CACHE = {}


def get_nc(n_layers=L):
    if n_layers not in _NC_CACHE:
        _NC_CACHE[n_layers] = build_nc(n_layers)
    return _NC_CACHE[n_layers]


def run(inputs, n_layers=L, trace=False):
    nc = get_nc(n_layers)
    in_maps = prepare(inputs)
    res = bass_utils.run_bass_kernel_spmd(
        nc, in_maps, core_ids=list(range(N_CORES)), trace=trace)
    outs = np.empty((B, S, HD), np.float32)
    for core in range(N_CORES):
        b, sb = core // 4, core % 4
        ot = res.results[core]["out"]  # [FT, 128, T_OWN]
        outs[b, sb * T_OWN:(sb + 1) * T_OWN] = ot.reshape(HD, T_OWN).T
    return outs, res


def kernel(**inputs) -> np.ndarray:
    out, _ = run(inputs)
    return out

